# revision 1
# baseline (speedup 1.0000x reference)
"""APPNP (GCN-normalized propagation) distributed Bass kernel for 8 TRN2 cores.

Strategy (dst-sharded message passing):
  - Nodes sharded across 8 cores (6250/core, padded to 6272 = 49*128 rows).
  - Prologue per core: atom-embedding gather (dma_gather from the flattened
    [9*119, 128] table) summed over 9 feature columns -> h0acc; 3-layer MLP
    on the TensorEngine (f32); produces h (bf16, tile layout) + h0s = 0.1*h
    (bf16) per shard.
  - Per-iteration exchange: each shard is split into an A half (rows 0:3200)
    and B half (3200:6272); two AllGathers produce full bf16 h copies in two
    Shared DRAM buffers of 25600/24576 rows (< 32768, so dma_gather's int16
    indices reach everything). Buffers are double-buffered across iterations
    and AG_a is issued mid-iteration (as soon as the A-half blends finish),
    so the collectives hide behind gather/matmul work.
  - Per iteration: dma_gather h[src] for every in-edge of the core's dst
    shard (incl. self-loops, which are ordinary edges with coef
    0.9*dinv^2), edges grouped by 64-wide dst block into 128-slot tiles;
    TensorEngine segment-sum via one-hot(0.9*coef) S matrices (bf16,
    SBUF-resident) accumulating in PSUM; h0s injected into PSUM with an
    identity matmul; the scalar engine evicts PSUM -> h (bf16) / out (f32).
    The vector engine does no per-edge work at all.
  - Per-block/domain tile counts are equalized across cores so all 8 cores
    run one SPMD instruction stream (padding tiles gather idx 0 with S=0).

kernel(**inputs) takes FULL inputs, shards on host, runs the NEFF on cores
0-7, and returns the full [50000, 128] f32 output.
"""

import numpy as np
import ml_dtypes

import concourse.bacc as bacc
import concourse.bass as bass
import concourse.mybir as mybir
import concourse.tile as tile
from concourse.bass_utils import run_bass_kernel_spmd

# Problem constants (hardcoded; must match reference.py)
N_NODES = 50000
N_EDGES = 800000
D = 128
NUM_ITER = 10
NUM_LAYER = 3
ALPHA = 0.1
NUM_ATOM_FEATS = 9
ATOM_VOCAB = 119

NC = 8
SHARD = N_NODES // NC            # 6250
SHARD_PAD = 6272                 # 49 * 128
NCOL = SHARD_PAD // 128          # 49
W = 64                           # dst block width
NBLK = SHARD_PAD // W            # 98
CHUNK = 1024                     # max idxs per dma_gather
HALF_A_ROWS = 3200               # shard rows in exchange buffer A (25 cols)
HALF_B_ROWS = 3072               # shard rows in exchange buffer B (24 cols)
N_A = NC * HALF_A_ROWS           # 25600 (< 32768 -> int16 safe)
N_B = NC * HALF_B_ROWS           # 24576 (< 32768 -> int16 safe)
COLS_A = HALF_A_ROWS // 128      # 25
NQ = 4                           # SWDGE queues

BF16 = mybir.dt.bfloat16
F32 = mybir.dt.float32
I16 = mybir.dt.int16
AF = mybir.ActivationFunctionType


def _wrap_idxs(idx):
    """slot i -> partition i%16 (replicated x8), col i//16."""
    n = idx.shape[0]
    assert n % 16 == 0
    w = idx.reshape(n // 16, 16).T.astype(np.int16)
    return np.ascontiguousarray(np.tile(w, (8, 1)))


def _pad128(a, fill=0):
    n = a.shape[0]
    m = (-n) % 128
    if m == 0:
        return a
    return np.concatenate([a, np.full((m,) + a.shape[1:], fill, a.dtype)])


def _preprocess(edge_index):
    """Host-side graph preprocessing -> per-core structures."""
    src = np.asarray(edge_index[0], dtype=np.int64)
    dst = np.asarray(edge_index[1], dtype=np.int64)
    deg = np.bincount(dst, minlength=N_NODES).astype(np.float64) + 1.0
    dinv = 1.0 / np.sqrt(deg)
    coef = ((1.0 - ALPHA) * dinv[src] * dinv[dst]).astype(np.float32)
    dinv2 = ((1.0 - ALPHA) * dinv * dinv).astype(np.float32)  # self-loop term
    rank = src // SHARD
    r = src % SHARD
    isa = r < HALF_A_ROWS
    srow = np.where(isa, rank * HALF_A_ROWS + r,
                    rank * HALF_B_ROWS + (r - HALF_A_ROWS))

    cores = []
    for c in range(NC):
        m = (dst >= c * SHARD) & (dst < (c + 1) * SHARD)
        nodes = np.arange(SHARD)
        self_isa = nodes < HALF_A_ROWS
        self_row = np.where(self_isa, c * HALF_A_ROWS + nodes,
                            c * HALF_B_ROWS + (nodes - HALF_A_ROWS))
        ldst = np.concatenate([(dst[m] - c * SHARD).astype(np.int64), nodes])
        lsrow = np.concatenate([srow[m], self_row])
        lcoef = np.concatenate([coef[m], dinv2[c * SHARD + nodes]])
        blk = ldst // W
        off = ldst % W
        islo = np.concatenate([isa[m], self_isa])

        streams = {"lo": [], "hi": []}     # list of idx arrays
        s_tiles = []                       # list of [128, W] f32 tile mats
        tiles_by_block = [[] for _ in range(NBLK)]  # (dom, tile_pos_in_stream)
        stream_ntiles = {"lo": 0, "hi": 0}
        for b in range(NBLK):
            bm = blk == b
            for dom, dm in (("lo", islo), ("hi", ~islo)):
                sel = bm & dm
                n = int(sel.sum())
                if n == 0:
                    continue
                idx = _pad128(lsrow[sel].astype(np.int64))
                cf = _pad128(lcoef[sel])
                of = _pad128(off[sel].astype(np.int64))
                ntile = idx.shape[0] // 128
                for t in range(ntile):
                    s = np.zeros((128, W), np.float32)
                    s[np.arange(128), of[t * 128:(t + 1) * 128]] = \
                        cf[t * 128:(t + 1) * 128]
                    tiles_by_block[b].append((dom, stream_ntiles[dom] + t,
                                              len(s_tiles)))
                    s_tiles.append(s)
                streams[dom].append(idx)
                stream_ntiles[dom] += ntile

        lo_idx = (np.concatenate(streams["lo"]) if streams["lo"]
                  else np.zeros(0, np.int64))
        hi_idx = (np.concatenate(streams["hi"]) if streams["hi"]
                  else np.zeros(0, np.int64))
        s_all = (np.stack(s_tiles) if s_tiles
                 else np.zeros((0, 128, W), np.float32))
        # S as SBUF layout [128, ntiles*W]
        s_sb = np.ascontiguousarray(
            s_all.transpose(1, 0, 2).reshape(128, -1)).astype(ml_dtypes.bfloat16)
        cores.append(dict(
            lo_idx=lo_idx, hi_idx=hi_idx, s_sb=s_sb,
            tiles_by_block=tiles_by_block,
            n_lo=lo_idx.shape[0], n_hi=hi_idx.shape[0],
            ntiles=len(s_tiles),
        ))
    return cores


def _chunks(total):
    """Split a stream of `total` slots (multiple of 128) into <=1024 chunks."""
    out = []
    o = 0
    while o < total:
        n = min(CHUNK, total - o)
        out.append((o, n))
        o += n
    return out


def _equalize(cores_meta):
    """Pad per-block/domain tile counts to the max across cores so all cores
    share one instruction stream. Padding tiles gather idx 0 with S=0."""
    # per core: per block, per dom tile count
    cnt = np.zeros((NC, NBLK, 2), np.int64)
    for c, m in enumerate(cores_meta):
        for b in range(NBLK):
            for dom, tpos, sidx in m["tiles_by_block"][b]:
                cnt[c, b, 0 if dom == "lo" else 1] += 1
    mx = cnt.max(axis=0)  # [NBLK, 2]

    new = []
    for c, m in enumerate(cores_meta):
        lo_parts, hi_parts, s_parts = [], [], []
        tiles_by_block = [[] for _ in range(NBLK)]
        lo_idx, hi_idx = m["lo_idx"], m["hi_idx"]
        # existing tiles grouped by block/dom in stream order
        ptr = {"lo": 0, "hi": 0}
        sidx_of = {}
        for b in range(NBLK):
            for dom, tpos, sidx in m["tiles_by_block"][b]:
                sidx_of[(b, dom, tpos)] = sidx
        s_all = m["s_sb"].reshape(128, -1, W)
        lo_nt, hi_nt = 0, 0
        s_n = 0
        for b in range(NBLK):
            for di, dom in enumerate(("lo", "hi")):
                have = [t for t in m["tiles_by_block"][b] if t[0] == dom]
                need = int(mx[b, di])
                for k in range(need):
                    if k < len(have):
                        _, tpos, sidx = have[k]
                        idx_arr = (lo_idx if dom == "lo" else hi_idx)[
                            tpos * 128:(tpos + 1) * 128]
                        s_mat = s_all[:, sidx, :]
                    else:
                        idx_arr = np.zeros(128, np.int64)
                        s_mat = np.zeros((128, W), ml_dtypes.bfloat16)
                    (lo_parts if dom == "lo" else hi_parts).append(idx_arr)
                    s_parts.append(np.asarray(s_mat))
                    nt = lo_nt if dom == "lo" else hi_nt
                    tiles_by_block[b].append((dom, nt, s_n))
                    s_n += 1
                    if dom == "lo":
                        lo_nt += 1
                    else:
                        hi_nt += 1
        lo_cat = (np.concatenate(lo_parts) if lo_parts
                  else np.zeros(0, np.int64))
        hi_cat = (np.concatenate(hi_parts) if hi_parts
                  else np.zeros(0, np.int64))
        s_cat = (np.stack(s_parts) if s_parts
                 else np.zeros((0, 128, W), ml_dtypes.bfloat16))
        s_sb = np.ascontiguousarray(
            np.asarray(s_cat).transpose(1, 0, 2).reshape(128, -1))
        new.append(dict(
            lo_idx=lo_cat, hi_idx=hi_cat, s_sb=s_sb,
            tiles_by_block=tiles_by_block,
            n_lo=lo_cat.shape[0], n_hi=hi_cat.shape[0], ntiles=s_n,
        ))
    return new


def _build_uniform(meta0):
    """Build the (identical-across-cores) program from equalized metadata."""
    n_lo, n_hi, ntiles = meta0["n_lo"], meta0["n_hi"], meta0["ntiles"]
    tiles_by_block = meta0["tiles_by_block"]
    n_emb = NUM_ATOM_FEATS * SHARD_PAD

    nc = bacc.Bacc("TRN2", target_bir_lowering=False, debug=False,
                   num_devices=NC, num_swdge_queues=NQ)

    emb = nc.dram_tensor("emb", [NUM_ATOM_FEATS * ATOM_VOCAB, D], F32,
                         kind="ExternalInput")
    emb_idx = nc.dram_tensor("emb_idx", [128, n_emb // 16], I16,
                             kind="ExternalInput")
    ws = nc.dram_tensor("ws", [NUM_LAYER * D, D], F32, kind="ExternalInput")
    bs = nc.dram_tensor("bs", [NUM_LAYER, D], F32, kind="ExternalInput")
    ident = nc.dram_tensor("ident", [128, 128], F32, kind="ExternalInput")
    idx_lo_d = nc.dram_tensor("idx_lo", [128, max(n_lo, 16) // 16], I16,
                              kind="ExternalInput")
    idx_hi_d = nc.dram_tensor("idx_hi", [128, max(n_hi, 16) // 16], I16,
                              kind="ExternalInput")
    s_d = nc.dram_tensor("s", [128, max(ntiles, 1) * W], BF16,
                         kind="ExternalInput")
    identb_d = nc.dram_tensor("identb", [128, 128], BF16,
                              kind="ExternalInput")
    out_d = nc.dram_tensor("out", [SHARD_PAD, D], F32, kind="ExternalOutput")

    ag_in_a = [nc.dram_tensor(f"ag_in_a{i}", [HALF_A_ROWS, D], BF16,
                              kind="Internal") for i in range(2)]
    ag_in_b = [nc.dram_tensor(f"ag_in_b{i}", [HALF_B_ROWS, D], BF16,
                              kind="Internal") for i in range(2)]
    ag_out_a = [nc.dram_tensor(f"ag_out_a{i}", [N_A, D], BF16,
                               kind="Internal", addr_space="Shared")
                for i in range(2)]
    ag_out_b = [nc.dram_tensor(f"ag_out_b{i}", [N_B, D], BF16,
                               kind="Internal", addr_space="Shared")
                for i in range(2)]

    def _emit_ag(which, buf):
        src = (ag_in_a if which == "a" else ag_in_b)[buf]
        dst = (ag_out_a if which == "a" else ag_out_b)[buf]
        nc.gpsimd.collective_compute(
            "AllGather", mybir.AluOpType.bypass,
            replica_groups=[list(range(NC))],
            ins=[src[:].opt()], outs=[dst[:].opt()])

    lo_chunks = _chunks(n_lo)
    hi_chunks = _chunks(n_hi)

    with tile.TileContext(nc) as tc:
      with tc.tile_pool(name="persist", bufs=1) as persist:
        # ---------------- prologue: embedding + MLP ----------------
        with (
            tc.tile_pool(name="pro", bufs=1) as pro,
            tc.tile_pool(name="embg", bufs=12) as embg,
            tc.tile_pool(name="prps", bufs=2, space="PSUM") as prps,
        ):
            h0acc = pro.tile([128, NCOL, D], F32, tag="h0acc")
            eidx = pro.tile([128, n_emb // 16], I16, tag="eidx")
            nc.sync.dma_start(eidx[:], emb_idx[:])
            idn = pro.tile([128, 128], F32, tag="idn")
            nc.sync.dma_start(idn[:], ident[:])
            # Sum the 9 per-feature embedding gathers on the TensorEngine
            # (identity-stationary matmuls accumulating in PSUM), chunk-major
            # so at most a couple of PSUM banks are live.
            qn = 0
            with tc.tile_pool(name="embp", bufs=4, space="PSUM") as embp:
                for (o, n) in _chunks(SHARD_PAD):
                    gs = []
                    for f in range(NUM_ATOM_FEATS):
                        g = embg.tile([128, 8, D], F32, tag="eg")
                        so = f * SHARD_PAD + o
                        nc.gpsimd.dma_gather(
                            g[:, 0:n // 128, :], emb[:, :],
                            eidx[:, so // 16:(so + n) // 16], n, n, D,
                            queue_num=qn % NQ)
                        qn += 1
                        gs.append(g)
                    for half in range(0, n // 128, 4):
                        w = min(4, n // 128 - half)
                        ep = embp.tile([128, 512], F32, tag="ep")
                        for f in range(NUM_ATOM_FEATS):
                            nc.tensor.matmul(
                                ep[:, 0:w * 128],
                                idn[:, :],
                                gs[f][:, half:half + w, :],
                                start=(f == 0),
                                stop=(f == NUM_ATOM_FEATS - 1))
                        nc.scalar.activation(
                            h0acc[:, o // 128 + half:o // 128 + half + w, :],
                            ep[:, 0:w * 128].rearrange(
                                "p (a b) -> p a b", b=128), AF.Copy)
            w_sb = pro.tile([128, NUM_LAYER * D], F32, tag="w")
            b_sb = pro.tile([128, NUM_LAYER], F32, tag="b")
            for l in range(NUM_LAYER):
                nc.sync.dma_start(w_sb[:, l * D:(l + 1) * D],
                                  ws[l * D:(l + 1) * D, :])
                nc.sync.dma_start(b_sb[:, l:l + 1],
                                  bs[l:l + 1, :].rearrange("a k -> k a"))

            hT = pro.tile([128, SHARD_PAD], F32, tag="hT")
            hT2 = pro.tile([128, SHARD_PAD], F32, tag="hT2")
            # transpose h0acc tiles into hT
            for cidx in range(NCOL):
                pt = prps.tile([128, 128], F32, tag="pt")
                nc.tensor.transpose(pt[:], h0acc[:, cidx, :], idn[:])
                nc.vector.tensor_copy(hT[:, cidx * 128:(cidx + 1) * 128],
                                      pt[:])
            # 3 MLP layers (f32)
            cur, nxt = hT, hT2
            mlp_chunks = [(o, min(512, SHARD_PAD - o))
                          for o in range(0, SHARD_PAD, 512)]
            for l in range(NUM_LAYER):
                for (o, n) in mlp_chunks:
                    ps = prps.tile([128, 512], F32, tag="mlp")
                    nc.tensor.matmul(ps[:, 0:n],
                                     w_sb[:, l * D:(l + 1) * D],
                                     cur[:, o:o + n], start=True, stop=True)
                    nc.scalar.activation(
                        nxt[:, o:o + n], ps[:, 0:n],
                        AF.Relu if l != NUM_LAYER - 1 else AF.Identity,
                        bias=b_sb[:, l:l + 1])
                cur, nxt = nxt, cur

            # transpose back; produce h (bf16) and h0s = 0.1*h (f32)
            h_sb = persist.tile([128, NCOL, D], BF16, tag="h")
            h0s = persist.tile([128, NCOL, D], BF16, tag="h0s")
            for cidx in range(NCOL):
                pt = prps.tile([128, 128], F32, tag="pt")
                nc.tensor.transpose(pt[:], cur[:, cidx * 128:(cidx + 1) * 128],
                                    idn[:])
                nc.vector.tensor_copy(h_sb[:, cidx, :], pt[:])
                nc.scalar.activation(h0s[:, cidx, :], pt[:], AF.Copy,
                                     scale=ALPHA)
            nc.sync.dma_start(
                ag_in_a[0][:].rearrange("(c p) f -> p c f", p=128),
                h_sb[:, 0:COLS_A, :])
            nc.sync.dma_start(
                ag_in_b[0][:].rearrange("(c p) f -> p c f", p=128),
                h_sb[:, COLS_A:NCOL, :])

        # ---------------- main loop ----------------
        with (
            tc.tile_pool(name="sconst", bufs=1) as sconst,
            tc.tile_pool(name="glo", bufs=10) as glo_pool,
            tc.tile_pool(name="ghi", bufs=6) as ghi_pool,
            tc.tile_pool(name="ps", bufs=8, space="PSUM") as ps_pool,
            tc.tile_pool(name="stage", bufs=4) as stage_pool,
        ):
            s_sb = sconst.tile([128, max(ntiles, 1) * W], BF16, tag="s")
            nc.sync.dma_start(s_sb[:], s_d[:])
            ilo = sconst.tile([128, max(n_lo, 16) // 16], I16, tag="ilo")
            nc.sync.dma_start(ilo[:], idx_lo_d[:])
            ihi = sconst.tile([128, max(n_hi, 16) // 16], I16, tag="ihi")
            nc.sync.dma_start(ihi[:], idx_hi_d[:])
            identb = sconst.tile([128, 128], BF16, tag="identb")
            nc.sync.dma_start(identb[:], identb_d[:])

            _emit_ag("a", 0)
            _emit_ag("b", 0)

            for it in range(NUM_ITER):
                buf = it % 2
                lo_view = ag_out_a[buf][:, :]
                hi_view = ag_out_b[buf][:, :]

                # issue gathers, interleaved a/b
                lo_tiles_bufs = {}
                hi_tiles_bufs = {}
                qn = 0
                li, hi_i = 0, 0
                order = []
                while li < len(lo_chunks) or hi_i < len(hi_chunks):
                    if li < len(lo_chunks):
                        order.append(("lo", li)); li += 1
                    if hi_i < len(hi_chunks):
                        order.append(("hi", hi_i)); hi_i += 1
                for dom, ci in order:
                    (o, n) = (lo_chunks if dom == "lo" else hi_chunks)[ci]
                    pool = glo_pool if dom == "lo" else ghi_pool
                    view = lo_view if dom == "lo" else hi_view
                    isb = ilo if dom == "lo" else ihi
                    g = pool.tile([128, 8, D], BF16, tag="g" + dom)
                    nc.gpsimd.dma_gather(
                        g[:, 0:n // 128, :], view,
                        isb[:, o // 16:(o + n) // 16], n, n, D,
                        queue_num=qn % NQ)
                    qn += 1
                    (lo_tiles_bufs if dom == "lo" else hi_tiles_bufs)[ci] = g

                # segment-sum matmuls + evict, block-pair by block-pair
                last = it == NUM_ITER - 1
                for p in range(NBLK // 2):
                    col = p
                    psum = ps_pool.tile([128, D], F32, tag="ps")
                    for half in range(2):
                        tl = tiles_by_block[2 * p + half]
                        ph = half * 64
                        # inject h0s (= 0.1*h0) via identity matmul
                        nc.tensor.matmul(
                            psum[ph:ph + 64, :],
                            identb[:, ph:ph + 64],
                            h0s[:, col, :],
                            start=True, stop=(len(tl) == 0))
                        for j, (dom, tpos, sidx) in enumerate(tl):
                            bufs = (lo_tiles_bufs if dom == "lo"
                                    else hi_tiles_bufs)
                            g = bufs[tpos // 8]
                            nc.tensor.matmul(
                                psum[ph:ph + 64, :],
                                s_sb[:, sidx * W:(sidx + 1) * W],
                                g[:, tpos % 8, :],
                                start=False, stop=(j == len(tl) - 1))
                    if last:
                        st = stage_pool.tile([128, D], F32, tag="st")
                        nc.scalar.activation(st[:], psum[:, :], AF.Copy)
                        nc.sync.dma_start(
                            out_d[p * 128:(p + 1) * 128, :], st[:])
                    else:
                        nc.scalar.activation(h_sb[:, col, :], psum[:, :],
                                             AF.Copy)
                    if not last:
                        if p == COLS_A - 1:
                            nc.sync.dma_start(
                                ag_in_a[1 - buf][:].rearrange(
                                    "(c p) f -> p c f", p=128),
                                h_sb[:, 0:COLS_A, :])
                            _emit_ag("a", 1 - buf)
                        elif p == NBLK // 2 - 1:
                            nc.sync.dma_start(
                                ag_in_b[1 - buf][:].rearrange(
                                    "(c p) f -> p c f", p=128),
                                h_sb[:, COLS_A:NCOL, :])
                            _emit_ag("b", 1 - buf)

    nc.compile()
    return nc


_CACHE = {}


def _get_compiled(edge_index):
    key = hash(np.asarray(edge_index).tobytes())
    if key not in _CACHE:
        cores = _preprocess(edge_index)
        cores = _equalize(cores)
        nc = _build_uniform(cores[0])
        _CACHE[key] = (nc, cores)
    return _CACHE[key]


def _make_in_maps(x, atom_emb, Ws, bs, cores_meta):
    x = np.asarray(x)
    emb_t = np.ascontiguousarray(
        np.asarray(atom_emb, dtype=np.float32).reshape(
            NUM_ATOM_FEATS * ATOM_VOCAB, D))
    ws_t = np.ascontiguousarray(
        np.asarray(Ws, dtype=np.float32).reshape(NUM_LAYER * D, D))
    bs_t = np.ascontiguousarray(np.asarray(bs, dtype=np.float32))
    ident = np.eye(128, dtype=np.float32)

    in_maps = []
    for c, m in enumerate(cores_meta):
        # embedding idx: per feature stream of SHARD_PAD slots
        ei = np.zeros(NUM_ATOM_FEATS * SHARD_PAD, np.int64)
        xs = x[c * SHARD:(c + 1) * SHARD]  # [SHARD, 9]
        for f in range(NUM_ATOM_FEATS):
            ei[f * SHARD_PAD:f * SHARD_PAD + SHARD] = \
                f * ATOM_VOCAB + xs[:, f]
        lo = m["lo_idx"] if m["n_lo"] else np.zeros(16, np.int64)
        hi_ = m["hi_idx"] if m["n_hi"] else np.zeros(16, np.int64)
        in_maps.append({
            "emb": emb_t,
            "emb_idx": _wrap_idxs(ei),
            "ws": ws_t,
            "bs": bs_t,
            "ident": ident,
            "idx_lo": _wrap_idxs(lo),
            "idx_hi": _wrap_idxs(hi_),
            "s": np.ascontiguousarray(m["s_sb"]),
            "identb": np.eye(128, dtype=ml_dtypes.bfloat16),
        })
    return in_maps


def kernel(x, edge_index, atom_emb, Ws, bs):
    nc, cores_meta = _get_compiled(edge_index)
    in_maps = _make_in_maps(x, atom_emb, Ws, bs, cores_meta)
    res = run_bass_kernel_spmd(nc, in_maps, core_ids=list(range(NC)))
    out = np.concatenate(
        [res.results[c]["out"][:SHARD] for c in range(NC)], axis=0)
    return np.ascontiguousarray(out.astype(np.float32))


def run_profiled(x, edge_index, atom_emb, Ws, bs):
    """Like kernel() but with NTFF profiling; returns (out, exec_time_ns)."""
    import ntff_hook
    ntff_hook.install()
    nc, cores_meta = _get_compiled(edge_index)
    in_maps = _make_in_maps(x, atom_emb, Ws, bs, cores_meta)
    res = run_bass_kernel_spmd(nc, in_maps, core_ids=list(range(NC)),
                               trace=True)
    out = np.concatenate(
        [res.results[c]["out"][:SHARD] for c in range(NC)], axis=0)
    return np.ascontiguousarray(out.astype(np.float32)), res.exec_time_ns



# revision 2
# speedup vs baseline: 1.0102x; 1.0102x over previous
"""APPNP (GCN-normalized propagation) distributed Bass kernel for 8 TRN2 cores.

v2 strategy (dst-sharded message passing, gather-descriptor-optimized):
  - Nodes sharded across 8 cores (6250/core, padded to 6272 = 49*128 rows).
  - Per-core node->row permutation balances per-(block,dom) in-edge counts
    across cores so SPMD equalization padding is minimal.
  - Prologue: atom embedding via one-hot matmuls (stationary = padded
    embedding table, moving = host-built one-hot of x) directly producing
    hT; 3-layer MLP in transposed space; PE transpose back -> h (bf16),
    h0s = 0.1*h.
  - Exchange: two AllGathers per iteration into DRAM tables
    a (rows 0:2560/core -> 20480 rows) and b (rows 2560:6272 -> 29696 rows),
    both < 32768 so int16 gather indices reach everything. a is issued
    mid-iteration (after block-pair 19), b at the end; the next iteration
    issues ~40 a-sourced chunks first so b's wire time is absorbed.
  - Per iteration: dma_gather h[src] for in-edges grouped in 64-wide dst
    blocks (128-slot tiles), TensorEngine segment-sum via one-hot S
    matrices (bf16, SBUF-resident) in PSUM; h0s injected via identity
    matmul; self-loops are NOT slots: the Vector engine evicts PSUM with
    h_new = selfw * h_old + psum in one scalar_tensor_tensor op.
"""

import numpy as np
import ml_dtypes

import concourse.bacc as bacc
import concourse.bass as bass
import concourse.mybir as mybir
import concourse.tile as tile
from concourse.bass_utils import run_bass_kernel_spmd
from concourse.instruction_name_ordered_set import InstructionNameOrderedSet

# Problem constants (must match reference.py)
N_NODES = 50000
N_EDGES = 800000
D = 128
NUM_ITER = 10
NUM_LAYER = 3
ALPHA = 0.1
NUM_ATOM_FEATS = 9
ATOM_VOCAB = 119

NC = 8
SHARD = N_NODES // NC            # 6250
SHARD_PAD = 6272                 # 49 * 128
NCOL = SHARD_PAD // 128          # 49
W = 64                           # dst block width
NBLK = SHARD_PAD // W            # 98
CHUNK = 1024                     # max idxs per dma_gather
COLS_A = 30                      # shard cols in exchange table a
ROWS_A = COLS_A * 128            # 3840
ROWS_B = SHARD_PAD - ROWS_A      # 2432
N_A = NC * ROWS_A                # 30720 (< 32768)
N_B = NC * ROWS_B                # 19456 (< 32768)
NQ = 4                           # SWDGE queues
ABSORB = 0                       # lo chunks issued before first hi chunk
CHAIN = False                    # chain gather emission order

BF16 = mybir.dt.bfloat16
F32 = mybir.dt.float32
I16 = mybir.dt.int16
AF = mybir.ActivationFunctionType


def _wrap_idxs(idx):
    """slot i -> partition i%16 (replicated x8), col i//16."""
    n = idx.shape[0]
    assert n % 16 == 0
    w = idx.reshape(n // 16, 16).T.astype(np.int16)
    return np.ascontiguousarray(np.tile(w, (8, 1)))


def _pad128(a, fill=0):
    n = a.shape[0]
    m = (-n) % 128
    if m == 0:
        return a
    return np.concatenate([a, np.full((m,) + a.shape[1:], fill, a.dtype)])


def _balance_perm(indeg):
    """Greedy LPT: assign 6250 local nodes to 98 blocks of <=64 nodes,
    balancing total in-degree per block. Returns node_of_row[6272] with -1
    for pad rows (all pads in the last block)."""
    import heapq
    order = np.argsort(-indeg, kind="stable")
    cap = np.full(NBLK, 64, np.int64)
    cap[NBLK - 1] = SHARD - 64 * (NBLK - 1)  # 42 real nodes in last block
    fill = [[] for _ in range(NBLK)]
    heap = [(0, b) for b in range(NBLK)]
    heapq.heapify(heap)
    for nid in order:
        while True:
            tot, b = heapq.heappop(heap)
            if len(fill[b]) < cap[b]:
                break
        fill[b].append(nid)
        if len(fill[b]) < cap[b]:
            heapq.heappush(heap, (tot + int(indeg[nid]), b))
    node_of_row = np.full(SHARD_PAD, -1, np.int64)
    for b in range(NBLK):
        for j, nid in enumerate(fill[b]):
            node_of_row[b * 64 + j] = nid
    return node_of_row


def _preprocess(edge_index):
    """Host-side graph preprocessing -> per-core structures."""
    src = np.asarray(edge_index[0], dtype=np.int64)
    dst = np.asarray(edge_index[1], dtype=np.int64)
    deg = np.bincount(dst, minlength=N_NODES).astype(np.float64) + 1.0
    dinv = 1.0 / np.sqrt(deg)
    coef = ((1.0 - ALPHA) * dinv[src] * dinv[dst]).astype(np.float32)
    selfw_g = ((1.0 - ALPHA) * dinv * dinv).astype(np.float32)

    # pass A: per-core balanced permutation (total in-degree)
    indeg_all = np.bincount(dst, minlength=N_NODES)
    node_of_row = np.zeros((NC, SHARD_PAD), np.int64)
    row_of_node = np.zeros(N_NODES, np.int64)  # global node -> local row
    for c in range(NC):
        nr = _balance_perm(indeg_all[c * SHARD:(c + 1) * SHARD])
        node_of_row[c] = nr
        valid = nr >= 0
        row_of_node[c * SHARD + nr[valid]] = np.nonzero(valid)[0]

    # pass B: with src sides fixed by pass A, pack nodes into blocks so each
    # (block, dom) in-edge count lands just UNDER a multiple of 128 (the
    # gather-tile quantum) and aligns across cores. Targets are global (the
    # max core's totals) so SPMD equalization adds almost nothing.
    src_isa = (row_of_node[src] % SHARD_PAD) < ROWS_A
    lo_in = np.bincount(dst[src_isa], minlength=N_NODES)
    hi_in = np.bincount(dst[~src_isa], minlength=N_NODES)
    RESID = 104  # target residue mod 128 (margin 24 to the next tile)

    def _targets(total_max, nb):
        base_q = max(0, int((total_max / nb - RESID) // 128))
        t = np.full(nb, base_q * 128 + RESID, np.float64)
        k = 0
        while t.sum() < total_max and k < nb:
            t[k] += 128
            k += 1
        while t.sum() < total_max:
            t += 128
        return t

    side_meta = []
    for side in (0, 1):
        if side == 0:
            blocks = list(range(0, ROWS_A // W))
        else:
            blocks = list(range(ROWS_A // W, NBLK))
        lmax = hmax = 0.0
        for c in range(NC):
            rows0 = 0 if side == 0 else ROWS_A
            nrows = ROWS_A if side == 0 else ROWS_B
            nodes = node_of_row[c][rows0:rows0 + nrows]
            nodes = nodes[nodes >= 0]
            lmax = max(lmax, lo_in[c * SHARD + nodes].sum())
            hmax = max(hmax, hi_in[c * SHARD + nodes].sum())
        nb = len(blocks)
        side_meta.append((blocks, _targets(lmax, nb), _targets(hmax, nb)))

    for c in range(NC):
        nr_new = np.full(SHARD_PAD, -1, np.int64)
        for side in (0, 1):
            blocks, T_lo, T_hi = side_meta[side]
            rows0 = 0 if side == 0 else ROWS_A
            nrows = ROWS_A if side == 0 else ROWS_B
            old_nodes = node_of_row[c][rows0:rows0 + nrows]
            old_nodes = old_nodes[old_nodes >= 0]
            li = lo_in[c * SHARD + old_nodes].astype(np.float64)
            hi_ = hi_in[c * SHARD + old_nodes].astype(np.float64)
            nb = len(blocks)
            caps = np.array([64 if b != NBLK - 1 else
                             SHARD - 64 * (NBLK - 1) for b in blocks])
            order_n = np.argsort(-(li + hi_), kind="stable")
            cur = np.zeros((nb, 2))
            cnt = np.zeros(nb, np.int64)
            assign = np.zeros(len(old_nodes), np.int64)
            for j in order_n:
                cost = np.maximum((cur[:, 0] + li[j]) / T_lo,
                                  (cur[:, 1] + hi_[j]) / T_hi)
                cost[cnt >= caps] = np.inf
                bsel = int(np.argmin(cost))
                assign[j] = bsel
                cur[bsel, 0] += li[j]
                cur[bsel, 1] += hi_[j]
                cnt[bsel] += 1
            # swap refinement: push overshoot (beyond targets) to zero
            def over(cb):
                return (max(0.0, cb[0]) + max(0.0, cb[1]))
            ex = cur - np.stack([T_lo, T_hi], axis=1)
            rng = np.random.default_rng(c)
            for _ in range(4):
                bad = np.nonzero((ex[:, 0] > 0) | (ex[:, 1] > 0))[0]
                if bad.size == 0:
                    break
                improved = False
                for b1 in bad:
                    js = np.nonzero(assign == b1)[0]
                    cands = rng.permutation(nb)[:20]
                    done = False
                    for b2 in cands:
                        if b2 == b1:
                            continue
                        for j1 in js[np.argsort(-(li[js] + hi_[js]))][:12]:
                            js2 = np.nonzero(assign == b2)[0]
                            if js2.size == 0:
                                continue
                            d1 = np.array([li[j1], hi_[j1]])
                            base = (over(ex[b1]) + over(ex[b2]))
                            d2s = np.stack([li[js2], hi_[js2]], axis=1)
                            nb1 = ex[b1] - d1 + d2s
                            nb2 = ex[b2] + d1 - d2s
                            costs = (np.maximum(nb1, 0).sum(axis=1) +
                                     np.maximum(nb2, 0).sum(axis=1))
                            kk = int(np.argmin(costs))
                            if costs[kk] < base - 0.5:
                                j2 = js2[kk]
                                ex[b1] = nb1[kk]
                                ex[b2] = nb2[kk]
                                cur[b1] += d2s[kk] - d1
                                cur[b2] += d1 - d2s[kk]
                                assign[j1], assign[j2] = b2, b1
                                improved = True
                                done = True
                                break
                        if done:
                            break
                if not improved:
                    break
            for bi, b in enumerate(blocks):
                nodes_b = old_nodes[assign == bi]
                for j2, nid in enumerate(nodes_b):
                    nr_new[b * 64 + j2] = nid
        node_of_row[c] = nr_new
        valid = nr_new >= 0
        row_of_node[c * SHARD + nr_new[valid]] = np.nonzero(valid)[0]

    # edge srow (exchange-table row of the source)
    src_core = src // SHARD
    r = row_of_node[src]
    isa = r < ROWS_A
    srow = np.where(isa, src_core * ROWS_A + r,
                    src_core * ROWS_B + (r - ROWS_A))

    cores = []
    for c in range(NC):
        m = (dst >= c * SHARD) & (dst < (c + 1) * SHARD)
        ldr = row_of_node[dst[m]]  # local row of each in-edge's dst
        lsrow = srow[m]
        lcoef = coef[m]
        lisa = isa[m]
        blk = ldr // W
        off = ldr % W

        streams = {"lo": [], "hi": []}
        s_tiles = []
        tiles_by_block = [[] for _ in range(NBLK)]
        stream_ntiles = {"lo": 0, "hi": 0}
        for b in range(NBLK):
            bm = blk == b
            for dom, dm in (("lo", lisa), ("hi", ~lisa)):
                sel = bm & dm
                n = int(sel.sum())
                if n == 0:
                    continue
                idx = _pad128(lsrow[sel].astype(np.int64))
                cf = _pad128(lcoef[sel])
                of = _pad128(off[sel].astype(np.int64))
                ntile = idx.shape[0] // 128
                for t in range(ntile):
                    s = np.zeros((128, W), np.float32)
                    s[np.arange(128), of[t * 128:(t + 1) * 128]] = \
                        cf[t * 128:(t + 1) * 128]
                    tiles_by_block[b].append((dom, stream_ntiles[dom] + t,
                                              len(s_tiles)))
                    s_tiles.append(s)
                streams[dom].append(idx)
                stream_ntiles[dom] += ntile

        lo_idx = (np.concatenate(streams["lo"]) if streams["lo"]
                  else np.zeros(0, np.int64))
        hi_idx = (np.concatenate(streams["hi"]) if streams["hi"]
                  else np.zeros(0, np.int64))
        s_all = (np.stack(s_tiles) if s_tiles
                 else np.zeros((0, 128, W), np.float32))
        s_sb = np.ascontiguousarray(
            s_all.transpose(1, 0, 2).reshape(128, -1)).astype(ml_dtypes.bfloat16)
        # selfw per row [128, NCOL]
        sw = np.zeros(SHARD_PAD, np.float32)
        nr = node_of_row[c]
        valid = nr >= 0
        sw[valid] = selfw_g[c * SHARD + nr[valid]]
        cores.append(dict(
            lo_idx=lo_idx, hi_idx=hi_idx, s_sb=s_sb,
            tiles_by_block=tiles_by_block,
            n_lo=lo_idx.shape[0], n_hi=hi_idx.shape[0],
            ntiles=len(s_tiles),
            node_of_row=node_of_row[c],
            selfw=np.ascontiguousarray(
                sw.reshape(NCOL, 128).T),  # [128, NCOL]
        ))
    return cores


def _chunks(total):
    out = []
    o = 0
    while o < total:
        n = min(CHUNK, total - o)
        out.append((o, n))
        o += n
    return out


def _equalize(cores_meta):
    """Pad per-block/dom tile counts to the max across cores (SPMD)."""
    cnt = np.zeros((NC, NBLK, 2), np.int64)
    for c, m in enumerate(cores_meta):
        for b in range(NBLK):
            for dom, tpos, sidx in m["tiles_by_block"][b]:
                cnt[c, b, 0 if dom == "lo" else 1] += 1
    mx = cnt.max(axis=0)

    new = []
    for c, m in enumerate(cores_meta):
        lo_parts, hi_parts, s_parts = [], [], []
        tiles_by_block = [[] for _ in range(NBLK)]
        lo_idx, hi_idx = m["lo_idx"], m["hi_idx"]
        s_all = m["s_sb"].reshape(128, -1, W)
        lo_nt, hi_nt = 0, 0
        s_n = 0
        for b in range(NBLK):
            for di, dom in enumerate(("lo", "hi")):
                have = [t for t in m["tiles_by_block"][b] if t[0] == dom]
                need = int(mx[b, di])
                for k in range(need):
                    if k < len(have):
                        _, tpos, sidx = have[k]
                        idx_arr = (lo_idx if dom == "lo" else hi_idx)[
                            tpos * 128:(tpos + 1) * 128]
                        s_mat = s_all[:, sidx, :]
                    else:
                        idx_arr = np.zeros(128, np.int64)
                        s_mat = np.zeros((128, W), ml_dtypes.bfloat16)
                    (lo_parts if dom == "lo" else hi_parts).append(idx_arr)
                    s_parts.append(np.asarray(s_mat))
                    nt = lo_nt if dom == "lo" else hi_nt
                    tiles_by_block[b].append((dom, nt, s_n))
                    s_n += 1
                    if dom == "lo":
                        lo_nt += 1
                    else:
                        hi_nt += 1
        lo_cat = (np.concatenate(lo_parts) if lo_parts
                  else np.zeros(0, np.int64))
        hi_cat = (np.concatenate(hi_parts) if hi_parts
                  else np.zeros(0, np.int64))
        s_cat = (np.stack(s_parts) if s_parts
                 else np.zeros((0, 128, W), ml_dtypes.bfloat16))
        s_sb = np.ascontiguousarray(
            np.asarray(s_cat).transpose(1, 0, 2).reshape(128, -1))
        new.append(dict(
            lo_idx=lo_cat, hi_idx=hi_cat, s_sb=s_sb,
            tiles_by_block=tiles_by_block,
            n_lo=lo_cat.shape[0], n_hi=hi_cat.shape[0], ntiles=s_n,
            node_of_row=m["node_of_row"], selfw=m["selfw"],
        ))
    return new


def _build_uniform(meta0, num_iter=NUM_ITER):
    n_lo, n_hi, ntiles = meta0["n_lo"], meta0["n_hi"], meta0["ntiles"]
    tiles_by_block = meta0["tiles_by_block"]

    nc = bacc.Bacc("TRN2", target_bir_lowering=False, debug=False,
                   num_devices=NC, num_swdge_queues=NQ)

    embtab = nc.dram_tensor("embtab", [NUM_ATOM_FEATS * 128, D], BF16,
                            kind="ExternalInput")
    oh_d = nc.dram_tensor("oh", [128, NCOL * NUM_ATOM_FEATS * 128], BF16,
                          kind="ExternalInput")
    ws = nc.dram_tensor("ws", [NUM_LAYER * D, D], F32, kind="ExternalInput")
    bs = nc.dram_tensor("bs", [NUM_LAYER, D], F32, kind="ExternalInput")
    ident = nc.dram_tensor("ident", [128, 128], F32, kind="ExternalInput")
    identb_d = nc.dram_tensor("identb", [128, 128], BF16,
                              kind="ExternalInput")
    selfw_d = nc.dram_tensor("selfw", [128, NCOL], F32, kind="ExternalInput")
    idx_lo_d = nc.dram_tensor("idx_lo", [128, max(n_lo, 16) // 16], I16,
                              kind="ExternalInput")
    idx_hi_d = nc.dram_tensor("idx_hi", [128, max(n_hi, 16) // 16], I16,
                              kind="ExternalInput")
    s_d = nc.dram_tensor("s", [128, max(ntiles, 1) * W], BF16,
                         kind="ExternalInput")
    out_d = nc.dram_tensor("out", [SHARD_PAD, D], F32, kind="ExternalOutput")

    ag_in_a = [nc.dram_tensor(f"ag_in_a{i}", [ROWS_A, D], BF16,
                              kind="Internal") for i in range(2)]
    ag_in_b = [nc.dram_tensor(f"ag_in_b{i}", [ROWS_B, D], BF16,
                              kind="Internal") for i in range(2)]
    ag_out_a = [nc.dram_tensor(f"ag_out_a{i}", [N_A, D], BF16,
                               kind="Internal", addr_space="Shared")
                for i in range(2)]
    ag_out_b = [nc.dram_tensor(f"ag_out_b{i}", [N_B, D], BF16,
                               kind="Internal", addr_space="Shared")
                for i in range(2)]

    def _emit_ag(which, buf):
        src = (ag_in_a if which == "a" else ag_in_b)[buf]
        dst = (ag_out_a if which == "a" else ag_out_b)[buf]
        return nc.gpsimd.collective_compute(
            "AllGather", mybir.AluOpType.bypass,
            replica_groups=[list(range(NC))],
            ins=[src[:].opt()], outs=[dst[:].opt()])

    lo_chunks = _chunks(n_lo)
    hi_chunks = _chunks(n_hi)

    with tile.TileContext(nc) as tc:
      with tc.tile_pool(name="persist", bufs=1) as persist:
        h_sb = persist.tile([128, NCOL, D], BF16, tag="h")
        h0s = persist.tile([128, NCOL, D], BF16, tag="h0s")
        selfw = persist.tile([128, NCOL], F32, tag="selfw")
        nc.sync.dma_start(selfw[:], selfw_d[:])
        identb = persist.tile([128, 128], BF16, tag="identb")
        nc.sync.dma_start(identb[:], identb_d[:])

        # ---------------- prologue: one-hot embedding + MLP ----------------
        with (
            tc.tile_pool(name="pro", bufs=1) as pro,
            tc.tile_pool(name="mlp", bufs=3) as mlp_pool,
            tc.tile_pool(name="prps", bufs=2, space="PSUM") as prps,
        ):
            emb_sb = pro.tile([128, NUM_ATOM_FEATS, D], BF16, tag="emb")
            nc.sync.dma_start(
                emb_sb[:],
                embtab[:, :].rearrange("(f p) d -> p f d", p=128))
            idn = pro.tile([128, 128], F32, tag="idn")
            nc.sync.dma_start(idn[:], ident[:])
            w_sb = pro.tile([128, NUM_LAYER * D], F32, tag="w")
            b_sb = pro.tile([128, NUM_LAYER], F32, tag="b")
            for l in range(NUM_LAYER):
                nc.sync.dma_start(w_sb[:, l * D:(l + 1) * D],
                                  ws[l * D:(l + 1) * D, :])
                nc.sync.dma_start(b_sb[:, l:l + 1],
                                  bs[l:l + 1, :].rearrange("a k -> k a"))
            oh_sb = pro.tile([128, NCOL * NUM_ATOM_FEATS * 128], BF16,
                             tag="oh")
            for col in range(NCOL):
                o = col * NUM_ATOM_FEATS * 128
                nc.sync.dma_start(oh_sb[:, o:o + NUM_ATOM_FEATS * 128],
                                  oh_d[:, o:o + NUM_ATOM_FEATS * 128])

            for col in range(NCOL):
                o = col * NUM_ATOM_FEATS * 128
                ps = prps.tile([128, 128], F32, tag="ps")
                for f in range(NUM_ATOM_FEATS):
                    nc.tensor.matmul(
                        ps[:], emb_sb[:, f, :],
                        oh_sb[:, o + f * 128:o + (f + 1) * 128],
                        start=(f == 0), stop=(f == NUM_ATOM_FEATS - 1))
                cur = mlp_pool.tile([128, 128], F32, tag="t")
                nc.scalar.activation(cur[:], ps[:], AF.Copy)
                for l in range(NUM_LAYER):
                    ps2 = prps.tile([128, 128], F32, tag="ps2")
                    nc.tensor.matmul(ps2[:], w_sb[:, l * D:(l + 1) * D],
                                     cur[:], start=True, stop=True)
                    cur = mlp_pool.tile([128, 128], F32, tag="t")
                    nc.scalar.activation(
                        cur[:], ps2[:],
                        AF.Relu if l != NUM_LAYER - 1 else AF.Identity,
                        bias=b_sb[:, l:l + 1])
                # transpose back: h [nodes, d]
                pt = prps.tile([128, 128], F32, tag="pt")
                nc.tensor.transpose(pt[:], cur[:], idn[:])
                nc.scalar.activation(h_sb[:, col, :], pt[:], AF.Copy)
                nc.scalar.activation(h0s[:, col, :], pt[:], AF.Copy,
                                     scale=ALPHA)
                if col == COLS_A - 1:
                    nc.sync.dma_start(
                        ag_in_a[0][:].rearrange("(c p) f -> p c f", p=128),
                        h_sb[:, 0:COLS_A, :])
                    _emit_ag("a", 0)
                elif col == NCOL - 1:
                    nc.sync.dma_start(
                        ag_in_b[0][:].rearrange("(c p) f -> p c f", p=128),
                        h_sb[:, COLS_A:NCOL, :])
                    _emit_ag("b", 0)

        # ---------------- main loop ----------------
        with (
            tc.tile_pool(name="sconst", bufs=1) as sconst,
            tc.tile_pool(name="glo", bufs=16) as glo_pool,
            tc.tile_pool(name="ghi", bufs=12) as ghi_pool,
            tc.tile_pool(name="ps", bufs=8, space="PSUM") as ps_pool,
            tc.tile_pool(name="stage", bufs=2) as stage_pool,
        ):
            s_sb = sconst.tile([128, max(ntiles, 1) * W], BF16, tag="s")
            nc.sync.dma_start(s_sb[:], s_d[:])
            ilo = sconst.tile([128, max(n_lo, 16) // 16], I16, tag="ilo")
            nc.sync.dma_start(ilo[:], idx_lo_d[:])
            ihi = sconst.tile([128, max(n_hi, 16) // 16], I16, tag="ihi")
            nc.sync.dma_start(ihi[:], idx_hi_d[:])

            qe_g = [0]
            LOP = 10  # lo chunks of iter t+1 emitted before AG_b(t)

            def emit_gather(dom, ci, buf):
                (o, n) = (lo_chunks if dom == "lo" else hi_chunks)[ci]
                pool = glo_pool if dom == "lo" else ghi_pool
                view = (ag_out_a if dom == "lo" else ag_out_b)[buf][:, :]
                isb = ilo if dom == "lo" else ihi
                g = pool.tile([128, 8, D], BF16, tag="g" + dom)
                gi = nc.gpsimd.dma_gather(
                    g[:, 0:n // 128, :], view,
                    isb[:, o // 16:(o + n) // 16], n, n, D,
                    queue_num=qe_g[0] % NQ)
                qe_g[0] += 1
                return g, gi

            pending = None  # prefix state for the next iteration
            for it in range(num_iter):
                buf = it % 2
                if pending is None:
                    lo_tiles_bufs = {}
                    lo_insts = []
                    for ci in range(min(LOP, len(lo_chunks))):
                        g, gi = emit_gather("lo", ci, buf)
                        lo_tiles_bufs[ci] = g
                        lo_insts.append(gi)
                else:
                    lo_tiles_bufs, lo_insts = pending

                hi_tiles_bufs = {}
                gath_insts = []
                hi_insts = []
                order = []
                li, hii = min(LOP, len(lo_chunks)), 0
                while li < len(lo_chunks) or hii < len(hi_chunks):
                    if li < len(lo_chunks):
                        order.append(("lo", li)); li += 1
                    if hii < len(hi_chunks):
                        order.append(("hi", hii)); hii += 1
                for dom, ci in order:
                    g, gi = emit_gather(dom, ci, buf)
                    gath_insts.append(gi)
                    (lo_insts if dom == "lo" else hi_insts).append(gi)
                    (lo_tiles_bufs if dom == "lo" else hi_tiles_bufs)[ci] = g

                # soft absorb: schedule the first hi chunks after a few
                # post-prefix lo chunks so the engine doesn't park on the
                # AG_b wait while runnable lo gathers sit behind it
                for j in range(min(6, len(hi_insts))):
                    anchor = min(LOP + 2 + 2 * j, len(lo_insts) - 1)
                    dd = InstructionNameOrderedSet()
                    dd.add(lo_insts[anchor].ins.name)
                    hi_insts[j].ins.add_nosync_dependencies_from(dd)

                last = it == num_iter - 1
                for p in range(NBLK // 2):
                    col = p
                    psum = ps_pool.tile([128, D], F32, tag="ps")
                    for half in range(2):
                        tl = tiles_by_block[2 * p + half]
                        ph = half * 64
                        nc.tensor.matmul(
                            psum[ph:ph + 64, :],
                            identb[:, ph:ph + 64],
                            h0s[:, col, :],
                            start=True, stop=(len(tl) == 0))
                        for j, (dom, tpos, sidx) in enumerate(tl):
                            bufs = (lo_tiles_bufs if dom == "lo"
                                    else hi_tiles_bufs)
                            g = bufs[tpos // 8]
                            nc.tensor.matmul(
                                psum[ph:ph + 64, :],
                                s_sb[:, sidx * W:(sidx + 1) * W],
                                g[:, tpos % 8, :],
                                start=False, stop=(j == len(tl) - 1))
                    # evict: h_new = selfw * h_old + psum  (one DVE op)
                    if last:
                        st = stage_pool.tile([128, D], F32, tag="st")
                        nc.vector.scalar_tensor_tensor(
                            st[:], h_sb[:, col, :], selfw[:, col:col + 1],
                            psum[:, :], mybir.AluOpType.mult,
                            mybir.AluOpType.add)
                        nc.sync.dma_start(
                            out_d[p * 128:(p + 1) * 128, :], st[:])
                    else:
                        nc.vector.scalar_tensor_tensor(
                            h_sb[:, col, :], h_sb[:, col, :],
                            selfw[:, col:col + 1],
                            psum[:, :], mybir.AluOpType.mult,
                            mybir.AluOpType.add)
                        if p == COLS_A - 1:
                            nc.sync.dma_start(
                                ag_in_a[1 - buf][:].rearrange(
                                    "(c p) f -> p c f", p=128),
                                h_sb[:, 0:COLS_A, :])
                            ag_a = _emit_ag("a", 1 - buf)
                            # pin the AG trigger into the GpSimd gather
                            # stream at ~75% so its wire time overlaps the
                            # remaining gathers (GpSimd is the only engine
                            # that can trigger collectives)
                            gpos = (len(gath_insts) * 3) // 4
                            d1 = InstructionNameOrderedSet()
                            d1.add(gath_insts[gpos].ins.name)
                            ag_a.ins.add_nosync_dependencies_from(d1)
                            d2 = InstructionNameOrderedSet()
                            d2.add(ag_a.ins.name)
                            gath_insts[gpos + 1].ins.add_nosync_dependencies_from(d2)

                # software pipelining: emit the next iteration's first LOP
                # lo gathers (they only need AG_a of this iteration) BEFORE
                # staging/triggering AG_b, so they run during AG_b's wire
                # instead of idling at the iteration boundary.
                if not last:
                    nbuf = 1 - buf
                    np_bufs = {}
                    np_insts = []
                    for ci in range(min(LOP, len(lo_chunks))):
                        g, gi = emit_gather("lo", ci, nbuf)
                        np_bufs[ci] = g
                        np_insts.append(gi)
                    pending = (np_bufs, np_insts)
                    nc.sync.dma_start(
                        ag_in_b[nbuf][:].rearrange(
                            "(c p) f -> p c f", p=128),
                        h_sb[:, COLS_A:NCOL, :])
                    _emit_ag("b", nbuf)

    # Post-scheduling: align each gather's SWDGE queue with its DMASW lane
    # (lanes are assigned round-robin in scheduled order and their sems are
    # queue-locked in ucode, so queue must follow lane, not emission order).
    import re as _re
    for _blk in nc.m.functions[0].blocks:
        for _inst in _blk.instructions:
            if isinstance(_inst, mybir.InstDMAGatherAnt):
                _si = _inst.sync_info
                _lane = None
                for _u in (_si.on_update if _si else []):
                    _m = _re.match(r"DMASW(\d+)_", _u.ant_name or "")
                    if _m:
                        _lane = int(_m.group(1))
                if _lane is not None:
                    _inst.queue_num = _lane % NQ
    nc.compile()
    return nc


_CACHE = {}


def _get_compiled(edge_index, num_iter=NUM_ITER):
    key = (hash(np.asarray(edge_index).tobytes()), num_iter)
    if key not in _CACHE:
        cores = _preprocess(edge_index)
        cores = _equalize(cores)
        nc = _build_uniform(cores[0], num_iter=num_iter)
        _CACHE[key] = (nc, cores)
    return _CACHE[key]


def _make_in_maps(x, atom_emb, Ws, bs, cores_meta):
    x = np.asarray(x)
    emb_pad = np.zeros((NUM_ATOM_FEATS * 128, D), ml_dtypes.bfloat16)
    ae = np.asarray(atom_emb, dtype=np.float32)
    for f in range(NUM_ATOM_FEATS):
        emb_pad[f * 128:f * 128 + ATOM_VOCAB] = ae[f]
    ws_t = np.ascontiguousarray(
        np.asarray(Ws, dtype=np.float32).reshape(NUM_LAYER * D, D))
    bs_t = np.ascontiguousarray(np.asarray(bs, dtype=np.float32))
    ident = np.eye(128, dtype=np.float32)
    identb = np.eye(128, dtype=ml_dtypes.bfloat16)

    in_maps = []
    for c, m in enumerate(cores_meta):
        nr = m["node_of_row"]  # [SHARD_PAD] local node or -1
        # one-hot: [128 vocab-pad, NCOL*9*128] with oh[v, (col,f,n)] = 1
        oh = np.zeros((128, NCOL * NUM_ATOM_FEATS * 128), ml_dtypes.bfloat16)
        xs = x[c * SHARD:(c + 1) * SHARD]  # [SHARD, 9]
        rows = np.arange(SHARD_PAD)
        valid = nr >= 0
        for f in range(NUM_ATOM_FEATS):
            vals = np.zeros(SHARD_PAD, np.int64)
            vals[valid] = xs[nr[valid], f]
            cols = (rows // 128) * NUM_ATOM_FEATS * 128 + f * 128 + rows % 128
            oh[vals[valid], cols[valid]] = 1.0
        lo = m["lo_idx"] if m["n_lo"] else np.zeros(16, np.int64)
        hi_ = m["hi_idx"] if m["n_hi"] else np.zeros(16, np.int64)
        in_maps.append({
            "embtab": emb_pad,
            "oh": np.ascontiguousarray(oh),
            "ws": ws_t,
            "bs": bs_t,
            "ident": ident,
            "identb": identb,
            "selfw": np.ascontiguousarray(m["selfw"]),
            "idx_lo": _wrap_idxs(lo),
            "idx_hi": _wrap_idxs(hi_),
            "s": np.ascontiguousarray(m["s_sb"]),
        })
    return in_maps


def _unpermute(res, cores_meta):
    out = np.zeros((N_NODES, D), np.float32)
    for c, m in enumerate(cores_meta):
        nr = m["node_of_row"]
        valid = nr >= 0
        r = np.asarray(res[c]["out"], dtype=np.float32)
        out[c * SHARD + nr[valid]] = r[valid]
    return out


def kernel(x, edge_index, atom_emb, Ws, bs):
    nc, cores_meta = _get_compiled(edge_index)
    in_maps = _make_in_maps(x, atom_emb, Ws, bs, cores_meta)
    res = run_bass_kernel_spmd(nc, in_maps, core_ids=list(range(NC)))
    return np.ascontiguousarray(_unpermute(res.results, cores_meta))


def run_profiled(x, edge_index, atom_emb, Ws, bs):
    import ntff_hook
    ntff_hook.install()
    nc, cores_meta = _get_compiled(edge_index)
    in_maps = _make_in_maps(x, atom_emb, Ws, bs, cores_meta)
    res = run_bass_kernel_spmd(nc, in_maps, core_ids=list(range(NC)),
                               trace=True)
    return (np.ascontiguousarray(_unpermute(res.results, cores_meta)),
            res.exec_time_ns)


# revision 3
# speedup vs baseline: 1.0768x; 1.0660x over previous
"""APPNP (GCN-normalized propagation) distributed Bass kernel for 8 TRN2 cores.

v2 strategy (dst-sharded message passing, gather-descriptor-optimized):
  - Nodes sharded across 8 cores (6250/core, padded to 6272 = 49*128 rows).
  - Per-core node->row permutation balances per-(block,dom) in-edge counts
    across cores so SPMD equalization padding is minimal.
  - Prologue: atom embedding via one-hot matmuls (stationary = padded
    embedding table, moving = host-built one-hot of x) directly producing
    hT; 3-layer MLP in transposed space; PE transpose back -> h (bf16),
    h0s = 0.1*h.
  - Exchange: two AllGathers per iteration into DRAM tables
    a (rows 0:2560/core -> 20480 rows) and b (rows 2560:6272 -> 29696 rows),
    both < 32768 so int16 gather indices reach everything. a is issued
    mid-iteration (after block-pair 19), b at the end; the next iteration
    issues ~40 a-sourced chunks first so b's wire time is absorbed.
  - Per iteration: dma_gather h[src] for in-edges grouped in 64-wide dst
    blocks (128-slot tiles), TensorEngine segment-sum via one-hot S
    matrices (bf16, SBUF-resident) in PSUM; h0s injected via identity
    matmul; self-loops are NOT slots: the Vector engine evicts PSUM with
    h_new = selfw * h_old + psum in one scalar_tensor_tensor op.
"""

import numpy as np
import ml_dtypes

import concourse.bacc as bacc
import concourse.bass as bass
import concourse.mybir as mybir
import concourse.tile as tile
from concourse.bass_utils import run_bass_kernel_spmd
from concourse.instruction_name_ordered_set import InstructionNameOrderedSet

# Problem constants (must match reference.py)
N_NODES = 50000
N_EDGES = 800000
D = 128
NUM_ITER = 10
NUM_LAYER = 3
ALPHA = 0.1
NUM_ATOM_FEATS = 9
ATOM_VOCAB = 119

NC = 8
SHARD = N_NODES // NC            # 6250
SHARD_PAD = 6272                 # 49 * 128
NCOL = SHARD_PAD // 128          # 49
W = 64                           # dst block width
NBLK = SHARD_PAD // W            # 98
CHUNK = 1024                     # max idxs per dma_gather
COLS_A = 30                      # shard cols in exchange table a
ROWS_A = COLS_A * 128            # 3840
ROWS_B = SHARD_PAD - ROWS_A      # 2432
N_A = NC * ROWS_A                # 30720 (< 32768)
N_B = NC * ROWS_B                # 19456 (< 32768)
NQ = 4                           # SWDGE queues
ABSORB = 0                       # lo chunks issued before first hi chunk
CHAIN = False                    # chain gather emission order

BF16 = mybir.dt.bfloat16
F32 = mybir.dt.float32
I16 = mybir.dt.int16
AF = mybir.ActivationFunctionType


def _wrap_idxs(idx):
    """slot i -> partition i%16 (replicated x8), col i//16."""
    n = idx.shape[0]
    assert n % 16 == 0
    w = idx.reshape(n // 16, 16).T.astype(np.int16)
    return np.ascontiguousarray(np.tile(w, (8, 1)))


def _pad128(a, fill=0):
    n = a.shape[0]
    m = (-n) % 128
    if m == 0:
        return a
    return np.concatenate([a, np.full((m,) + a.shape[1:], fill, a.dtype)])


def _balance_perm(indeg):
    """Greedy LPT: assign 6250 local nodes to 98 blocks of <=64 nodes,
    balancing total in-degree per block. Returns node_of_row[6272] with -1
    for pad rows (all pads in the last block)."""
    import heapq
    order = np.argsort(-indeg, kind="stable")
    cap = np.full(NBLK, 64, np.int64)
    cap[NBLK - 1] = SHARD - 64 * (NBLK - 1)  # 42 real nodes in last block
    fill = [[] for _ in range(NBLK)]
    heap = [(0, b) for b in range(NBLK)]
    heapq.heapify(heap)
    for nid in order:
        while True:
            tot, b = heapq.heappop(heap)
            if len(fill[b]) < cap[b]:
                break
        fill[b].append(nid)
        if len(fill[b]) < cap[b]:
            heapq.heappush(heap, (tot + int(indeg[nid]), b))
    node_of_row = np.full(SHARD_PAD, -1, np.int64)
    for b in range(NBLK):
        for j, nid in enumerate(fill[b]):
            node_of_row[b * 64 + j] = nid
    return node_of_row


def _preprocess(edge_index):
    """Host-side graph preprocessing -> per-core structures."""
    src = np.asarray(edge_index[0], dtype=np.int64)
    dst = np.asarray(edge_index[1], dtype=np.int64)
    deg = np.bincount(dst, minlength=N_NODES).astype(np.float64) + 1.0
    dinv = 1.0 / np.sqrt(deg)
    coef = ((1.0 - ALPHA) * dinv[src] * dinv[dst]).astype(np.float32)
    selfw_g = ((1.0 - ALPHA) * dinv * dinv).astype(np.float32)

    # pass A: per-core balanced permutation (total in-degree)
    indeg_all = np.bincount(dst, minlength=N_NODES)
    node_of_row = np.zeros((NC, SHARD_PAD), np.int64)
    row_of_node = np.zeros(N_NODES, np.int64)  # global node -> local row
    for c in range(NC):
        nr = _balance_perm(indeg_all[c * SHARD:(c + 1) * SHARD])
        node_of_row[c] = nr
        valid = nr >= 0
        row_of_node[c * SHARD + nr[valid]] = np.nonzero(valid)[0]

    # pass B: with src sides fixed by pass A, pack nodes into blocks so each
    # (block, dom) in-edge count lands just UNDER a multiple of 128 (the
    # gather-tile quantum) and aligns across cores. Targets are global (the
    # max core's totals) so SPMD equalization adds almost nothing.
    src_isa = (row_of_node[src] % SHARD_PAD) < ROWS_A
    lo_in = np.bincount(dst[src_isa], minlength=N_NODES)
    hi_in = np.bincount(dst[~src_isa], minlength=N_NODES)
    RESID = 104  # target residue mod 128 (margin 24 to the next tile)

    def _targets(total_max, nb):
        base_q = max(0, int((total_max / nb - RESID) // 128))
        t = np.full(nb, base_q * 128 + RESID, np.float64)
        k = 0
        while t.sum() < total_max and k < nb:
            t[k] += 128
            k += 1
        while t.sum() < total_max:
            t += 128
        return t

    side_meta = []
    for side in (0, 1):
        if side == 0:
            blocks = list(range(0, ROWS_A // W))
        else:
            blocks = list(range(ROWS_A // W, NBLK))
        lmax = hmax = 0.0
        for c in range(NC):
            rows0 = 0 if side == 0 else ROWS_A
            nrows = ROWS_A if side == 0 else ROWS_B
            nodes = node_of_row[c][rows0:rows0 + nrows]
            nodes = nodes[nodes >= 0]
            lmax = max(lmax, lo_in[c * SHARD + nodes].sum())
            hmax = max(hmax, hi_in[c * SHARD + nodes].sum())
        nb = len(blocks)
        side_meta.append((blocks, _targets(lmax, nb), _targets(hmax, nb)))

    for c in range(NC):
        nr_new = np.full(SHARD_PAD, -1, np.int64)
        for side in (0, 1):
            blocks, T_lo, T_hi = side_meta[side]
            rows0 = 0 if side == 0 else ROWS_A
            nrows = ROWS_A if side == 0 else ROWS_B
            old_nodes = node_of_row[c][rows0:rows0 + nrows]
            old_nodes = old_nodes[old_nodes >= 0]
            li = lo_in[c * SHARD + old_nodes].astype(np.float64)
            hi_ = hi_in[c * SHARD + old_nodes].astype(np.float64)
            nb = len(blocks)
            caps = np.array([64 if b != NBLK - 1 else
                             SHARD - 64 * (NBLK - 1) for b in blocks])
            order_n = np.argsort(-(li + hi_), kind="stable")
            cur = np.zeros((nb, 2))
            cnt = np.zeros(nb, np.int64)
            assign = np.zeros(len(old_nodes), np.int64)
            for j in order_n:
                cost = np.maximum((cur[:, 0] + li[j]) / T_lo,
                                  (cur[:, 1] + hi_[j]) / T_hi)
                cost[cnt >= caps] = np.inf
                bsel = int(np.argmin(cost))
                assign[j] = bsel
                cur[bsel, 0] += li[j]
                cur[bsel, 1] += hi_[j]
                cnt[bsel] += 1
            # swap refinement: push overshoot (beyond targets) to zero
            def over(cb):
                return (max(0.0, cb[0]) + max(0.0, cb[1]))
            ex = cur - np.stack([T_lo, T_hi], axis=1)
            rng = np.random.default_rng(c)
            for _ in range(4):
                bad = np.nonzero((ex[:, 0] > 0) | (ex[:, 1] > 0))[0]
                if bad.size == 0:
                    break
                improved = False
                for b1 in bad:
                    js = np.nonzero(assign == b1)[0]
                    cands = rng.permutation(nb)[:20]
                    done = False
                    for b2 in cands:
                        if b2 == b1:
                            continue
                        for j1 in js[np.argsort(-(li[js] + hi_[js]))][:12]:
                            js2 = np.nonzero(assign == b2)[0]
                            if js2.size == 0:
                                continue
                            d1 = np.array([li[j1], hi_[j1]])
                            base = (over(ex[b1]) + over(ex[b2]))
                            d2s = np.stack([li[js2], hi_[js2]], axis=1)
                            nb1 = ex[b1] - d1 + d2s
                            nb2 = ex[b2] + d1 - d2s
                            costs = (np.maximum(nb1, 0).sum(axis=1) +
                                     np.maximum(nb2, 0).sum(axis=1))
                            kk = int(np.argmin(costs))
                            if costs[kk] < base - 0.5:
                                j2 = js2[kk]
                                ex[b1] = nb1[kk]
                                ex[b2] = nb2[kk]
                                cur[b1] += d2s[kk] - d1
                                cur[b2] += d1 - d2s[kk]
                                assign[j1], assign[j2] = b2, b1
                                improved = True
                                done = True
                                break
                        if done:
                            break
                if not improved:
                    break
            for bi, b in enumerate(blocks):
                nodes_b = old_nodes[assign == bi]
                for j2, nid in enumerate(nodes_b):
                    nr_new[b * 64 + j2] = nid
        node_of_row[c] = nr_new
        valid = nr_new >= 0
        row_of_node[c * SHARD + nr_new[valid]] = np.nonzero(valid)[0]

    # edge srow (exchange-table row of the source)
    src_core = src // SHARD
    r = row_of_node[src]
    isa = r < ROWS_A
    srow = np.where(isa, src_core * ROWS_A + r,
                    src_core * ROWS_B + (r - ROWS_A))

    cores = []
    for c in range(NC):
        m = (dst >= c * SHARD) & (dst < (c + 1) * SHARD)
        ldr = row_of_node[dst[m]]  # local row of each in-edge's dst
        lsrow = srow[m]
        lcoef = coef[m]
        lisa = isa[m]
        blk = ldr // W
        off = ldr % W

        streams = {"lo": [], "hi": []}
        s_tiles = []
        tiles_by_block = [[] for _ in range(NBLK)]
        stream_ntiles = {"lo": 0, "hi": 0}
        for b in range(NBLK):
            bm = blk == b
            for dom, dm in (("lo", lisa), ("hi", ~lisa)):
                sel = bm & dm
                n = int(sel.sum())
                if n == 0:
                    continue
                idx = _pad128(lsrow[sel].astype(np.int64))
                cf = _pad128(lcoef[sel])
                of = _pad128(off[sel].astype(np.int64))
                ntile = idx.shape[0] // 128
                for t in range(ntile):
                    s = np.zeros((128, W), np.float32)
                    s[np.arange(128), of[t * 128:(t + 1) * 128]] = \
                        cf[t * 128:(t + 1) * 128]
                    tiles_by_block[b].append((dom, stream_ntiles[dom] + t,
                                              len(s_tiles)))
                    s_tiles.append(s)
                streams[dom].append(idx)
                stream_ntiles[dom] += ntile

        lo_idx = (np.concatenate(streams["lo"]) if streams["lo"]
                  else np.zeros(0, np.int64))
        hi_idx = (np.concatenate(streams["hi"]) if streams["hi"]
                  else np.zeros(0, np.int64))
        s_all = (np.stack(s_tiles) if s_tiles
                 else np.zeros((0, 128, W), np.float32))
        s_sb = np.ascontiguousarray(
            s_all.transpose(1, 0, 2).reshape(128, -1)).astype(ml_dtypes.bfloat16)
        # selfw per row [128, NCOL]
        sw = np.zeros(SHARD_PAD, np.float32)
        nr = node_of_row[c]
        valid = nr >= 0
        sw[valid] = selfw_g[c * SHARD + nr[valid]]
        cores.append(dict(
            lo_idx=lo_idx, hi_idx=hi_idx, s_sb=s_sb,
            tiles_by_block=tiles_by_block,
            n_lo=lo_idx.shape[0], n_hi=hi_idx.shape[0],
            ntiles=len(s_tiles),
            node_of_row=node_of_row[c],
            selfw=np.ascontiguousarray(
                sw.reshape(NCOL, 128).T),  # [128, NCOL]
        ))
    return cores


def _chunks(total):
    out = []
    o = 0
    while o < total:
        n = min(CHUNK, total - o)
        out.append((o, n))
        o += n
    return out


def _equalize(cores_meta):
    """Pad per-block/dom tile counts to the max across cores (SPMD)."""
    cnt = np.zeros((NC, NBLK, 2), np.int64)
    for c, m in enumerate(cores_meta):
        for b in range(NBLK):
            for dom, tpos, sidx in m["tiles_by_block"][b]:
                cnt[c, b, 0 if dom == "lo" else 1] += 1
    mx = cnt.max(axis=0)

    new = []
    for c, m in enumerate(cores_meta):
        lo_parts, hi_parts, s_parts = [], [], []
        tiles_by_block = [[] for _ in range(NBLK)]
        lo_idx, hi_idx = m["lo_idx"], m["hi_idx"]
        s_all = m["s_sb"].reshape(128, -1, W)
        lo_nt, hi_nt = 0, 0
        s_n = 0
        for b in range(NBLK):
            for di, dom in enumerate(("lo", "hi")):
                have = [t for t in m["tiles_by_block"][b] if t[0] == dom]
                need = int(mx[b, di])
                for k in range(need):
                    if k < len(have):
                        _, tpos, sidx = have[k]
                        idx_arr = (lo_idx if dom == "lo" else hi_idx)[
                            tpos * 128:(tpos + 1) * 128]
                        s_mat = s_all[:, sidx, :]
                    else:
                        idx_arr = np.zeros(128, np.int64)
                        s_mat = np.zeros((128, W), ml_dtypes.bfloat16)
                    (lo_parts if dom == "lo" else hi_parts).append(idx_arr)
                    s_parts.append(np.asarray(s_mat))
                    nt = lo_nt if dom == "lo" else hi_nt
                    tiles_by_block[b].append((dom, nt, s_n))
                    s_n += 1
                    if dom == "lo":
                        lo_nt += 1
                    else:
                        hi_nt += 1
        lo_cat = (np.concatenate(lo_parts) if lo_parts
                  else np.zeros(0, np.int64))
        hi_cat = (np.concatenate(hi_parts) if hi_parts
                  else np.zeros(0, np.int64))
        s_cat = (np.stack(s_parts) if s_parts
                 else np.zeros((0, 128, W), ml_dtypes.bfloat16))
        s_sb = np.ascontiguousarray(
            np.asarray(s_cat).transpose(1, 0, 2).reshape(128, -1))
        new.append(dict(
            lo_idx=lo_cat, hi_idx=hi_cat, s_sb=s_sb,
            tiles_by_block=tiles_by_block,
            n_lo=lo_cat.shape[0], n_hi=hi_cat.shape[0], ntiles=s_n,
            node_of_row=m["node_of_row"], selfw=m["selfw"],
        ))
    return new


def _build_uniform(meta0, num_iter=NUM_ITER):
    n_lo, n_hi, ntiles = meta0["n_lo"], meta0["n_hi"], meta0["ntiles"]
    tiles_by_block = meta0["tiles_by_block"]

    nc = bacc.Bacc("TRN2", target_bir_lowering=False, debug=False,
                   num_devices=NC, num_swdge_queues=NQ)

    embtab = nc.dram_tensor("embtab", [NUM_ATOM_FEATS * 128, D], BF16,
                            kind="ExternalInput")
    oh_d = nc.dram_tensor("oh", [128, NCOL * NUM_ATOM_FEATS * 128], BF16,
                          kind="ExternalInput")
    ws = nc.dram_tensor("ws", [NUM_LAYER * D, D], F32, kind="ExternalInput")
    bs = nc.dram_tensor("bs", [NUM_LAYER, D], F32, kind="ExternalInput")
    ident = nc.dram_tensor("ident", [128, 128], F32, kind="ExternalInput")
    identb_d = nc.dram_tensor("identb", [128, 128], BF16,
                              kind="ExternalInput")
    selfw_d = nc.dram_tensor("selfw", [128, NCOL], F32, kind="ExternalInput")
    idx_lo_d = nc.dram_tensor("idx_lo", [128, max(n_lo, 16) // 16], I16,
                              kind="ExternalInput")
    idx_hi_d = nc.dram_tensor("idx_hi", [128, max(n_hi, 16) // 16], I16,
                              kind="ExternalInput")
    s_d = nc.dram_tensor("s", [128, max(ntiles, 1) * W], BF16,
                         kind="ExternalInput")
    out_d = nc.dram_tensor("out", [SHARD_PAD, D], F32, kind="ExternalOutput")

    ag_in_a = [nc.dram_tensor(f"ag_in_a{i}", [ROWS_A, D], BF16,
                              kind="Internal") for i in range(2)]
    ag_in_b = [nc.dram_tensor(f"ag_in_b{i}", [ROWS_B, D], BF16,
                              kind="Internal") for i in range(2)]
    ag_out_a = [nc.dram_tensor(f"ag_out_a{i}", [N_A, D], BF16,
                               kind="Internal", addr_space="Shared")
                for i in range(2)]
    ag_out_b = [nc.dram_tensor(f"ag_out_b{i}", [N_B, D], BF16,
                               kind="Internal", addr_space="Shared")
                for i in range(2)]

    def _emit_ag(which, buf):
        src = (ag_in_a if which == "a" else ag_in_b)[buf]
        dst = (ag_out_a if which == "a" else ag_out_b)[buf]
        return nc.gpsimd.collective_compute(
            "AllGather", mybir.AluOpType.bypass,
            replica_groups=[list(range(NC))],
            ins=[src[:].opt()], outs=[dst[:].opt()])

    lo_chunks = _chunks(n_lo)
    hi_chunks = _chunks(n_hi)

    with tile.TileContext(nc) as tc:
      with tc.tile_pool(name="persist", bufs=1) as persist:
        h_sb = persist.tile([128, NCOL, D], BF16, tag="h")
        h0s = persist.tile([128, NCOL, D], BF16, tag="h0s")
        selfw = persist.tile([128, NCOL], F32, tag="selfw")
        nc.sync.dma_start(selfw[:], selfw_d[:])
        identb = persist.tile([128, 128], BF16, tag="identb")
        nc.sync.dma_start(identb[:], identb_d[:])

        # ---------------- prologue: one-hot embedding + MLP ----------------
        with (
            tc.tile_pool(name="pro", bufs=1) as pro,
            tc.tile_pool(name="mlp", bufs=3) as mlp_pool,
            tc.tile_pool(name="prps", bufs=2, space="PSUM") as prps,
        ):
            emb_sb = pro.tile([128, NUM_ATOM_FEATS, D], BF16, tag="emb")
            nc.sync.dma_start(
                emb_sb[:],
                embtab[:, :].rearrange("(f p) d -> p f d", p=128))
            idn = pro.tile([128, 128], F32, tag="idn")
            nc.sync.dma_start(idn[:], ident[:])
            w_sb = pro.tile([128, NUM_LAYER * D], F32, tag="w")
            b_sb = pro.tile([128, NUM_LAYER], F32, tag="b")
            for l in range(NUM_LAYER):
                nc.sync.dma_start(w_sb[:, l * D:(l + 1) * D],
                                  ws[l * D:(l + 1) * D, :])
                nc.sync.dma_start(b_sb[:, l:l + 1],
                                  bs[l:l + 1, :].rearrange("a k -> k a"))
            oh_sb = pro.tile([128, NCOL * NUM_ATOM_FEATS * 128], BF16,
                             tag="oh")
            for col in range(NCOL):
                o = col * NUM_ATOM_FEATS * 128
                nc.sync.dma_start(oh_sb[:, o:o + NUM_ATOM_FEATS * 128],
                                  oh_d[:, o:o + NUM_ATOM_FEATS * 128])

            for col in range(NCOL):
                o = col * NUM_ATOM_FEATS * 128
                ps = prps.tile([128, 128], F32, tag="ps")
                for f in range(NUM_ATOM_FEATS):
                    nc.tensor.matmul(
                        ps[:], emb_sb[:, f, :],
                        oh_sb[:, o + f * 128:o + (f + 1) * 128],
                        start=(f == 0), stop=(f == NUM_ATOM_FEATS - 1))
                cur = mlp_pool.tile([128, 128], F32, tag="t")
                nc.scalar.activation(cur[:], ps[:], AF.Copy)
                for l in range(NUM_LAYER):
                    ps2 = prps.tile([128, 128], F32, tag="ps2")
                    nc.tensor.matmul(ps2[:], w_sb[:, l * D:(l + 1) * D],
                                     cur[:], start=True, stop=True)
                    cur = mlp_pool.tile([128, 128], F32, tag="t")
                    nc.scalar.activation(
                        cur[:], ps2[:],
                        AF.Relu if l != NUM_LAYER - 1 else AF.Identity,
                        bias=b_sb[:, l:l + 1])
                # transpose back: h [nodes, d]
                pt = prps.tile([128, 128], F32, tag="pt")
                nc.tensor.transpose(pt[:], cur[:], idn[:])
                nc.scalar.activation(h_sb[:, col, :], pt[:], AF.Copy)
                nc.scalar.activation(h0s[:, col, :], pt[:], AF.Copy,
                                     scale=ALPHA)
                if col == COLS_A - 1:
                    nc.sync.dma_start(
                        ag_in_a[0][:].rearrange("(c p) f -> p c f", p=128),
                        h_sb[:, 0:COLS_A, :])
                    _emit_ag("a", 0)
                elif col == NCOL - 1:
                    nc.sync.dma_start(
                        ag_in_b[0][:].rearrange("(c p) f -> p c f", p=128),
                        h_sb[:, COLS_A:NCOL, :])

        # ---------------- main loop ----------------
        with (
            tc.tile_pool(name="sconst", bufs=1) as sconst,
            tc.tile_pool(name="glo", bufs=18) as glo_pool,
            tc.tile_pool(name="ghi", bufs=12) as ghi_pool,
            tc.tile_pool(name="ps", bufs=8, space="PSUM") as ps_pool,
            tc.tile_pool(name="stage", bufs=2) as stage_pool,
        ):
            s_sb = sconst.tile([128, max(ntiles, 1) * W], BF16, tag="s")
            nc.sync.dma_start(s_sb[:], s_d[:])
            ilo = sconst.tile([128, max(n_lo, 16) // 16], I16, tag="ilo")
            nc.sync.dma_start(ilo[:], idx_lo_d[:])
            ihi = sconst.tile([128, max(n_hi, 16) // 16], I16, tag="ihi")
            nc.sync.dma_start(ihi[:], idx_hi_d[:])

            qe_g = [0]
            LOP = 12  # lo chunks of iter t+1 emitted before AG_b(t)

            def emit_gather(dom, ci, buf):
                (o, n) = (lo_chunks if dom == "lo" else hi_chunks)[ci]
                pool = glo_pool if dom == "lo" else ghi_pool
                view = (ag_out_a if dom == "lo" else ag_out_b)[buf][:, :]
                isb = ilo if dom == "lo" else ihi
                g = pool.tile([128, 8, D], BF16, tag="g" + dom)
                gi = nc.gpsimd.dma_gather(
                    g[:, 0:n // 128, :], view,
                    isb[:, o // 16:(o + n) // 16], n, n, D,
                    queue_num=qe_g[0] % NQ)
                qe_g[0] += 1
                return g, gi

            pending = None  # prefix state for the next iteration
            for it in range(num_iter):
                buf = it % 2
                if pending is None:
                    lo_tiles_bufs = {}
                    lo_insts = []
                    for ci in range(min(LOP, len(lo_chunks))):
                        g, gi = emit_gather("lo", ci, buf)
                        lo_tiles_bufs[ci] = g
                        lo_insts.append(gi)
                    # prologue staged table b; trigger its AllGather now so
                    # the wire overlaps the prefix gathers above
                    _emit_ag("b", 0)
                else:
                    lo_tiles_bufs, lo_insts = pending

                hi_tiles_bufs = {}
                gath_insts = []
                hi_insts = []
                order = []
                li, hii = min(LOP, len(lo_chunks)), 0
                while li < len(lo_chunks) or hii < len(hi_chunks):
                    if li < len(lo_chunks):
                        order.append(("lo", li)); li += 1
                    if hii < len(hi_chunks):
                        order.append(("hi", hii)); hii += 1
                for dom, ci in order:
                    g, gi = emit_gather(dom, ci, buf)
                    gath_insts.append(gi)
                    (lo_insts if dom == "lo" else hi_insts).append(gi)
                    (lo_tiles_bufs if dom == "lo" else hi_tiles_bufs)[ci] = g

                # soft absorb: schedule the first hi chunks after a few
                # post-prefix lo chunks so the engine doesn't park on the
                # AG_b wait while runnable lo gathers sit behind it
                for j in range(min(6, len(hi_insts))):
                    anchor = min(LOP + 2 + 2 * j, len(lo_insts) - 1)
                    dd = InstructionNameOrderedSet()
                    dd.add(lo_insts[anchor].ins.name)
                    hi_insts[j].ins.add_nosync_dependencies_from(dd)

                last = it == num_iter - 1
                for p in range(NBLK // 2):
                    col = p
                    psum = ps_pool.tile([128, D], F32, tag="ps")
                    for half in range(2):
                        tl = tiles_by_block[2 * p + half]
                        ph = half * 64
                        nc.tensor.matmul(
                            psum[ph:ph + 64, :],
                            identb[:, ph:ph + 64],
                            h0s[:, col, :],
                            start=True, stop=(len(tl) == 0))
                        for j, (dom, tpos, sidx) in enumerate(tl):
                            bufs = (lo_tiles_bufs if dom == "lo"
                                    else hi_tiles_bufs)
                            g = bufs[tpos // 8]
                            nc.tensor.matmul(
                                psum[ph:ph + 64, :],
                                s_sb[:, sidx * W:(sidx + 1) * W],
                                g[:, tpos % 8, :],
                                start=False, stop=(j == len(tl) - 1))
                    # evict: h_new = selfw * h_old + psum  (one DVE op)
                    if last:
                        st = stage_pool.tile([128, D], F32, tag="st")
                        nc.vector.scalar_tensor_tensor(
                            st[:], h_sb[:, col, :], selfw[:, col:col + 1],
                            psum[:, :], mybir.AluOpType.mult,
                            mybir.AluOpType.add)
                        nc.sync.dma_start(
                            out_d[p * 128:(p + 1) * 128, :], st[:])
                    else:
                        nc.vector.scalar_tensor_tensor(
                            h_sb[:, col, :], h_sb[:, col, :],
                            selfw[:, col:col + 1],
                            psum[:, :], mybir.AluOpType.mult,
                            mybir.AluOpType.add)
                        if p == COLS_A - 1:
                            nc.sync.dma_start(
                                ag_in_a[1 - buf][:].rearrange(
                                    "(c p) f -> p c f", p=128),
                                h_sb[:, 0:COLS_A, :])
                            ag_a = _emit_ag("a", 1 - buf)
                            # pin the AG trigger into the GpSimd gather
                            # stream at ~75% so its wire time overlaps the
                            # remaining gathers (GpSimd is the only engine
                            # that can trigger collectives)
                            gpos = (len(gath_insts) * 3) // 4
                            d1 = InstructionNameOrderedSet()
                            d1.add(gath_insts[gpos].ins.name)
                            ag_a.ins.add_nosync_dependencies_from(d1)
                            d2 = InstructionNameOrderedSet()
                            d2.add(ag_a.ins.name)
                            gath_insts[gpos + 1].ins.add_nosync_dependencies_from(d2)

                # software pipelining: emit the next iteration's first LOP
                # lo gathers (they only need AG_a of this iteration) BEFORE
                # staging/triggering AG_b, so they run during AG_b's wire
                # instead of idling at the iteration boundary.
                if not last:
                    nbuf = 1 - buf
                    np_bufs = {}
                    np_insts = []
                    for ci in range(min(LOP, len(lo_chunks))):
                        g, gi = emit_gather("lo", ci, nbuf)
                        np_bufs[ci] = g
                        np_insts.append(gi)
                    pending = (np_bufs, np_insts)
                    nc.sync.dma_start(
                        ag_in_b[nbuf][:].rearrange(
                            "(c p) f -> p c f", p=128),
                        h_sb[:, COLS_A:NCOL, :])
                    _emit_ag("b", nbuf)

    # Post-scheduling: align each gather's SWDGE queue with its DMASW lane
    # (lanes are assigned round-robin in scheduled order and their sems are
    # queue-locked in ucode, so queue must follow lane, not emission order).
    import re as _re
    for _blk in nc.m.functions[0].blocks:
        for _inst in _blk.instructions:
            if isinstance(_inst, mybir.InstDMAGatherAnt):
                _si = _inst.sync_info
                _lane = None
                for _u in (_si.on_update if _si else []):
                    _m = _re.match(r"DMASW(\d+)_", _u.ant_name or "")
                    if _m:
                        _lane = int(_m.group(1))
                if _lane is not None:
                    _inst.queue_num = _lane % NQ
    nc.compile()
    return nc


_CACHE = {}


def _get_compiled(edge_index, num_iter=NUM_ITER):
    key = (hash(np.asarray(edge_index).tobytes()), num_iter)
    if key not in _CACHE:
        cores = _preprocess(edge_index)
        cores = _equalize(cores)
        nc = _build_uniform(cores[0], num_iter=num_iter)
        _CACHE[key] = (nc, cores)
    return _CACHE[key]


def _make_in_maps(x, atom_emb, Ws, bs, cores_meta):
    x = np.asarray(x)
    emb_pad = np.zeros((NUM_ATOM_FEATS * 128, D), ml_dtypes.bfloat16)
    ae = np.asarray(atom_emb, dtype=np.float32)
    for f in range(NUM_ATOM_FEATS):
        emb_pad[f * 128:f * 128 + ATOM_VOCAB] = ae[f]
    ws_t = np.ascontiguousarray(
        np.asarray(Ws, dtype=np.float32).reshape(NUM_LAYER * D, D))
    bs_t = np.ascontiguousarray(np.asarray(bs, dtype=np.float32))
    ident = np.eye(128, dtype=np.float32)
    identb = np.eye(128, dtype=ml_dtypes.bfloat16)

    in_maps = []
    for c, m in enumerate(cores_meta):
        nr = m["node_of_row"]  # [SHARD_PAD] local node or -1
        # one-hot: [128 vocab-pad, NCOL*9*128] with oh[v, (col,f,n)] = 1
        oh = np.zeros((128, NCOL * NUM_ATOM_FEATS * 128), ml_dtypes.bfloat16)
        xs = x[c * SHARD:(c + 1) * SHARD]  # [SHARD, 9]
        rows = np.arange(SHARD_PAD)
        valid = nr >= 0
        for f in range(NUM_ATOM_FEATS):
            vals = np.zeros(SHARD_PAD, np.int64)
            vals[valid] = xs[nr[valid], f]
            cols = (rows // 128) * NUM_ATOM_FEATS * 128 + f * 128 + rows % 128
            oh[vals[valid], cols[valid]] = 1.0
        lo = m["lo_idx"] if m["n_lo"] else np.zeros(16, np.int64)
        hi_ = m["hi_idx"] if m["n_hi"] else np.zeros(16, np.int64)
        in_maps.append({
            "embtab": emb_pad,
            "oh": np.ascontiguousarray(oh),
            "ws": ws_t,
            "bs": bs_t,
            "ident": ident,
            "identb": identb,
            "selfw": np.ascontiguousarray(m["selfw"]),
            "idx_lo": _wrap_idxs(lo),
            "idx_hi": _wrap_idxs(hi_),
            "s": np.ascontiguousarray(m["s_sb"]),
        })
    return in_maps


def _unpermute(res, cores_meta):
    out = np.zeros((N_NODES, D), np.float32)
    for c, m in enumerate(cores_meta):
        nr = m["node_of_row"]
        valid = nr >= 0
        r = np.asarray(res[c]["out"], dtype=np.float32)
        out[c * SHARD + nr[valid]] = r[valid]
    return out


def kernel(x, edge_index, atom_emb, Ws, bs):
    nc, cores_meta = _get_compiled(edge_index)
    in_maps = _make_in_maps(x, atom_emb, Ws, bs, cores_meta)
    res = run_bass_kernel_spmd(nc, in_maps, core_ids=list(range(NC)))
    return np.ascontiguousarray(_unpermute(res.results, cores_meta))


def run_profiled(x, edge_index, atom_emb, Ws, bs):
    import ntff_hook
    ntff_hook.install()
    nc, cores_meta = _get_compiled(edge_index)
    in_maps = _make_in_maps(x, atom_emb, Ws, bs, cores_meta)
    res = run_bass_kernel_spmd(nc, in_maps, core_ids=list(range(NC)),
                               trace=True)
    return (np.ascontiguousarray(_unpermute(res.results, cores_meta)),
            res.exec_time_ns)


# revision 4
# speedup vs baseline: 1.0852x; 1.0077x over previous
"""APPNP (GCN-normalized propagation) distributed Bass kernel for 8 TRN2 cores.

v2 strategy (dst-sharded message passing, gather-descriptor-optimized):
  - Nodes sharded across 8 cores (6250/core, padded to 6272 = 49*128 rows).
  - Per-core node->row permutation balances per-(block,dom) in-edge counts
    across cores so SPMD equalization padding is minimal.
  - Prologue: atom embedding via one-hot matmuls (stationary = padded
    embedding table, moving = host-built one-hot of x) directly producing
    hT; 3-layer MLP in transposed space; PE transpose back -> h (bf16),
    h0s = 0.1*h.
  - Exchange: two AllGathers per iteration into DRAM tables
    a (rows 0:2560/core -> 20480 rows) and b (rows 2560:6272 -> 29696 rows),
    both < 32768 so int16 gather indices reach everything. a is issued
    mid-iteration (after block-pair 19), b at the end; the next iteration
    issues ~40 a-sourced chunks first so b's wire time is absorbed.
  - Per iteration: dma_gather h[src] for in-edges grouped in 64-wide dst
    blocks (128-slot tiles), TensorEngine segment-sum via one-hot S
    matrices (bf16, SBUF-resident) in PSUM; h0s injected via identity
    matmul; self-loops are NOT slots: the Vector engine evicts PSUM with
    h_new = selfw * h_old + psum in one scalar_tensor_tensor op.
"""

import numpy as np
import ml_dtypes

import concourse.bacc as bacc
import concourse.bass as bass
import concourse.mybir as mybir
import concourse.tile as tile
from concourse.bass_utils import run_bass_kernel_spmd
from concourse.instruction_name_ordered_set import InstructionNameOrderedSet

# Problem constants (must match reference.py)
N_NODES = 50000
N_EDGES = 800000
D = 128
NUM_ITER = 10
NUM_LAYER = 3
ALPHA = 0.1
NUM_ATOM_FEATS = 9
ATOM_VOCAB = 119

NC = 8
SHARD = N_NODES // NC            # 6250
SHARD_PAD = 6272                 # 49 * 128
NCOL = SHARD_PAD // 128          # 49
W = 64                           # dst block width
NBLK = SHARD_PAD // W            # 98
CHUNK = 1024                     # max idxs per dma_gather
COLS_A = 30                      # shard cols in exchange table a
ROWS_A = COLS_A * 128            # 3840
ROWS_B = SHARD_PAD - ROWS_A      # 2432
N_A = NC * ROWS_A                # 30720 (< 32768)
N_B = NC * ROWS_B                # 19456 (< 32768)
NQ = 4                           # SWDGE queues
ABSORB = 0                       # lo chunks issued before first hi chunk
CHAIN = False                    # chain gather emission order

BF16 = mybir.dt.bfloat16
F32 = mybir.dt.float32
I16 = mybir.dt.int16
AF = mybir.ActivationFunctionType


def _wrap_idxs(idx):
    """slot i -> partition i%16 (replicated x8), col i//16."""
    n = idx.shape[0]
    assert n % 16 == 0
    w = idx.reshape(n // 16, 16).T.astype(np.int16)
    return np.ascontiguousarray(np.tile(w, (8, 1)))


def _pad128(a, fill=0):
    n = a.shape[0]
    m = (-n) % 128
    if m == 0:
        return a
    return np.concatenate([a, np.full((m,) + a.shape[1:], fill, a.dtype)])


def _balance_perm(indeg):
    """Greedy LPT: assign 6250 local nodes to 98 blocks of <=64 nodes,
    balancing total in-degree per block. Returns node_of_row[6272] with -1
    for pad rows (all pads in the last block)."""
    import heapq
    order = np.argsort(-indeg, kind="stable")
    cap = np.full(NBLK, 64, np.int64)
    cap[NBLK - 1] = SHARD - 64 * (NBLK - 1)  # 42 real nodes in last block
    fill = [[] for _ in range(NBLK)]
    heap = [(0, b) for b in range(NBLK)]
    heapq.heapify(heap)
    for nid in order:
        while True:
            tot, b = heapq.heappop(heap)
            if len(fill[b]) < cap[b]:
                break
        fill[b].append(nid)
        if len(fill[b]) < cap[b]:
            heapq.heappush(heap, (tot + int(indeg[nid]), b))
    node_of_row = np.full(SHARD_PAD, -1, np.int64)
    for b in range(NBLK):
        for j, nid in enumerate(fill[b]):
            node_of_row[b * 64 + j] = nid
    return node_of_row


def _preprocess(edge_index):
    """Host-side graph preprocessing -> per-core structures."""
    src = np.asarray(edge_index[0], dtype=np.int64)
    dst = np.asarray(edge_index[1], dtype=np.int64)
    deg = np.bincount(dst, minlength=N_NODES).astype(np.float64) + 1.0
    dinv = 1.0 / np.sqrt(deg)
    coef = ((1.0 - ALPHA) * dinv[src] * dinv[dst]).astype(np.float32)
    selfw_g = ((1.0 - ALPHA) * dinv * dinv).astype(np.float32)

    # pass A: per-core balanced permutation (total in-degree)
    indeg_all = np.bincount(dst, minlength=N_NODES)
    node_of_row = np.zeros((NC, SHARD_PAD), np.int64)
    row_of_node = np.zeros(N_NODES, np.int64)  # global node -> local row
    for c in range(NC):
        nr = _balance_perm(indeg_all[c * SHARD:(c + 1) * SHARD])
        node_of_row[c] = nr
        valid = nr >= 0
        row_of_node[c * SHARD + nr[valid]] = np.nonzero(valid)[0]

    # pass B: with src sides fixed by pass A, pack nodes into blocks so each
    # (block, dom) in-edge count lands just UNDER a multiple of 128 (the
    # gather-tile quantum) and aligns across cores. Targets are global (the
    # max core's totals) so SPMD equalization adds almost nothing.
    src_isa = (row_of_node[src] % SHARD_PAD) < ROWS_A
    lo_in = np.bincount(dst[src_isa], minlength=N_NODES)
    hi_in = np.bincount(dst[~src_isa], minlength=N_NODES)
    RESID = 118  # target residue mod 128 (margin 10 to the next tile)

    def _targets(total_max, nb):
        base_q = max(0, int((total_max / nb - RESID) // 128))
        t = np.full(nb, base_q * 128 + RESID, np.float64)
        k = 0
        while t.sum() < total_max and k < nb:
            t[k] += 128
            k += 1
        while t.sum() < total_max:
            t += 128
        return t

    side_meta = []
    for side in (0, 1):
        if side == 0:
            blocks = list(range(0, ROWS_A // W))
        else:
            blocks = list(range(ROWS_A // W, NBLK))
        lmax = hmax = 0.0
        for c in range(NC):
            rows0 = 0 if side == 0 else ROWS_A
            nrows = ROWS_A if side == 0 else ROWS_B
            nodes = node_of_row[c][rows0:rows0 + nrows]
            nodes = nodes[nodes >= 0]
            lmax = max(lmax, lo_in[c * SHARD + nodes].sum())
            hmax = max(hmax, hi_in[c * SHARD + nodes].sum())
        nb = len(blocks)
        side_meta.append((blocks, _targets(lmax, nb), _targets(hmax, nb)))

    for c in range(NC):
        nr_new = np.full(SHARD_PAD, -1, np.int64)
        for side in (0, 1):
            blocks, T_lo, T_hi = side_meta[side]
            rows0 = 0 if side == 0 else ROWS_A
            nrows = ROWS_A if side == 0 else ROWS_B
            old_nodes = node_of_row[c][rows0:rows0 + nrows]
            old_nodes = old_nodes[old_nodes >= 0]
            li = lo_in[c * SHARD + old_nodes].astype(np.float64)
            hi_ = hi_in[c * SHARD + old_nodes].astype(np.float64)
            nb = len(blocks)
            caps = np.array([64 if b != NBLK - 1 else
                             SHARD - 64 * (NBLK - 1) for b in blocks])
            order_n = np.argsort(-(li + hi_), kind="stable")
            cur = np.zeros((nb, 2))
            cnt = np.zeros(nb, np.int64)
            assign = np.zeros(len(old_nodes), np.int64)
            for j in order_n:
                cost = np.maximum((cur[:, 0] + li[j]) / T_lo,
                                  (cur[:, 1] + hi_[j]) / T_hi)
                cost[cnt >= caps] = np.inf
                bsel = int(np.argmin(cost))
                assign[j] = bsel
                cur[bsel, 0] += li[j]
                cur[bsel, 1] += hi_[j]
                cnt[bsel] += 1
            # swap refinement: push overshoot (beyond targets) to zero
            def over(cb):
                return (max(0.0, cb[0]) + max(0.0, cb[1]))
            ex = cur - np.stack([T_lo, T_hi], axis=1)
            rng = np.random.default_rng(c)
            for _ in range(4):
                bad = np.nonzero((ex[:, 0] > 0) | (ex[:, 1] > 0))[0]
                if bad.size == 0:
                    break
                improved = False
                for b1 in bad:
                    js = np.nonzero(assign == b1)[0]
                    cands = rng.permutation(nb)[:20]
                    done = False
                    for b2 in cands:
                        if b2 == b1:
                            continue
                        for j1 in js[np.argsort(-(li[js] + hi_[js]))][:12]:
                            js2 = np.nonzero(assign == b2)[0]
                            if js2.size == 0:
                                continue
                            d1 = np.array([li[j1], hi_[j1]])
                            base = (over(ex[b1]) + over(ex[b2]))
                            d2s = np.stack([li[js2], hi_[js2]], axis=1)
                            nb1 = ex[b1] - d1 + d2s
                            nb2 = ex[b2] + d1 - d2s
                            costs = (np.maximum(nb1, 0).sum(axis=1) +
                                     np.maximum(nb2, 0).sum(axis=1))
                            kk = int(np.argmin(costs))
                            if costs[kk] < base - 0.5:
                                j2 = js2[kk]
                                ex[b1] = nb1[kk]
                                ex[b2] = nb2[kk]
                                cur[b1] += d2s[kk] - d1
                                cur[b2] += d1 - d2s[kk]
                                assign[j1], assign[j2] = b2, b1
                                improved = True
                                done = True
                                break
                        if done:
                            break
                if not improved:
                    break
            for bi, b in enumerate(blocks):
                nodes_b = old_nodes[assign == bi]
                for j2, nid in enumerate(nodes_b):
                    nr_new[b * 64 + j2] = nid
        node_of_row[c] = nr_new
        valid = nr_new >= 0
        row_of_node[c * SHARD + nr_new[valid]] = np.nonzero(valid)[0]

    # edge srow (exchange-table row of the source)
    src_core = src // SHARD
    r = row_of_node[src]
    isa = r < ROWS_A
    srow = np.where(isa, src_core * ROWS_A + r,
                    src_core * ROWS_B + (r - ROWS_A))

    cores = []
    for c in range(NC):
        m = (dst >= c * SHARD) & (dst < (c + 1) * SHARD)
        ldr = row_of_node[dst[m]]  # local row of each in-edge's dst
        lsrow = srow[m]
        lcoef = coef[m]
        lisa = isa[m]
        blk = ldr // W
        off = ldr % W

        streams = {"lo": [], "hi": []}
        s_tiles = []
        tiles_by_block = [[] for _ in range(NBLK)]
        stream_ntiles = {"lo": 0, "hi": 0}
        for b in range(NBLK):
            bm = blk == b
            for dom, dm in (("lo", lisa), ("hi", ~lisa)):
                sel = bm & dm
                n = int(sel.sum())
                if n == 0:
                    continue
                idx = _pad128(lsrow[sel].astype(np.int64))
                cf = _pad128(lcoef[sel])
                of = _pad128(off[sel].astype(np.int64))
                ntile = idx.shape[0] // 128
                for t in range(ntile):
                    s = np.zeros((128, W), np.float32)
                    s[np.arange(128), of[t * 128:(t + 1) * 128]] = \
                        cf[t * 128:(t + 1) * 128]
                    tiles_by_block[b].append((dom, stream_ntiles[dom] + t,
                                              len(s_tiles)))
                    s_tiles.append(s)
                streams[dom].append(idx)
                stream_ntiles[dom] += ntile

        lo_idx = (np.concatenate(streams["lo"]) if streams["lo"]
                  else np.zeros(0, np.int64))
        hi_idx = (np.concatenate(streams["hi"]) if streams["hi"]
                  else np.zeros(0, np.int64))
        s_all = (np.stack(s_tiles) if s_tiles
                 else np.zeros((0, 128, W), np.float32))
        s_sb = np.ascontiguousarray(
            s_all.transpose(1, 0, 2).reshape(128, -1)).astype(ml_dtypes.bfloat16)
        # selfw per row [128, NCOL]
        sw = np.zeros(SHARD_PAD, np.float32)
        nr = node_of_row[c]
        valid = nr >= 0
        sw[valid] = selfw_g[c * SHARD + nr[valid]]
        cores.append(dict(
            lo_idx=lo_idx, hi_idx=hi_idx, s_sb=s_sb,
            tiles_by_block=tiles_by_block,
            n_lo=lo_idx.shape[0], n_hi=hi_idx.shape[0],
            ntiles=len(s_tiles),
            node_of_row=node_of_row[c],
            selfw=np.ascontiguousarray(
                sw.reshape(NCOL, 128).T),  # [128, NCOL]
        ))
    return cores


def _chunks(total):
    out = []
    o = 0
    while o < total:
        n = min(CHUNK, total - o)
        out.append((o, n))
        o += n
    return out


def _equalize(cores_meta):
    """Pad per-block/dom tile counts to the max across cores (SPMD)."""
    cnt = np.zeros((NC, NBLK, 2), np.int64)
    for c, m in enumerate(cores_meta):
        for b in range(NBLK):
            for dom, tpos, sidx in m["tiles_by_block"][b]:
                cnt[c, b, 0 if dom == "lo" else 1] += 1
    mx = cnt.max(axis=0)

    new = []
    for c, m in enumerate(cores_meta):
        lo_parts, hi_parts, s_parts = [], [], []
        tiles_by_block = [[] for _ in range(NBLK)]
        lo_idx, hi_idx = m["lo_idx"], m["hi_idx"]
        s_all = m["s_sb"].reshape(128, -1, W)
        lo_nt, hi_nt = 0, 0
        s_n = 0
        for b in range(NBLK):
            for di, dom in enumerate(("lo", "hi")):
                have = [t for t in m["tiles_by_block"][b] if t[0] == dom]
                need = int(mx[b, di])
                for k in range(need):
                    if k < len(have):
                        _, tpos, sidx = have[k]
                        idx_arr = (lo_idx if dom == "lo" else hi_idx)[
                            tpos * 128:(tpos + 1) * 128]
                        s_mat = s_all[:, sidx, :]
                    else:
                        idx_arr = np.zeros(128, np.int64)
                        s_mat = np.zeros((128, W), ml_dtypes.bfloat16)
                    (lo_parts if dom == "lo" else hi_parts).append(idx_arr)
                    s_parts.append(np.asarray(s_mat))
                    nt = lo_nt if dom == "lo" else hi_nt
                    tiles_by_block[b].append((dom, nt, s_n))
                    s_n += 1
                    if dom == "lo":
                        lo_nt += 1
                    else:
                        hi_nt += 1
        lo_cat = (np.concatenate(lo_parts) if lo_parts
                  else np.zeros(0, np.int64))
        hi_cat = (np.concatenate(hi_parts) if hi_parts
                  else np.zeros(0, np.int64))
        s_cat = (np.stack(s_parts) if s_parts
                 else np.zeros((0, 128, W), ml_dtypes.bfloat16))
        s_sb = np.ascontiguousarray(
            np.asarray(s_cat).transpose(1, 0, 2).reshape(128, -1))
        new.append(dict(
            lo_idx=lo_cat, hi_idx=hi_cat, s_sb=s_sb,
            tiles_by_block=tiles_by_block,
            n_lo=lo_cat.shape[0], n_hi=hi_cat.shape[0], ntiles=s_n,
            node_of_row=m["node_of_row"], selfw=m["selfw"],
        ))
    return new


def _build_uniform(meta0, num_iter=NUM_ITER):
    n_lo, n_hi, ntiles = meta0["n_lo"], meta0["n_hi"], meta0["ntiles"]
    tiles_by_block = meta0["tiles_by_block"]

    nc = bacc.Bacc("TRN2", target_bir_lowering=False, debug=False,
                   num_devices=NC, num_swdge_queues=NQ)

    embtab = nc.dram_tensor("embtab", [NUM_ATOM_FEATS * 128, D], BF16,
                            kind="ExternalInput")
    oh_d = nc.dram_tensor("oh", [128, NCOL * NUM_ATOM_FEATS * 128], BF16,
                          kind="ExternalInput")
    ws = nc.dram_tensor("ws", [NUM_LAYER * D, D], F32, kind="ExternalInput")
    bs = nc.dram_tensor("bs", [NUM_LAYER, D], F32, kind="ExternalInput")
    ident = nc.dram_tensor("ident", [128, 128], F32, kind="ExternalInput")
    identb_d = nc.dram_tensor("identb", [128, 128], BF16,
                              kind="ExternalInput")
    selfw_d = nc.dram_tensor("selfw", [128, NCOL], F32, kind="ExternalInput")
    idx_lo_d = nc.dram_tensor("idx_lo", [128, max(n_lo, 16) // 16], I16,
                              kind="ExternalInput")
    idx_hi_d = nc.dram_tensor("idx_hi", [128, max(n_hi, 16) // 16], I16,
                              kind="ExternalInput")
    s_d = nc.dram_tensor("s", [128, max(ntiles, 1) * W], BF16,
                         kind="ExternalInput")
    out_d = nc.dram_tensor("out", [SHARD_PAD, D], F32, kind="ExternalOutput")

    ag_in_a = [nc.dram_tensor(f"ag_in_a{i}", [ROWS_A, D], BF16,
                              kind="Internal") for i in range(2)]
    ag_in_b = [nc.dram_tensor(f"ag_in_b{i}", [ROWS_B, D], BF16,
                              kind="Internal") for i in range(2)]
    ag_out_a = [nc.dram_tensor(f"ag_out_a{i}", [N_A, D], BF16,
                               kind="Internal", addr_space="Shared")
                for i in range(2)]
    ag_out_b = [nc.dram_tensor(f"ag_out_b{i}", [N_B, D], BF16,
                               kind="Internal", addr_space="Shared")
                for i in range(2)]

    def _emit_ag(which, buf):
        src = (ag_in_a if which == "a" else ag_in_b)[buf]
        dst = (ag_out_a if which == "a" else ag_out_b)[buf]
        return nc.gpsimd.collective_compute(
            "AllGather", mybir.AluOpType.bypass,
            replica_groups=[list(range(NC))],
            ins=[src[:].opt()], outs=[dst[:].opt()])

    lo_chunks = _chunks(n_lo)
    hi_chunks = _chunks(n_hi)

    with tile.TileContext(nc) as tc:
      with tc.tile_pool(name="persist", bufs=1) as persist:
        h_sb = persist.tile([128, NCOL, D], BF16, tag="h")
        h0s = persist.tile([128, NCOL, D], BF16, tag="h0s")
        selfw = persist.tile([128, NCOL], F32, tag="selfw")
        nc.sync.dma_start(selfw[:], selfw_d[:])
        identb = persist.tile([128, 128], BF16, tag="identb")
        nc.sync.dma_start(identb[:], identb_d[:])

        # ---------------- prologue: one-hot embedding + MLP ----------------
        with (
            tc.tile_pool(name="pro", bufs=1) as pro,
            tc.tile_pool(name="mlp", bufs=3) as mlp_pool,
            tc.tile_pool(name="prps", bufs=2, space="PSUM") as prps,
        ):
            emb_sb = pro.tile([128, NUM_ATOM_FEATS, D], BF16, tag="emb")
            nc.sync.dma_start(
                emb_sb[:],
                embtab[:, :].rearrange("(f p) d -> p f d", p=128))
            idn = pro.tile([128, 128], F32, tag="idn")
            nc.sync.dma_start(idn[:], ident[:])
            w_sb = pro.tile([128, NUM_LAYER * D], F32, tag="w")
            b_sb = pro.tile([128, NUM_LAYER], F32, tag="b")
            for l in range(NUM_LAYER):
                nc.sync.dma_start(w_sb[:, l * D:(l + 1) * D],
                                  ws[l * D:(l + 1) * D, :])
                nc.sync.dma_start(b_sb[:, l:l + 1],
                                  bs[l:l + 1, :].rearrange("a k -> k a"))
            oh_sb = pro.tile([128, NCOL * NUM_ATOM_FEATS * 128], BF16,
                             tag="oh")
            for col in range(NCOL):
                o = col * NUM_ATOM_FEATS * 128
                nc.sync.dma_start(oh_sb[:, o:o + NUM_ATOM_FEATS * 128],
                                  oh_d[:, o:o + NUM_ATOM_FEATS * 128])

            for col in range(NCOL):
                o = col * NUM_ATOM_FEATS * 128
                ps = prps.tile([128, 128], F32, tag="ps")
                for f in range(NUM_ATOM_FEATS):
                    nc.tensor.matmul(
                        ps[:], emb_sb[:, f, :],
                        oh_sb[:, o + f * 128:o + (f + 1) * 128],
                        start=(f == 0), stop=(f == NUM_ATOM_FEATS - 1))
                cur = mlp_pool.tile([128, 128], F32, tag="t")
                nc.scalar.activation(cur[:], ps[:], AF.Copy)
                for l in range(NUM_LAYER):
                    ps2 = prps.tile([128, 128], F32, tag="ps2")
                    nc.tensor.matmul(ps2[:], w_sb[:, l * D:(l + 1) * D],
                                     cur[:], start=True, stop=True)
                    cur = mlp_pool.tile([128, 128], F32, tag="t")
                    nc.scalar.activation(
                        cur[:], ps2[:],
                        AF.Relu if l != NUM_LAYER - 1 else AF.Identity,
                        bias=b_sb[:, l:l + 1])
                # transpose back: h [nodes, d]
                pt = prps.tile([128, 128], F32, tag="pt")
                nc.tensor.transpose(pt[:], cur[:], idn[:])
                nc.scalar.activation(h_sb[:, col, :], pt[:], AF.Copy)
                nc.scalar.activation(h0s[:, col, :], pt[:], AF.Copy,
                                     scale=ALPHA)
                if col == COLS_A - 1:
                    nc.sync.dma_start(
                        ag_in_a[0][:].rearrange("(c p) f -> p c f", p=128),
                        h_sb[:, 0:COLS_A, :])
                    _emit_ag("a", 0)
                elif col == NCOL - 1:
                    nc.sync.dma_start(
                        ag_in_b[0][:].rearrange("(c p) f -> p c f", p=128),
                        h_sb[:, COLS_A:NCOL, :])

        # ---------------- main loop ----------------
        with (
            tc.tile_pool(name="sconst", bufs=1) as sconst,
            tc.tile_pool(name="glo", bufs=18) as glo_pool,
            tc.tile_pool(name="ghi", bufs=12) as ghi_pool,
            tc.tile_pool(name="ps", bufs=8, space="PSUM") as ps_pool,
            tc.tile_pool(name="stage", bufs=2) as stage_pool,
        ):
            s_sb = sconst.tile([128, max(ntiles, 1) * W], BF16, tag="s")
            nc.sync.dma_start(s_sb[:], s_d[:])
            ilo = sconst.tile([128, max(n_lo, 16) // 16], I16, tag="ilo")
            nc.sync.dma_start(ilo[:], idx_lo_d[:])
            ihi = sconst.tile([128, max(n_hi, 16) // 16], I16, tag="ihi")
            nc.sync.dma_start(ihi[:], idx_hi_d[:])

            qe_g = [0]
            LOP = 12  # lo chunks of iter t+1 emitted before AG_b(t)

            def emit_gather(dom, ci, buf):
                (o, n) = (lo_chunks if dom == "lo" else hi_chunks)[ci]
                pool = glo_pool if dom == "lo" else ghi_pool
                view = (ag_out_a if dom == "lo" else ag_out_b)[buf][:, :]
                isb = ilo if dom == "lo" else ihi
                g = pool.tile([128, 8, D], BF16, tag="g" + dom)
                gi = nc.gpsimd.dma_gather(
                    g[:, 0:n // 128, :], view,
                    isb[:, o // 16:(o + n) // 16], n, n, D,
                    queue_num=qe_g[0] % NQ)
                qe_g[0] += 1
                return g, gi

            pending = None  # prefix state for the next iteration
            for it in range(num_iter):
                buf = it % 2
                if pending is None:
                    lo_tiles_bufs = {}
                    lo_insts = []
                    for ci in range(min(LOP, len(lo_chunks))):
                        g, gi = emit_gather("lo", ci, buf)
                        lo_tiles_bufs[ci] = g
                        lo_insts.append(gi)
                    # prologue staged table b; trigger its AllGather now so
                    # the wire overlaps the prefix gathers above
                    _emit_ag("b", 0)
                else:
                    lo_tiles_bufs, lo_insts = pending

                hi_tiles_bufs = {}
                gath_insts = []
                hi_insts = []
                order = []
                li, hii = min(LOP, len(lo_chunks)), 0
                while li < len(lo_chunks) or hii < len(hi_chunks):
                    if li < len(lo_chunks):
                        order.append(("lo", li)); li += 1
                    if hii < len(hi_chunks):
                        order.append(("hi", hii)); hii += 1
                for dom, ci in order:
                    g, gi = emit_gather(dom, ci, buf)
                    gath_insts.append(gi)
                    (lo_insts if dom == "lo" else hi_insts).append(gi)
                    (lo_tiles_bufs if dom == "lo" else hi_tiles_bufs)[ci] = g

                # soft absorb: schedule the first hi chunks after a few
                # post-prefix lo chunks so the engine doesn't park on the
                # AG_b wait while runnable lo gathers sit behind it
                for j in range(min(6, len(hi_insts))):
                    anchor = min(LOP + 2 + 2 * j, len(lo_insts) - 1)
                    dd = InstructionNameOrderedSet()
                    dd.add(lo_insts[anchor].ins.name)
                    hi_insts[j].ins.add_nosync_dependencies_from(dd)

                last = it == num_iter - 1
                for p in range(NBLK // 2):
                    col = p
                    psum = ps_pool.tile([128, D], F32, tag="ps")
                    for half in range(2):
                        tl = tiles_by_block[2 * p + half]
                        ph = half * 64
                        nc.tensor.matmul(
                            psum[ph:ph + 64, :],
                            identb[:, ph:ph + 64],
                            h0s[:, col, :],
                            start=True, stop=(len(tl) == 0))
                        for j, (dom, tpos, sidx) in enumerate(tl):
                            bufs = (lo_tiles_bufs if dom == "lo"
                                    else hi_tiles_bufs)
                            g = bufs[tpos // 8]
                            nc.tensor.matmul(
                                psum[ph:ph + 64, :],
                                s_sb[:, sidx * W:(sidx + 1) * W],
                                g[:, tpos % 8, :],
                                start=False, stop=(j == len(tl) - 1))
                    # evict: h_new = selfw * h_old + psum  (one DVE op)
                    if last:
                        st = stage_pool.tile([128, D], F32, tag="st")
                        nc.vector.scalar_tensor_tensor(
                            st[:], h_sb[:, col, :], selfw[:, col:col + 1],
                            psum[:, :], mybir.AluOpType.mult,
                            mybir.AluOpType.add)
                        nc.sync.dma_start(
                            out_d[p * 128:(p + 1) * 128, :], st[:])
                    else:
                        nc.vector.scalar_tensor_tensor(
                            h_sb[:, col, :], h_sb[:, col, :],
                            selfw[:, col:col + 1],
                            psum[:, :], mybir.AluOpType.mult,
                            mybir.AluOpType.add)
                        if p == COLS_A - 1:
                            nc.sync.dma_start(
                                ag_in_a[1 - buf][:].rearrange(
                                    "(c p) f -> p c f", p=128),
                                h_sb[:, 0:COLS_A, :])
                            ag_a = _emit_ag("a", 1 - buf)
                            # pin the AG trigger into the GpSimd gather
                            # stream at ~75% so its wire time overlaps the
                            # remaining gathers (GpSimd is the only engine
                            # that can trigger collectives)
                            gpos = (len(gath_insts) * 3) // 4
                            d1 = InstructionNameOrderedSet()
                            d1.add(gath_insts[gpos].ins.name)
                            ag_a.ins.add_nosync_dependencies_from(d1)
                            d2 = InstructionNameOrderedSet()
                            d2.add(ag_a.ins.name)
                            gath_insts[gpos + 1].ins.add_nosync_dependencies_from(d2)

                # software pipelining: emit the next iteration's first LOP
                # lo gathers (they only need AG_a of this iteration) BEFORE
                # staging/triggering AG_b, so they run during AG_b's wire
                # instead of idling at the iteration boundary.
                if not last:
                    nbuf = 1 - buf
                    np_bufs = {}
                    np_insts = []
                    for ci in range(min(LOP, len(lo_chunks))):
                        g, gi = emit_gather("lo", ci, nbuf)
                        np_bufs[ci] = g
                        np_insts.append(gi)
                    pending = (np_bufs, np_insts)
                    nc.sync.dma_start(
                        ag_in_b[nbuf][:].rearrange(
                            "(c p) f -> p c f", p=128),
                        h_sb[:, COLS_A:NCOL, :])
                    _emit_ag("b", nbuf)

    # Post-scheduling: align each gather's SWDGE queue with its DMASW lane
    # (lanes are assigned round-robin in scheduled order and their sems are
    # queue-locked in ucode, so queue must follow lane, not emission order).
    import re as _re
    for _blk in nc.m.functions[0].blocks:
        for _inst in _blk.instructions:
            if isinstance(_inst, mybir.InstDMAGatherAnt):
                _si = _inst.sync_info
                _lane = None
                for _u in (_si.on_update if _si else []):
                    _m = _re.match(r"DMASW(\d+)_", _u.ant_name or "")
                    if _m:
                        _lane = int(_m.group(1))
                if _lane is not None:
                    _inst.queue_num = _lane % NQ
    nc.compile()
    return nc


_CACHE = {}


def _get_compiled(edge_index, num_iter=NUM_ITER):
    key = (hash(np.asarray(edge_index).tobytes()), num_iter)
    if key not in _CACHE:
        cores = _preprocess(edge_index)
        cores = _equalize(cores)
        nc = _build_uniform(cores[0], num_iter=num_iter)
        _CACHE[key] = (nc, cores)
    return _CACHE[key]


def _make_in_maps(x, atom_emb, Ws, bs, cores_meta):
    x = np.asarray(x)
    emb_pad = np.zeros((NUM_ATOM_FEATS * 128, D), ml_dtypes.bfloat16)
    ae = np.asarray(atom_emb, dtype=np.float32)
    for f in range(NUM_ATOM_FEATS):
        emb_pad[f * 128:f * 128 + ATOM_VOCAB] = ae[f]
    ws_t = np.ascontiguousarray(
        np.asarray(Ws, dtype=np.float32).reshape(NUM_LAYER * D, D))
    bs_t = np.ascontiguousarray(np.asarray(bs, dtype=np.float32))
    ident = np.eye(128, dtype=np.float32)
    identb = np.eye(128, dtype=ml_dtypes.bfloat16)

    in_maps = []
    for c, m in enumerate(cores_meta):
        nr = m["node_of_row"]  # [SHARD_PAD] local node or -1
        # one-hot: [128 vocab-pad, NCOL*9*128] with oh[v, (col,f,n)] = 1
        oh = np.zeros((128, NCOL * NUM_ATOM_FEATS * 128), ml_dtypes.bfloat16)
        xs = x[c * SHARD:(c + 1) * SHARD]  # [SHARD, 9]
        rows = np.arange(SHARD_PAD)
        valid = nr >= 0
        for f in range(NUM_ATOM_FEATS):
            vals = np.zeros(SHARD_PAD, np.int64)
            vals[valid] = xs[nr[valid], f]
            cols = (rows // 128) * NUM_ATOM_FEATS * 128 + f * 128 + rows % 128
            oh[vals[valid], cols[valid]] = 1.0
        lo = m["lo_idx"] if m["n_lo"] else np.zeros(16, np.int64)
        hi_ = m["hi_idx"] if m["n_hi"] else np.zeros(16, np.int64)
        in_maps.append({
            "embtab": emb_pad,
            "oh": np.ascontiguousarray(oh),
            "ws": ws_t,
            "bs": bs_t,
            "ident": ident,
            "identb": identb,
            "selfw": np.ascontiguousarray(m["selfw"]),
            "idx_lo": _wrap_idxs(lo),
            "idx_hi": _wrap_idxs(hi_),
            "s": np.ascontiguousarray(m["s_sb"]),
        })
    return in_maps


def _unpermute(res, cores_meta):
    out = np.zeros((N_NODES, D), np.float32)
    for c, m in enumerate(cores_meta):
        nr = m["node_of_row"]
        valid = nr >= 0
        r = np.asarray(res[c]["out"], dtype=np.float32)
        out[c * SHARD + nr[valid]] = r[valid]
    return out


def kernel(x, edge_index, atom_emb, Ws, bs):
    nc, cores_meta = _get_compiled(edge_index)
    in_maps = _make_in_maps(x, atom_emb, Ws, bs, cores_meta)
    res = run_bass_kernel_spmd(nc, in_maps, core_ids=list(range(NC)))
    return np.ascontiguousarray(_unpermute(res.results, cores_meta))


def run_profiled(x, edge_index, atom_emb, Ws, bs):
    import ntff_hook
    ntff_hook.install()
    nc, cores_meta = _get_compiled(edge_index)
    in_maps = _make_in_maps(x, atom_emb, Ws, bs, cores_meta)
    res = run_bass_kernel_spmd(nc, in_maps, core_ids=list(range(NC)),
                               trace=True)
    return (np.ascontiguousarray(_unpermute(res.results, cores_meta)),
            res.exec_time_ns)


# revision 5
# speedup vs baseline: 1.0962x; 1.0102x over previous
"""APPNP (GCN-normalized propagation) distributed Bass kernel for 8 TRN2 cores.

v2 strategy (dst-sharded message passing, gather-descriptor-optimized):
  - Nodes sharded across 8 cores (6250/core, padded to 6272 = 49*128 rows).
  - Per-core node->row permutation balances per-(block,dom) in-edge counts
    across cores so SPMD equalization padding is minimal.
  - Prologue: atom embedding via one-hot matmuls (stationary = padded
    embedding table, moving = host-built one-hot of x) directly producing
    hT; 3-layer MLP in transposed space; PE transpose back -> h (bf16),
    h0s = 0.1*h.
  - Exchange: two AllGathers per iteration into DRAM tables
    a (rows 0:2560/core -> 20480 rows) and b (rows 2560:6272 -> 29696 rows),
    both < 32768 so int16 gather indices reach everything. a is issued
    mid-iteration (after block-pair 19), b at the end; the next iteration
    issues ~40 a-sourced chunks first so b's wire time is absorbed.
  - Per iteration: dma_gather h[src] for in-edges grouped in 64-wide dst
    blocks (128-slot tiles), TensorEngine segment-sum via one-hot S
    matrices (bf16, SBUF-resident) in PSUM; h0s injected via identity
    matmul; self-loops are NOT slots: the Vector engine evicts PSUM with
    h_new = selfw * h_old + psum in one scalar_tensor_tensor op.
"""

import numpy as np
import ml_dtypes

import concourse.bacc as bacc
import concourse.bass as bass
import concourse.mybir as mybir
import concourse.tile as tile
from concourse.bass_utils import run_bass_kernel_spmd
from concourse.instruction_name_ordered_set import InstructionNameOrderedSet

# Problem constants (must match reference.py)
N_NODES = 50000
N_EDGES = 800000
D = 128
NUM_ITER = 10
NUM_LAYER = 3
ALPHA = 0.1
NUM_ATOM_FEATS = 9
ATOM_VOCAB = 119

NC = 8
SHARD = N_NODES // NC            # 6250
SHARD_PAD = 6272                 # 49 * 128
NCOL = SHARD_PAD // 128          # 49
W = 64                           # dst block width
NBLK = SHARD_PAD // W            # 98
CHUNK = 1024                     # max idxs per dma_gather
COLS_A = 30                      # shard cols in exchange table a
ROWS_A = COLS_A * 128            # 3840
ROWS_B = SHARD_PAD - ROWS_A      # 2432
N_A = NC * ROWS_A                # 30720 (< 32768)
N_B = NC * ROWS_B                # 19456 (< 32768)
NQ = 4                           # SWDGE queues
ABSORB = 0                       # lo chunks issued before first hi chunk
CHAIN = False                    # chain gather emission order

BF16 = mybir.dt.bfloat16
F32 = mybir.dt.float32
I16 = mybir.dt.int16
AF = mybir.ActivationFunctionType


def _wrap_idxs(idx):
    """slot i -> partition i%16 (replicated x8), col i//16."""
    n = idx.shape[0]
    assert n % 16 == 0
    w = idx.reshape(n // 16, 16).T.astype(np.int16)
    return np.ascontiguousarray(np.tile(w, (8, 1)))


def _pad128(a, fill=0):
    n = a.shape[0]
    m = (-n) % 128
    if m == 0:
        return a
    return np.concatenate([a, np.full((m,) + a.shape[1:], fill, a.dtype)])


def _balance_perm(indeg):
    """Greedy LPT: assign 6250 local nodes to 98 blocks of <=64 nodes,
    balancing total in-degree per block. Returns node_of_row[6272] with -1
    for pad rows (all pads in the last block)."""
    import heapq
    order = np.argsort(-indeg, kind="stable")
    cap = np.full(NBLK, 64, np.int64)
    cap[NBLK - 1] = SHARD - 64 * (NBLK - 1)  # 42 real nodes in last block
    fill = [[] for _ in range(NBLK)]
    heap = [(0, b) for b in range(NBLK)]
    heapq.heapify(heap)
    for nid in order:
        while True:
            tot, b = heapq.heappop(heap)
            if len(fill[b]) < cap[b]:
                break
        fill[b].append(nid)
        if len(fill[b]) < cap[b]:
            heapq.heappush(heap, (tot + int(indeg[nid]), b))
    node_of_row = np.full(SHARD_PAD, -1, np.int64)
    for b in range(NBLK):
        for j, nid in enumerate(fill[b]):
            node_of_row[b * 64 + j] = nid
    return node_of_row


def _preprocess(edge_index):
    """Host-side graph preprocessing -> per-core structures."""
    src = np.asarray(edge_index[0], dtype=np.int64)
    dst = np.asarray(edge_index[1], dtype=np.int64)
    deg = np.bincount(dst, minlength=N_NODES).astype(np.float64) + 1.0
    dinv = 1.0 / np.sqrt(deg)
    coef = ((1.0 - ALPHA) * dinv[src] * dinv[dst]).astype(np.float32)
    selfw_g = ((1.0 - ALPHA) * dinv * dinv).astype(np.float32)

    # pass A: per-core balanced permutation (total in-degree)
    indeg_all = np.bincount(dst, minlength=N_NODES)
    node_of_row = np.zeros((NC, SHARD_PAD), np.int64)
    row_of_node = np.zeros(N_NODES, np.int64)  # global node -> local row
    for c in range(NC):
        nr = _balance_perm(indeg_all[c * SHARD:(c + 1) * SHARD])
        node_of_row[c] = nr
        valid = nr >= 0
        row_of_node[c * SHARD + nr[valid]] = np.nonzero(valid)[0]

    # pass B: with src sides fixed by pass A, pack nodes into blocks so each
    # (block, dom) in-edge count lands just UNDER a multiple of 128 (the
    # gather-tile quantum) and aligns across cores. Targets are global (the
    # max core's totals) so SPMD equalization adds almost nothing.
    src_isa = (row_of_node[src] % SHARD_PAD) < ROWS_A
    lo_in = np.bincount(dst[src_isa], minlength=N_NODES)
    hi_in = np.bincount(dst[~src_isa], minlength=N_NODES)
    RESID = 118  # target residue mod 128 (margin 10 to the next tile)

    def _targets(total_max, nb):
        base_q = max(0, int((total_max / nb - RESID) // 128))
        t = np.full(nb, base_q * 128 + RESID, np.float64)
        k = 0
        while t.sum() < total_max and k < nb:
            t[k] += 128
            k += 1
        while t.sum() < total_max:
            t += 128
        return t

    side_meta = []
    for side in (0, 1):
        if side == 0:
            blocks = list(range(0, ROWS_A // W))
        else:
            blocks = list(range(ROWS_A // W, NBLK))
        lmax = hmax = 0.0
        for c in range(NC):
            rows0 = 0 if side == 0 else ROWS_A
            nrows = ROWS_A if side == 0 else ROWS_B
            nodes = node_of_row[c][rows0:rows0 + nrows]
            nodes = nodes[nodes >= 0]
            lmax = max(lmax, lo_in[c * SHARD + nodes].sum())
            hmax = max(hmax, hi_in[c * SHARD + nodes].sum())
        nb = len(blocks)
        side_meta.append((blocks, _targets(lmax, nb), _targets(hmax, nb)))

    for c in range(NC):
        nr_new = np.full(SHARD_PAD, -1, np.int64)
        for side in (0, 1):
            blocks, T_lo, T_hi = side_meta[side]
            rows0 = 0 if side == 0 else ROWS_A
            nrows = ROWS_A if side == 0 else ROWS_B
            old_nodes = node_of_row[c][rows0:rows0 + nrows]
            old_nodes = old_nodes[old_nodes >= 0]
            li = lo_in[c * SHARD + old_nodes].astype(np.float64)
            hi_ = hi_in[c * SHARD + old_nodes].astype(np.float64)
            nb = len(blocks)
            caps = np.array([64 if b != NBLK - 1 else
                             SHARD - 64 * (NBLK - 1) for b in blocks])
            order_n = np.argsort(-(li + hi_), kind="stable")
            cur = np.zeros((nb, 2))
            cnt = np.zeros(nb, np.int64)
            assign = np.zeros(len(old_nodes), np.int64)
            for j in order_n:
                cost = np.maximum((cur[:, 0] + li[j]) / T_lo,
                                  (cur[:, 1] + hi_[j]) / T_hi)
                cost[cnt >= caps] = np.inf
                bsel = int(np.argmin(cost))
                assign[j] = bsel
                cur[bsel, 0] += li[j]
                cur[bsel, 1] += hi_[j]
                cnt[bsel] += 1
            # swap refinement: push overshoot (beyond targets) to zero
            def over(cb):
                return (max(0.0, cb[0]) + max(0.0, cb[1]))
            ex = cur - np.stack([T_lo, T_hi], axis=1)
            rng = np.random.default_rng(c)
            for _ in range(4):
                bad = np.nonzero((ex[:, 0] > 0) | (ex[:, 1] > 0))[0]
                if bad.size == 0:
                    break
                improved = False
                for b1 in bad:
                    js = np.nonzero(assign == b1)[0]
                    cands = rng.permutation(nb)[:20]
                    done = False
                    for b2 in cands:
                        if b2 == b1:
                            continue
                        for j1 in js[np.argsort(-(li[js] + hi_[js]))][:12]:
                            js2 = np.nonzero(assign == b2)[0]
                            if js2.size == 0:
                                continue
                            d1 = np.array([li[j1], hi_[j1]])
                            base = (over(ex[b1]) + over(ex[b2]))
                            d2s = np.stack([li[js2], hi_[js2]], axis=1)
                            nb1 = ex[b1] - d1 + d2s
                            nb2 = ex[b2] + d1 - d2s
                            costs = (np.maximum(nb1, 0).sum(axis=1) +
                                     np.maximum(nb2, 0).sum(axis=1))
                            kk = int(np.argmin(costs))
                            if costs[kk] < base - 0.5:
                                j2 = js2[kk]
                                ex[b1] = nb1[kk]
                                ex[b2] = nb2[kk]
                                cur[b1] += d2s[kk] - d1
                                cur[b2] += d1 - d2s[kk]
                                assign[j1], assign[j2] = b2, b1
                                improved = True
                                done = True
                                break
                        if done:
                            break
                if not improved:
                    break
            for bi, b in enumerate(blocks):
                nodes_b = old_nodes[assign == bi]
                for j2, nid in enumerate(nodes_b):
                    nr_new[b * 64 + j2] = nid
        node_of_row[c] = nr_new
        valid = nr_new >= 0
        row_of_node[c * SHARD + nr_new[valid]] = np.nonzero(valid)[0]

    # edge srow (exchange-table row of the source)
    src_core = src // SHARD
    r = row_of_node[src]
    isa = r < ROWS_A
    srow = np.where(isa, src_core * ROWS_A + r,
                    src_core * ROWS_B + (r - ROWS_A))

    cores = []
    for c in range(NC):
        m = (dst >= c * SHARD) & (dst < (c + 1) * SHARD)
        ldr = row_of_node[dst[m]]  # local row of each in-edge's dst
        lsrow = srow[m]
        lcoef = coef[m]
        lisa = isa[m]
        blk = ldr // W
        off = ldr % W

        streams = {"lo": [], "hi": []}
        s_tiles = []
        tiles_by_block = [[] for _ in range(NBLK)]
        stream_ntiles = {"lo": 0, "hi": 0}
        for b in range(NBLK):
            bm = blk == b
            for dom, dm in (("lo", lisa), ("hi", ~lisa)):
                sel = bm & dm
                n = int(sel.sum())
                if n == 0:
                    continue
                idx = _pad128(lsrow[sel].astype(np.int64))
                cf = _pad128(lcoef[sel])
                of = _pad128(off[sel].astype(np.int64))
                ntile = idx.shape[0] // 128
                for t in range(ntile):
                    s = np.zeros((128, W), np.float32)
                    s[np.arange(128), of[t * 128:(t + 1) * 128]] = \
                        cf[t * 128:(t + 1) * 128]
                    tiles_by_block[b].append((dom, stream_ntiles[dom] + t,
                                              len(s_tiles)))
                    s_tiles.append(s)
                streams[dom].append(idx)
                stream_ntiles[dom] += ntile

        lo_idx = (np.concatenate(streams["lo"]) if streams["lo"]
                  else np.zeros(0, np.int64))
        hi_idx = (np.concatenate(streams["hi"]) if streams["hi"]
                  else np.zeros(0, np.int64))
        s_all = (np.stack(s_tiles) if s_tiles
                 else np.zeros((0, 128, W), np.float32))
        s_sb = np.ascontiguousarray(
            s_all.transpose(1, 0, 2).reshape(128, -1)).astype(ml_dtypes.bfloat16)
        # selfw per row [128, NCOL]
        sw = np.zeros(SHARD_PAD, np.float32)
        nr = node_of_row[c]
        valid = nr >= 0
        sw[valid] = selfw_g[c * SHARD + nr[valid]]
        cores.append(dict(
            lo_idx=lo_idx, hi_idx=hi_idx, s_sb=s_sb,
            tiles_by_block=tiles_by_block,
            n_lo=lo_idx.shape[0], n_hi=hi_idx.shape[0],
            ntiles=len(s_tiles),
            node_of_row=node_of_row[c],
            selfw=np.ascontiguousarray(
                sw.reshape(NCOL, 128).T),  # [128, NCOL]
        ))
    return cores


def _chunks(total):
    out = []
    o = 0
    while o < total:
        n = min(CHUNK, total - o)
        out.append((o, n))
        o += n
    return out


def _equalize(cores_meta):
    """Pad per-block/dom tile counts to the max across cores (SPMD)."""
    cnt = np.zeros((NC, NBLK, 2), np.int64)
    for c, m in enumerate(cores_meta):
        for b in range(NBLK):
            for dom, tpos, sidx in m["tiles_by_block"][b]:
                cnt[c, b, 0 if dom == "lo" else 1] += 1
    mx = cnt.max(axis=0)

    new = []
    for c, m in enumerate(cores_meta):
        lo_parts, hi_parts, s_parts = [], [], []
        tiles_by_block = [[] for _ in range(NBLK)]
        lo_idx, hi_idx = m["lo_idx"], m["hi_idx"]
        s_all = m["s_sb"].reshape(128, -1, W)
        lo_nt, hi_nt = 0, 0
        s_n = 0
        for b in range(NBLK):
            for di, dom in enumerate(("lo", "hi")):
                have = [t for t in m["tiles_by_block"][b] if t[0] == dom]
                need = int(mx[b, di])
                for k in range(need):
                    if k < len(have):
                        _, tpos, sidx = have[k]
                        idx_arr = (lo_idx if dom == "lo" else hi_idx)[
                            tpos * 128:(tpos + 1) * 128]
                        s_mat = s_all[:, sidx, :]
                    else:
                        idx_arr = np.zeros(128, np.int64)
                        s_mat = np.zeros((128, W), ml_dtypes.bfloat16)
                    (lo_parts if dom == "lo" else hi_parts).append(idx_arr)
                    s_parts.append(np.asarray(s_mat))
                    nt = lo_nt if dom == "lo" else hi_nt
                    tiles_by_block[b].append((dom, nt, s_n))
                    s_n += 1
                    if dom == "lo":
                        lo_nt += 1
                    else:
                        hi_nt += 1
        lo_cat = (np.concatenate(lo_parts) if lo_parts
                  else np.zeros(0, np.int64))
        hi_cat = (np.concatenate(hi_parts) if hi_parts
                  else np.zeros(0, np.int64))
        s_cat = (np.stack(s_parts) if s_parts
                 else np.zeros((0, 128, W), ml_dtypes.bfloat16))
        s_sb = np.ascontiguousarray(
            np.asarray(s_cat).transpose(1, 0, 2).reshape(128, -1))
        new.append(dict(
            lo_idx=lo_cat, hi_idx=hi_cat, s_sb=s_sb,
            tiles_by_block=tiles_by_block,
            n_lo=lo_cat.shape[0], n_hi=hi_cat.shape[0], ntiles=s_n,
            node_of_row=m["node_of_row"], selfw=m["selfw"],
        ))
    return new


def _build_uniform(meta0, num_iter=NUM_ITER):
    n_lo, n_hi, ntiles = meta0["n_lo"], meta0["n_hi"], meta0["ntiles"]
    tiles_by_block = meta0["tiles_by_block"]

    nc = bacc.Bacc("TRN2", target_bir_lowering=False, debug=False,
                   num_devices=NC, num_swdge_queues=NQ)

    embtab = nc.dram_tensor("embtab", [NUM_ATOM_FEATS * 128, D], BF16,
                            kind="ExternalInput")
    oh_d = nc.dram_tensor("oh", [128, NCOL * NUM_ATOM_FEATS * 128], BF16,
                          kind="ExternalInput")
    ws = nc.dram_tensor("ws", [NUM_LAYER * D, D], F32, kind="ExternalInput")
    bs = nc.dram_tensor("bs", [NUM_LAYER, D], F32, kind="ExternalInput")
    ident = nc.dram_tensor("ident", [128, 128], F32, kind="ExternalInput")
    identb_d = nc.dram_tensor("identb", [128, 128], BF16,
                              kind="ExternalInput")
    selfw_d = nc.dram_tensor("selfw", [128, NCOL], F32, kind="ExternalInput")
    idx_lo_d = nc.dram_tensor("idx_lo", [128, max(n_lo, 16) // 16], I16,
                              kind="ExternalInput")
    idx_hi_d = nc.dram_tensor("idx_hi", [128, max(n_hi, 16) // 16], I16,
                              kind="ExternalInput")
    s_d = nc.dram_tensor("s", [128, max(ntiles, 1) * W], BF16,
                         kind="ExternalInput")
    out_d = nc.dram_tensor("out", [SHARD_PAD, D], F32, kind="ExternalOutput")

    ag_in_a = [nc.dram_tensor(f"ag_in_a{i}", [ROWS_A, D], BF16,
                              kind="Internal") for i in range(2)]
    ag_in_b = [nc.dram_tensor(f"ag_in_b{i}", [ROWS_B, D], BF16,
                              kind="Internal") for i in range(2)]
    ag_out_a = [nc.dram_tensor(f"ag_out_a{i}", [N_A, D], BF16,
                               kind="Internal", addr_space="Shared")
                for i in range(2)]
    ag_out_b = [nc.dram_tensor(f"ag_out_b{i}", [N_B, D], BF16,
                               kind="Internal", addr_space="Shared")
                for i in range(2)]

    def _emit_ag(which, buf):
        src = (ag_in_a if which == "a" else ag_in_b)[buf]
        dst = (ag_out_a if which == "a" else ag_out_b)[buf]
        return nc.gpsimd.collective_compute(
            "AllGather", mybir.AluOpType.bypass,
            replica_groups=[list(range(NC))],
            ins=[src[:].opt()], outs=[dst[:].opt()])

    lo_chunks = _chunks(n_lo)
    hi_chunks = _chunks(n_hi)

    with tile.TileContext(nc) as tc:
      with tc.tile_pool(name="persist", bufs=1) as persist:
        h_sb = persist.tile([128, NCOL, D], BF16, tag="h")
        h0s = persist.tile([128, NCOL, D], BF16, tag="h0s")
        selfw = persist.tile([128, NCOL], F32, tag="selfw")
        nc.sync.dma_start(selfw[:], selfw_d[:])
        identb = persist.tile([128, 128], BF16, tag="identb")
        nc.sync.dma_start(identb[:], identb_d[:])

        # ---------------- prologue: one-hot embedding + MLP ----------------
        with (
            tc.tile_pool(name="pro", bufs=1) as pro,
            tc.tile_pool(name="mlp", bufs=3) as mlp_pool,
            tc.tile_pool(name="prps", bufs=2, space="PSUM") as prps,
        ):
            emb_sb = pro.tile([128, NUM_ATOM_FEATS, D], BF16, tag="emb")
            nc.sync.dma_start(
                emb_sb[:],
                embtab[:, :].rearrange("(f p) d -> p f d", p=128))
            idn = pro.tile([128, 128], F32, tag="idn")
            nc.sync.dma_start(idn[:], ident[:])
            w_sb = pro.tile([128, NUM_LAYER * D], F32, tag="w")
            b_sb = pro.tile([128, NUM_LAYER], F32, tag="b")
            for l in range(NUM_LAYER):
                nc.sync.dma_start(w_sb[:, l * D:(l + 1) * D],
                                  ws[l * D:(l + 1) * D, :])
                nc.sync.dma_start(b_sb[:, l:l + 1],
                                  bs[l:l + 1, :].rearrange("a k -> k a"))
            oh_sb = pro.tile([128, NCOL * NUM_ATOM_FEATS * 128], BF16,
                             tag="oh")
            for col in range(NCOL):
                o = col * NUM_ATOM_FEATS * 128
                nc.sync.dma_start(oh_sb[:, o:o + NUM_ATOM_FEATS * 128],
                                  oh_d[:, o:o + NUM_ATOM_FEATS * 128])

            for col in range(NCOL):
                o = col * NUM_ATOM_FEATS * 128
                ps = prps.tile([128, 128], F32, tag="ps")
                for f in range(NUM_ATOM_FEATS):
                    nc.tensor.matmul(
                        ps[:], emb_sb[:, f, :],
                        oh_sb[:, o + f * 128:o + (f + 1) * 128],
                        start=(f == 0), stop=(f == NUM_ATOM_FEATS - 1))
                cur = mlp_pool.tile([128, 128], F32, tag="t")
                nc.scalar.activation(cur[:], ps[:], AF.Copy)
                for l in range(NUM_LAYER):
                    ps2 = prps.tile([128, 128], F32, tag="ps2")
                    nc.tensor.matmul(ps2[:], w_sb[:, l * D:(l + 1) * D],
                                     cur[:], start=True, stop=True)
                    cur = mlp_pool.tile([128, 128], F32, tag="t")
                    nc.scalar.activation(
                        cur[:], ps2[:],
                        AF.Relu if l != NUM_LAYER - 1 else AF.Identity,
                        bias=b_sb[:, l:l + 1])
                # transpose back: h [nodes, d]
                pt = prps.tile([128, 128], F32, tag="pt")
                nc.tensor.transpose(pt[:], cur[:], idn[:])
                nc.scalar.activation(h_sb[:, col, :], pt[:], AF.Copy)
                nc.scalar.activation(h0s[:, col, :], pt[:], AF.Copy,
                                     scale=ALPHA)
                if col == COLS_A - 1:
                    nc.sync.dma_start(
                        ag_in_a[0][:].rearrange("(c p) f -> p c f", p=128),
                        h_sb[:, 0:COLS_A, :])
                    _emit_ag("a", 0)
                elif col == NCOL - 1:
                    nc.sync.dma_start(
                        ag_in_b[0][:].rearrange("(c p) f -> p c f", p=128),
                        h_sb[:, COLS_A:NCOL, :])

        # ---------------- main loop ----------------
        with (
            tc.tile_pool(name="sconst", bufs=1) as sconst,
            tc.tile_pool(name="glo", bufs=18) as glo_pool,
            tc.tile_pool(name="ghi", bufs=12) as ghi_pool,
            tc.tile_pool(name="ps", bufs=8, space="PSUM") as ps_pool,
            tc.tile_pool(name="stage", bufs=2) as stage_pool,
        ):
            s_sb = sconst.tile([128, max(ntiles, 1) * W], BF16, tag="s")
            nc.sync.dma_start(s_sb[:], s_d[:])
            ilo = sconst.tile([128, max(n_lo, 16) // 16], I16, tag="ilo")
            nc.sync.dma_start(ilo[:], idx_lo_d[:])
            ihi = sconst.tile([128, max(n_hi, 16) // 16], I16, tag="ihi")
            nc.sync.dma_start(ihi[:], idx_hi_d[:])

            qe_g = [0]
            LOP = 12  # lo chunks of iter t+1 emitted before AG_b(t)

            def emit_gather(dom, ci, buf):
                (o, n) = (lo_chunks if dom == "lo" else hi_chunks)[ci]
                pool = glo_pool if dom == "lo" else ghi_pool
                view = (ag_out_a if dom == "lo" else ag_out_b)[buf][:, :]
                isb = ilo if dom == "lo" else ihi
                g = pool.tile([128, 8, D], BF16, tag="g" + dom)
                gi = nc.gpsimd.dma_gather(
                    g[:, 0:n // 128, :], view,
                    isb[:, o // 16:(o + n) // 16], n, n, D,
                    queue_num=qe_g[0] % NQ)
                qe_g[0] += 1
                return g, gi

            pending = None  # prefix state for the next iteration
            for it in range(num_iter):
                buf = it % 2
                if pending is None:
                    lo_tiles_bufs = {}
                    lo_insts = []
                    for ci in range(min(LOP, len(lo_chunks))):
                        g, gi = emit_gather("lo", ci, buf)
                        lo_tiles_bufs[ci] = g
                        lo_insts.append(gi)
                    # prologue staged table b; trigger its AllGather now so
                    # the wire overlaps the prefix gathers above
                    _emit_ag("b", 0)
                else:
                    lo_tiles_bufs, lo_insts = pending

                hi_tiles_bufs = {}
                gath_insts = []
                hi_insts = []
                order = []
                li, hii = min(LOP, len(lo_chunks)), 0
                while li < len(lo_chunks) or hii < len(hi_chunks):
                    if li < len(lo_chunks):
                        order.append(("lo", li)); li += 1
                    if hii < len(hi_chunks):
                        order.append(("hi", hii)); hii += 1
                for dom, ci in order:
                    g, gi = emit_gather(dom, ci, buf)
                    gath_insts.append(gi)
                    (lo_insts if dom == "lo" else hi_insts).append(gi)
                    (lo_tiles_bufs if dom == "lo" else hi_tiles_bufs)[ci] = g

                # soft absorb: schedule the first hi chunks after a few
                # post-prefix lo chunks so the engine doesn't park on the
                # AG_b wait while runnable lo gathers sit behind it
                for j in range(min(6, len(hi_insts))):
                    anchor = min(LOP + 2 + 2 * j, len(lo_insts) - 1)
                    dd = InstructionNameOrderedSet()
                    dd.add(lo_insts[anchor].ins.name)
                    hi_insts[j].ins.add_nosync_dependencies_from(dd)

                last = it == num_iter - 1
                for p in range(NBLK // 2):
                    col = p
                    psum = ps_pool.tile([128, D], F32, tag="ps")
                    for half in range(2):
                        tl = tiles_by_block[2 * p + half]
                        ph = half * 64
                        nc.tensor.matmul(
                            psum[ph:ph + 64, :],
                            identb[:, ph:ph + 64],
                            h0s[:, col, :],
                            start=True, stop=(len(tl) == 0))
                        for j, (dom, tpos, sidx) in enumerate(tl):
                            bufs = (lo_tiles_bufs if dom == "lo"
                                    else hi_tiles_bufs)
                            g = bufs[tpos // 8]
                            nc.tensor.matmul(
                                psum[ph:ph + 64, :],
                                s_sb[:, sidx * W:(sidx + 1) * W],
                                g[:, tpos % 8, :],
                                start=False, stop=(j == len(tl) - 1))
                    # evict: h_new = selfw * h_old + psum  (one DVE op)
                    if last:
                        st = stage_pool.tile([128, D], F32, tag="st")
                        nc.vector.scalar_tensor_tensor(
                            st[:], h_sb[:, col, :], selfw[:, col:col + 1],
                            psum[:, :], mybir.AluOpType.mult,
                            mybir.AluOpType.add)
                        nc.sync.dma_start(
                            out_d[p * 128:(p + 1) * 128, :], st[:])
                    else:
                        nc.vector.scalar_tensor_tensor(
                            h_sb[:, col, :], h_sb[:, col, :],
                            selfw[:, col:col + 1],
                            psum[:, :], mybir.AluOpType.mult,
                            mybir.AluOpType.add)
                        if p == COLS_A - 1:
                            nc.sync.dma_start(
                                ag_in_a[1 - buf][:].rearrange(
                                    "(c p) f -> p c f", p=128),
                                h_sb[:, 0:COLS_A, :])
                            ag_a = _emit_ag("a", 1 - buf)
                            # pin the AG trigger into the GpSimd gather
                            # stream at ~75% so its wire time overlaps the
                            # remaining gathers (GpSimd is the only engine
                            # that can trigger collectives)
                            gpos = (len(gath_insts) * 3) // 4
                            d1 = InstructionNameOrderedSet()
                            d1.add(gath_insts[gpos].ins.name)
                            ag_a.ins.add_nosync_dependencies_from(d1)
                            d2 = InstructionNameOrderedSet()
                            d2.add(ag_a.ins.name)
                            gath_insts[gpos + 1].ins.add_nosync_dependencies_from(d2)

                # software pipelining: emit the next iteration's first LOP
                # lo gathers (they only need AG_a of this iteration) BEFORE
                # staging/triggering AG_b, so they run during AG_b's wire
                # instead of idling at the iteration boundary.
                if not last:
                    nbuf = 1 - buf
                    np_bufs = {}
                    np_insts = []
                    half = LOP // 2
                    for ci in range(min(half, len(lo_chunks))):
                        g, gi = emit_gather("lo", ci, nbuf)
                        np_bufs[ci] = g
                        np_insts.append(gi)
                    nc.sync.dma_start(
                        ag_in_b[nbuf][:].rearrange(
                            "(c p) f -> p c f", p=128),
                        h_sb[:, COLS_A:NCOL, :])
                    _emit_ag("b", nbuf)
                    for ci in range(min(half, len(lo_chunks)),
                                    min(LOP, len(lo_chunks))):
                        g, gi = emit_gather("lo", ci, nbuf)
                        np_bufs[ci] = g
                        np_insts.append(gi)
                    pending = (np_bufs, np_insts)

    # Post-scheduling: align each gather's SWDGE queue with its DMASW lane
    # (lanes are assigned round-robin in scheduled order and their sems are
    # queue-locked in ucode, so queue must follow lane, not emission order).
    import re as _re
    for _blk in nc.m.functions[0].blocks:
        for _inst in _blk.instructions:
            if isinstance(_inst, mybir.InstDMAGatherAnt):
                _si = _inst.sync_info
                _lane = None
                for _u in (_si.on_update if _si else []):
                    _m = _re.match(r"DMASW(\d+)_", _u.ant_name or "")
                    if _m:
                        _lane = int(_m.group(1))
                if _lane is not None:
                    _inst.queue_num = _lane % NQ
    nc.compile()
    return nc


_CACHE = {}


def _get_compiled(edge_index, num_iter=NUM_ITER):
    key = (hash(np.asarray(edge_index).tobytes()), num_iter)
    if key not in _CACHE:
        cores = _preprocess(edge_index)
        cores = _equalize(cores)
        nc = _build_uniform(cores[0], num_iter=num_iter)
        _CACHE[key] = (nc, cores)
    return _CACHE[key]


def _make_in_maps(x, atom_emb, Ws, bs, cores_meta):
    x = np.asarray(x)
    emb_pad = np.zeros((NUM_ATOM_FEATS * 128, D), ml_dtypes.bfloat16)
    ae = np.asarray(atom_emb, dtype=np.float32)
    for f in range(NUM_ATOM_FEATS):
        emb_pad[f * 128:f * 128 + ATOM_VOCAB] = ae[f]
    ws_t = np.ascontiguousarray(
        np.asarray(Ws, dtype=np.float32).reshape(NUM_LAYER * D, D))
    bs_t = np.ascontiguousarray(np.asarray(bs, dtype=np.float32))
    ident = np.eye(128, dtype=np.float32)
    identb = np.eye(128, dtype=ml_dtypes.bfloat16)

    in_maps = []
    for c, m in enumerate(cores_meta):
        nr = m["node_of_row"]  # [SHARD_PAD] local node or -1
        # one-hot: [128 vocab-pad, NCOL*9*128] with oh[v, (col,f,n)] = 1
        oh = np.zeros((128, NCOL * NUM_ATOM_FEATS * 128), ml_dtypes.bfloat16)
        xs = x[c * SHARD:(c + 1) * SHARD]  # [SHARD, 9]
        rows = np.arange(SHARD_PAD)
        valid = nr >= 0
        for f in range(NUM_ATOM_FEATS):
            vals = np.zeros(SHARD_PAD, np.int64)
            vals[valid] = xs[nr[valid], f]
            cols = (rows // 128) * NUM_ATOM_FEATS * 128 + f * 128 + rows % 128
            oh[vals[valid], cols[valid]] = 1.0
        lo = m["lo_idx"] if m["n_lo"] else np.zeros(16, np.int64)
        hi_ = m["hi_idx"] if m["n_hi"] else np.zeros(16, np.int64)
        in_maps.append({
            "embtab": emb_pad,
            "oh": np.ascontiguousarray(oh),
            "ws": ws_t,
            "bs": bs_t,
            "ident": ident,
            "identb": identb,
            "selfw": np.ascontiguousarray(m["selfw"]),
            "idx_lo": _wrap_idxs(lo),
            "idx_hi": _wrap_idxs(hi_),
            "s": np.ascontiguousarray(m["s_sb"]),
        })
    return in_maps


def _unpermute(res, cores_meta):
    out = np.zeros((N_NODES, D), np.float32)
    for c, m in enumerate(cores_meta):
        nr = m["node_of_row"]
        valid = nr >= 0
        r = np.asarray(res[c]["out"], dtype=np.float32)
        out[c * SHARD + nr[valid]] = r[valid]
    return out


def kernel(x, edge_index, atom_emb, Ws, bs):
    nc, cores_meta = _get_compiled(edge_index)
    in_maps = _make_in_maps(x, atom_emb, Ws, bs, cores_meta)
    res = run_bass_kernel_spmd(nc, in_maps, core_ids=list(range(NC)))
    return np.ascontiguousarray(_unpermute(res.results, cores_meta))


def run_profiled(x, edge_index, atom_emb, Ws, bs):
    import ntff_hook
    ntff_hook.install()
    nc, cores_meta = _get_compiled(edge_index)
    in_maps = _make_in_maps(x, atom_emb, Ws, bs, cores_meta)
    res = run_bass_kernel_spmd(nc, in_maps, core_ids=list(range(NC)),
                               trace=True)
    return (np.ascontiguousarray(_unpermute(res.results, cores_meta)),
            res.exec_time_ns)


# revision 6
# speedup vs baseline: 1.1577x; 1.0561x over previous
"""APPNP (GCN-normalized propagation) distributed Bass kernel for 8 TRN2 cores.

v2 strategy (dst-sharded message passing, gather-descriptor-optimized):
  - Nodes sharded across 8 cores (6250/core, padded to 6272 = 49*128 rows).
  - Per-core node->row permutation balances per-(block,dom) in-edge counts
    across cores so SPMD equalization padding is minimal.
  - Prologue: atom embedding via one-hot matmuls (stationary = padded
    embedding table, moving = host-built one-hot of x) directly producing
    hT; 3-layer MLP in transposed space; PE transpose back -> h (bf16),
    h0s = 0.1*h.
  - Exchange: two AllGathers per iteration into DRAM tables
    a (rows 0:2560/core -> 20480 rows) and b (rows 2560:6272 -> 29696 rows),
    both < 32768 so int16 gather indices reach everything. a is issued
    mid-iteration (after block-pair 19), b at the end; the next iteration
    issues ~40 a-sourced chunks first so b's wire time is absorbed.
  - Per iteration: dma_gather h[src] for in-edges grouped in 64-wide dst
    blocks (128-slot tiles), TensorEngine segment-sum via one-hot S
    matrices (bf16, SBUF-resident) in PSUM; h0s injected via identity
    matmul; self-loops are NOT slots: the Vector engine evicts PSUM with
    h_new = selfw * h_old + psum in one scalar_tensor_tensor op.
"""

import numpy as np
import ml_dtypes

import concourse.bacc as bacc
import concourse.bass as bass
import concourse.mybir as mybir
import concourse.tile as tile
from concourse.bass_utils import run_bass_kernel_spmd
from concourse.instruction_name_ordered_set import InstructionNameOrderedSet

# Problem constants (must match reference.py)
N_NODES = 50000
N_EDGES = 800000
D = 128
NUM_ITER = 10
NUM_LAYER = 3
ALPHA = 0.1
NUM_ATOM_FEATS = 9
ATOM_VOCAB = 119

NC = 8
SHARD = N_NODES // NC            # 6250
SHARD_PAD = 6272                 # 49 * 128
NCOL = SHARD_PAD // 128          # 49
W = 64                           # dst block width
NBLK = SHARD_PAD // W            # 98
CHUNK = 1024                     # max idxs per dma_gather
COLS_A = 30                      # shard cols in exchange table a
ROWS_A = COLS_A * 128            # 3840
ROWS_B = SHARD_PAD - ROWS_A      # 2432
N_A = NC * ROWS_A                # 30720 (< 32768)
N_B = NC * ROWS_B                # 19456 (< 32768)
NQ = 4                           # SWDGE queues
ABSORB = 0                       # lo chunks issued before first hi chunk
CHAIN = False                    # chain gather emission order

BF16 = mybir.dt.bfloat16
F32 = mybir.dt.float32
I16 = mybir.dt.int16
AF = mybir.ActivationFunctionType


def _wrap_idxs(idx):
    """slot i -> partition i%16 (replicated x8), col i//16."""
    n = idx.shape[0]
    assert n % 16 == 0
    w = idx.reshape(n // 16, 16).T.astype(np.int16)
    return np.ascontiguousarray(np.tile(w, (8, 1)))


def _pad128(a, fill=0):
    n = a.shape[0]
    m = (-n) % 128
    if m == 0:
        return a
    return np.concatenate([a, np.full((m,) + a.shape[1:], fill, a.dtype)])


def _balance_perm(indeg):
    """Greedy LPT: assign 6250 local nodes to 98 blocks of <=64 nodes,
    balancing total in-degree per block. Returns node_of_row[6272] with -1
    for pad rows (all pads in the last block)."""
    import heapq
    order = np.argsort(-indeg, kind="stable")
    cap = np.full(NBLK, 64, np.int64)
    cap[NBLK - 1] = SHARD - 64 * (NBLK - 1)  # 42 real nodes in last block
    fill = [[] for _ in range(NBLK)]
    heap = [(0, b) for b in range(NBLK)]
    heapq.heapify(heap)
    for nid in order:
        while True:
            tot, b = heapq.heappop(heap)
            if len(fill[b]) < cap[b]:
                break
        fill[b].append(nid)
        if len(fill[b]) < cap[b]:
            heapq.heappush(heap, (tot + int(indeg[nid]), b))
    node_of_row = np.full(SHARD_PAD, -1, np.int64)
    for b in range(NBLK):
        for j, nid in enumerate(fill[b]):
            node_of_row[b * 64 + j] = nid
    return node_of_row


def _preprocess(edge_index):
    """Host-side graph preprocessing -> per-core structures."""
    src = np.asarray(edge_index[0], dtype=np.int64)
    dst = np.asarray(edge_index[1], dtype=np.int64)
    deg = np.bincount(dst, minlength=N_NODES).astype(np.float64) + 1.0
    dinv = 1.0 / np.sqrt(deg)
    coef = ((1.0 - ALPHA) * dinv[src] * dinv[dst]).astype(np.float32)
    selfw_g = ((1.0 - ALPHA) * dinv * dinv).astype(np.float32)

    # pass A: per-core balanced permutation (total in-degree)
    indeg_all = np.bincount(dst, minlength=N_NODES)
    node_of_row = np.zeros((NC, SHARD_PAD), np.int64)
    row_of_node = np.zeros(N_NODES, np.int64)  # global node -> local row
    for c in range(NC):
        nr = _balance_perm(indeg_all[c * SHARD:(c + 1) * SHARD])
        node_of_row[c] = nr
        valid = nr >= 0
        row_of_node[c * SHARD + nr[valid]] = np.nonzero(valid)[0]

    # pass B: with src sides fixed by pass A, pack nodes into blocks so each
    # (block, dom) in-edge count lands just UNDER a multiple of 128 (the
    # gather-tile quantum) and aligns across cores. Targets are global (the
    # max core's totals) so SPMD equalization adds almost nothing.
    src_isa = (row_of_node[src] % SHARD_PAD) < ROWS_A
    lo_in = np.bincount(dst[src_isa], minlength=N_NODES)
    hi_in = np.bincount(dst[~src_isa], minlength=N_NODES)
    RESID = 118  # target residue mod 128 (margin 10 to the next tile)

    def _targets(total_max, nb):
        base_q = max(0, int((total_max / nb - RESID) // 128))
        t = np.full(nb, base_q * 128 + RESID, np.float64)
        k = 0
        while t.sum() < total_max and k < nb:
            t[k] += 128
            k += 1
        while t.sum() < total_max:
            t += 128
        return t

    side_meta = []
    for side in (0, 1):
        if side == 0:
            blocks = list(range(0, ROWS_A // W))
        else:
            blocks = list(range(ROWS_A // W, NBLK))
        lmax = hmax = 0.0
        for c in range(NC):
            rows0 = 0 if side == 0 else ROWS_A
            nrows = ROWS_A if side == 0 else ROWS_B
            nodes = node_of_row[c][rows0:rows0 + nrows]
            nodes = nodes[nodes >= 0]
            lmax = max(lmax, lo_in[c * SHARD + nodes].sum())
            hmax = max(hmax, hi_in[c * SHARD + nodes].sum())
        nb = len(blocks)
        side_meta.append((blocks, _targets(lmax, nb), _targets(hmax, nb)))

    for c in range(NC):
        nr_new = np.full(SHARD_PAD, -1, np.int64)
        for side in (0, 1):
            blocks, T_lo, T_hi = side_meta[side]
            rows0 = 0 if side == 0 else ROWS_A
            nrows = ROWS_A if side == 0 else ROWS_B
            old_nodes = node_of_row[c][rows0:rows0 + nrows]
            old_nodes = old_nodes[old_nodes >= 0]
            li = lo_in[c * SHARD + old_nodes].astype(np.float64)
            hi_ = hi_in[c * SHARD + old_nodes].astype(np.float64)
            nb = len(blocks)
            caps = np.array([64 if b != NBLK - 1 else
                             SHARD - 64 * (NBLK - 1) for b in blocks])
            order_n = np.argsort(-(li + hi_), kind="stable")
            cur = np.zeros((nb, 2))
            cnt = np.zeros(nb, np.int64)
            assign = np.zeros(len(old_nodes), np.int64)
            for j in order_n:
                cost = np.maximum((cur[:, 0] + li[j]) / T_lo,
                                  (cur[:, 1] + hi_[j]) / T_hi)
                cost[cnt >= caps] = np.inf
                bsel = int(np.argmin(cost))
                assign[j] = bsel
                cur[bsel, 0] += li[j]
                cur[bsel, 1] += hi_[j]
                cnt[bsel] += 1
            # swap refinement: push overshoot (beyond targets) to zero
            def over(cb):
                return (max(0.0, cb[0]) + max(0.0, cb[1]))
            ex = cur - np.stack([T_lo, T_hi], axis=1)
            rng = np.random.default_rng(c)
            for _ in range(4):
                bad = np.nonzero((ex[:, 0] > 0) | (ex[:, 1] > 0))[0]
                if bad.size == 0:
                    break
                improved = False
                for b1 in bad:
                    js = np.nonzero(assign == b1)[0]
                    cands = rng.permutation(nb)[:20]
                    done = False
                    for b2 in cands:
                        if b2 == b1:
                            continue
                        for j1 in js[np.argsort(-(li[js] + hi_[js]))][:12]:
                            js2 = np.nonzero(assign == b2)[0]
                            if js2.size == 0:
                                continue
                            d1 = np.array([li[j1], hi_[j1]])
                            base = (over(ex[b1]) + over(ex[b2]))
                            d2s = np.stack([li[js2], hi_[js2]], axis=1)
                            nb1 = ex[b1] - d1 + d2s
                            nb2 = ex[b2] + d1 - d2s
                            costs = (np.maximum(nb1, 0).sum(axis=1) +
                                     np.maximum(nb2, 0).sum(axis=1))
                            kk = int(np.argmin(costs))
                            if costs[kk] < base - 0.5:
                                j2 = js2[kk]
                                ex[b1] = nb1[kk]
                                ex[b2] = nb2[kk]
                                cur[b1] += d2s[kk] - d1
                                cur[b2] += d1 - d2s[kk]
                                assign[j1], assign[j2] = b2, b1
                                improved = True
                                done = True
                                break
                        if done:
                            break
                if not improved:
                    break
            for bi, b in enumerate(blocks):
                nodes_b = old_nodes[assign == bi]
                for j2, nid in enumerate(nodes_b):
                    nr_new[b * 64 + j2] = nid
        node_of_row[c] = nr_new
        valid = nr_new >= 0
        row_of_node[c * SHARD + nr_new[valid]] = np.nonzero(valid)[0]

    # edge srow (exchange-table row of the source)
    src_core = src // SHARD
    r = row_of_node[src]
    isa = r < ROWS_A
    srow = np.where(isa, src_core * ROWS_A + r,
                    src_core * ROWS_B + (r - ROWS_A))

    cores = []
    for c in range(NC):
        m = (dst >= c * SHARD) & (dst < (c + 1) * SHARD)
        ldr = row_of_node[dst[m]]  # local row of each in-edge's dst
        lsrow = srow[m]
        lcoef = coef[m]
        lisa = isa[m]
        blk = ldr // W
        off = ldr % W

        streams = {"lo": [], "hi": []}
        s_tiles = []
        tiles_by_block = [[] for _ in range(NBLK)]
        stream_ntiles = {"lo": 0, "hi": 0}
        for b in range(NBLK):
            bm = blk == b
            for dom, dm in (("lo", lisa), ("hi", ~lisa)):
                sel = bm & dm
                n = int(sel.sum())
                if n == 0:
                    continue
                idx = _pad128(lsrow[sel].astype(np.int64))
                cf = _pad128(lcoef[sel])
                of = _pad128(off[sel].astype(np.int64))
                ntile = idx.shape[0] // 128
                for t in range(ntile):
                    s = np.zeros((128, W), np.float32)
                    s[np.arange(128), of[t * 128:(t + 1) * 128]] = \
                        cf[t * 128:(t + 1) * 128]
                    tiles_by_block[b].append((dom, stream_ntiles[dom] + t,
                                              len(s_tiles)))
                    s_tiles.append(s)
                streams[dom].append(idx)
                stream_ntiles[dom] += ntile

        lo_idx = (np.concatenate(streams["lo"]) if streams["lo"]
                  else np.zeros(0, np.int64))
        hi_idx = (np.concatenate(streams["hi"]) if streams["hi"]
                  else np.zeros(0, np.int64))
        s_all = (np.stack(s_tiles) if s_tiles
                 else np.zeros((0, 128, W), np.float32))
        s_sb = np.ascontiguousarray(
            s_all.transpose(1, 0, 2).reshape(128, -1)).astype(ml_dtypes.bfloat16)
        # selfw per row [128, NCOL]
        sw = np.zeros(SHARD_PAD, np.float32)
        nr = node_of_row[c]
        valid = nr >= 0
        sw[valid] = selfw_g[c * SHARD + nr[valid]]
        cores.append(dict(
            lo_idx=lo_idx, hi_idx=hi_idx, s_sb=s_sb,
            tiles_by_block=tiles_by_block,
            n_lo=lo_idx.shape[0], n_hi=hi_idx.shape[0],
            ntiles=len(s_tiles),
            node_of_row=node_of_row[c],
            selfw=np.ascontiguousarray(
                sw.reshape(NCOL, 128).T),  # [128, NCOL]
        ))
    return cores


def _chunks(total):
    out = []
    o = 0
    while o < total:
        n = min(CHUNK, total - o)
        out.append((o, n))
        o += n
    return out


def _equalize(cores_meta):
    """Pad per-block/dom tile counts to the max across cores (SPMD)."""
    cnt = np.zeros((NC, NBLK, 2), np.int64)
    for c, m in enumerate(cores_meta):
        for b in range(NBLK):
            for dom, tpos, sidx in m["tiles_by_block"][b]:
                cnt[c, b, 0 if dom == "lo" else 1] += 1
    mx = cnt.max(axis=0)

    new = []
    for c, m in enumerate(cores_meta):
        lo_parts, hi_parts, s_parts = [], [], []
        tiles_by_block = [[] for _ in range(NBLK)]
        lo_idx, hi_idx = m["lo_idx"], m["hi_idx"]
        s_all = m["s_sb"].reshape(128, -1, W)
        lo_nt, hi_nt = 0, 0
        s_n = 0
        for b in range(NBLK):
            for di, dom in enumerate(("lo", "hi")):
                have = [t for t in m["tiles_by_block"][b] if t[0] == dom]
                need = int(mx[b, di])
                for k in range(need):
                    if k < len(have):
                        _, tpos, sidx = have[k]
                        idx_arr = (lo_idx if dom == "lo" else hi_idx)[
                            tpos * 128:(tpos + 1) * 128]
                        s_mat = s_all[:, sidx, :]
                    else:
                        idx_arr = np.zeros(128, np.int64)
                        s_mat = np.zeros((128, W), ml_dtypes.bfloat16)
                    (lo_parts if dom == "lo" else hi_parts).append(idx_arr)
                    s_parts.append(np.asarray(s_mat))
                    nt = lo_nt if dom == "lo" else hi_nt
                    tiles_by_block[b].append((dom, nt, s_n))
                    s_n += 1
                    if dom == "lo":
                        lo_nt += 1
                    else:
                        hi_nt += 1
        lo_cat = (np.concatenate(lo_parts) if lo_parts
                  else np.zeros(0, np.int64))
        hi_cat = (np.concatenate(hi_parts) if hi_parts
                  else np.zeros(0, np.int64))
        s_cat = (np.stack(s_parts) if s_parts
                 else np.zeros((0, 128, W), ml_dtypes.bfloat16))
        s_sb = np.ascontiguousarray(
            np.asarray(s_cat).transpose(1, 0, 2).reshape(128, -1))
        new.append(dict(
            lo_idx=lo_cat, hi_idx=hi_cat, s_sb=s_sb,
            tiles_by_block=tiles_by_block,
            n_lo=lo_cat.shape[0], n_hi=hi_cat.shape[0], ntiles=s_n,
            node_of_row=m["node_of_row"], selfw=m["selfw"],
        ))
    return new


def _build_uniform(meta0, num_iter=NUM_ITER):
    n_lo, n_hi, ntiles = meta0["n_lo"], meta0["n_hi"], meta0["ntiles"]
    tiles_by_block = meta0["tiles_by_block"]

    nc = bacc.Bacc("TRN2", target_bir_lowering=False, debug=False,
                   num_devices=NC, num_swdge_queues=NQ)

    embtab = nc.dram_tensor("embtab", [NUM_ATOM_FEATS * 128, D], BF16,
                            kind="ExternalInput")
    oh_d = nc.dram_tensor("oh", [128, NCOL * NUM_ATOM_FEATS * 128], BF16,
                          kind="ExternalInput")
    ws = nc.dram_tensor("ws", [NUM_LAYER * D, D], F32, kind="ExternalInput")
    bs = nc.dram_tensor("bs", [NUM_LAYER, D], F32, kind="ExternalInput")
    ident = nc.dram_tensor("ident", [128, 128], F32, kind="ExternalInput")
    identb_d = nc.dram_tensor("identb", [128, 128], BF16,
                              kind="ExternalInput")
    selfw_d = nc.dram_tensor("selfw", [128, NCOL], F32, kind="ExternalInput")
    idx_lo_d = nc.dram_tensor("idx_lo", [128, max(n_lo, 16) // 16], I16,
                              kind="ExternalInput")
    idx_hi_d = nc.dram_tensor("idx_hi", [128, max(n_hi, 16) // 16], I16,
                              kind="ExternalInput")
    s_d = nc.dram_tensor("s", [128, max(ntiles, 1) * W], BF16,
                         kind="ExternalInput")
    out_d = nc.dram_tensor("out", [SHARD_PAD, D], F32, kind="ExternalOutput")

    ag_in_a = [nc.dram_tensor(f"ag_in_a{i}", [ROWS_A, D], BF16,
                              kind="Internal") for i in range(2)]
    ag_in_b = [nc.dram_tensor(f"ag_in_b{i}", [ROWS_B, D], BF16,
                              kind="Internal") for i in range(2)]
    ag_out_a = [nc.dram_tensor(f"ag_out_a{i}", [N_A, D], BF16,
                               kind="Internal", addr_space="Shared")
                for i in range(2)]
    ag_out_b = [nc.dram_tensor(f"ag_out_b{i}", [N_B, D], BF16,
                               kind="Internal", addr_space="Shared")
                for i in range(2)]

    def _emit_ag(which, buf):
        src = (ag_in_a if which == "a" else ag_in_b)[buf]
        dst = (ag_out_a if which == "a" else ag_out_b)[buf]
        return nc.gpsimd.collective_compute(
            "AllGather", mybir.AluOpType.bypass,
            replica_groups=[list(range(NC))],
            ins=[src[:].opt()], outs=[dst[:].opt()])

    lo_chunks = _chunks(n_lo)
    hi_chunks = _chunks(n_hi)

    with tile.TileContext(nc) as tc:
      with tc.tile_pool(name="persist", bufs=1) as persist:
        h_sb = persist.tile([128, NCOL, D], BF16, tag="h")
        h0s = persist.tile([128, NCOL, D], BF16, tag="h0s")
        selfw = persist.tile([128, NCOL], F32, tag="selfw")
        nc.sync.dma_start(selfw[:], selfw_d[:])
        identb = persist.tile([128, 128], BF16, tag="identb")
        nc.sync.dma_start(identb[:], identb_d[:])

        # ---------------- prologue: one-hot embedding + MLP ----------------
        with (
            tc.tile_pool(name="pro", bufs=1) as pro,
            tc.tile_pool(name="mlp", bufs=3) as mlp_pool,
            tc.tile_pool(name="prps", bufs=2, space="PSUM") as prps,
        ):
            emb_sb = pro.tile([128, NUM_ATOM_FEATS, D], BF16, tag="emb")
            nc.sync.dma_start(
                emb_sb[:],
                embtab[:, :].rearrange("(f p) d -> p f d", p=128))
            idn = pro.tile([128, 128], F32, tag="idn")
            nc.sync.dma_start(idn[:], ident[:])
            w_sb = pro.tile([128, NUM_LAYER * D], F32, tag="w")
            b_sb = pro.tile([128, NUM_LAYER], F32, tag="b")
            for l in range(NUM_LAYER):
                nc.sync.dma_start(w_sb[:, l * D:(l + 1) * D],
                                  ws[l * D:(l + 1) * D, :])
                nc.sync.dma_start(b_sb[:, l:l + 1],
                                  bs[l:l + 1, :].rearrange("a k -> k a"))
            oh_sb = pro.tile([128, NCOL * NUM_ATOM_FEATS * 128], BF16,
                             tag="oh")
            for col in range(NCOL):
                o = col * NUM_ATOM_FEATS * 128
                nc.sync.dma_start(oh_sb[:, o:o + NUM_ATOM_FEATS * 128],
                                  oh_d[:, o:o + NUM_ATOM_FEATS * 128])

            for col in range(NCOL):
                o = col * NUM_ATOM_FEATS * 128
                ps = prps.tile([128, 128], F32, tag="ps")
                for f in range(NUM_ATOM_FEATS):
                    nc.tensor.matmul(
                        ps[:], emb_sb[:, f, :],
                        oh_sb[:, o + f * 128:o + (f + 1) * 128],
                        start=(f == 0), stop=(f == NUM_ATOM_FEATS - 1))
                cur = mlp_pool.tile([128, 128], F32, tag="t")
                nc.scalar.activation(cur[:], ps[:], AF.Copy)
                for l in range(NUM_LAYER):
                    ps2 = prps.tile([128, 128], F32, tag="ps2")
                    nc.tensor.matmul(ps2[:], w_sb[:, l * D:(l + 1) * D],
                                     cur[:], start=True, stop=True)
                    cur = mlp_pool.tile([128, 128], F32, tag="t")
                    nc.scalar.activation(
                        cur[:], ps2[:],
                        AF.Relu if l != NUM_LAYER - 1 else AF.Identity,
                        bias=b_sb[:, l:l + 1])
                # transpose back: h [nodes, d]
                pt = prps.tile([128, 128], F32, tag="pt")
                nc.tensor.transpose(pt[:], cur[:], idn[:])
                nc.scalar.activation(h_sb[:, col, :], pt[:], AF.Copy)
                nc.scalar.activation(h0s[:, col, :], pt[:], AF.Copy,
                                     scale=ALPHA)
                if col == COLS_A - 1:
                    nc.sync.dma_start(
                        ag_in_a[0][:].rearrange("(c p) f -> p c f", p=128),
                        h_sb[:, 0:COLS_A, :])
                    _emit_ag("a", 0)
                elif col == NCOL - 1:
                    nc.sync.dma_start(
                        ag_in_b[0][:].rearrange("(c p) f -> p c f", p=128),
                        h_sb[:, COLS_A:NCOL, :])

        # ---------------- main loop ----------------
        with (
            tc.tile_pool(name="sconst", bufs=1) as sconst,
            tc.tile_pool(name="glo", bufs=18) as glo_pool,
            tc.tile_pool(name="ghi", bufs=14) as ghi_pool,
            tc.tile_pool(name="ps", bufs=8, space="PSUM") as ps_pool,
            tc.tile_pool(name="stage", bufs=2) as stage_pool,
        ):
            s_sb = sconst.tile([128, max(ntiles, 1) * W], BF16, tag="s")
            nc.sync.dma_start(s_sb[:], s_d[:])
            ilo = sconst.tile([128, max(n_lo, 16) // 16], I16, tag="ilo")
            nc.sync.dma_start(ilo[:], idx_lo_d[:])
            ihi = sconst.tile([128, max(n_hi, 16) // 16], I16, tag="ihi")
            nc.sync.dma_start(ihi[:], idx_hi_d[:])

            qe_g = [0]
            LOP = 12  # lo chunks of iter t+1 emitted before AG_b(t)

            def emit_gather(dom, ci, buf):
                (o, n) = (lo_chunks if dom == "lo" else hi_chunks)[ci]
                pool = glo_pool if dom == "lo" else ghi_pool
                view = (ag_out_a if dom == "lo" else ag_out_b)[buf][:, :]
                isb = ilo if dom == "lo" else ihi
                g = pool.tile([128, 8, D], BF16, tag="g" + dom)
                gi = nc.gpsimd.dma_gather(
                    g[:, 0:n // 128, :], view,
                    isb[:, o // 16:(o + n) // 16], n, n, D,
                    queue_num=qe_g[0] % NQ)
                qe_g[0] += 1
                return g, gi

            pending = None  # prefix state for the next iteration
            for it in range(num_iter):
                buf = it % 2
                if pending is None:
                    lo_tiles_bufs = {}
                    lo_insts = []
                    for ci in range(min(LOP, len(lo_chunks))):
                        g, gi = emit_gather("lo", ci, buf)
                        lo_tiles_bufs[ci] = g
                        lo_insts.append(gi)
                    # prologue staged table b; trigger its AllGather now so
                    # the wire overlaps the prefix gathers above
                    _emit_ag("b", 0)
                else:
                    lo_tiles_bufs, lo_insts = pending

                hi_tiles_bufs = {}
                gath_insts = []
                hi_insts = []
                order = []
                li, hii = min(LOP, len(lo_chunks)), 0
                while li < len(lo_chunks) or hii < len(hi_chunks):
                    if li < len(lo_chunks):
                        order.append(("lo", li)); li += 1
                    if hii < len(hi_chunks):
                        order.append(("hi", hii)); hii += 1
                for dom, ci in order:
                    g, gi = emit_gather(dom, ci, buf)
                    gath_insts.append(gi)
                    (lo_insts if dom == "lo" else hi_insts).append(gi)
                    (lo_tiles_bufs if dom == "lo" else hi_tiles_bufs)[ci] = g

                # soft absorb: schedule the first hi chunks after a few
                # post-prefix lo chunks so the engine doesn't park on the
                # AG_b wait while runnable lo gathers sit behind it
                for j in range(min(6, len(hi_insts))):
                    anchor = min(LOP + 2 + 2 * j, len(lo_insts) - 1)
                    dd = InstructionNameOrderedSet()
                    dd.add(lo_insts[anchor].ins.name)
                    hi_insts[j].ins.add_nosync_dependencies_from(dd)

                last = it == num_iter - 1
                for p in range(NBLK // 2):
                    col = p
                    psum = ps_pool.tile([128, D], F32, tag="ps")
                    for half in range(2):
                        tl = tiles_by_block[2 * p + half]
                        ph = half * 64
                        nc.tensor.matmul(
                            psum[ph:ph + 64, :],
                            identb[:, ph:ph + 64],
                            h0s[:, col, :],
                            start=True, stop=(len(tl) == 0))
                        for j, (dom, tpos, sidx) in enumerate(tl):
                            bufs = (lo_tiles_bufs if dom == "lo"
                                    else hi_tiles_bufs)
                            g = bufs[tpos // 8]
                            nc.tensor.matmul(
                                psum[ph:ph + 64, :],
                                s_sb[:, sidx * W:(sidx + 1) * W],
                                g[:, tpos % 8, :],
                                start=False, stop=(j == len(tl) - 1))
                    # evict: h_new = selfw * h_old + psum  (one DVE op)
                    if last:
                        st = stage_pool.tile([128, D], F32, tag="st")
                        nc.vector.scalar_tensor_tensor(
                            st[:], h_sb[:, col, :], selfw[:, col:col + 1],
                            psum[:, :], mybir.AluOpType.mult,
                            mybir.AluOpType.add)
                        nc.sync.dma_start(
                            out_d[p * 128:(p + 1) * 128, :], st[:])
                    else:
                        nc.vector.scalar_tensor_tensor(
                            h_sb[:, col, :], h_sb[:, col, :],
                            selfw[:, col:col + 1],
                            psum[:, :], mybir.AluOpType.mult,
                            mybir.AluOpType.add)
                        if p == COLS_A - 1:
                            nc.sync.dma_start(
                                ag_in_a[1 - buf][:].rearrange(
                                    "(c p) f -> p c f", p=128),
                                h_sb[:, 0:COLS_A, :])
                            ag_a = _emit_ag("a", 1 - buf)
                            # pin the AG trigger into the GpSimd gather
                            # stream at ~75% so its wire time overlaps the
                            # remaining gathers (GpSimd is the only engine
                            # that can trigger collectives)
                            gpos = (len(gath_insts) * 3) // 4
                            d1 = InstructionNameOrderedSet()
                            d1.add(gath_insts[gpos].ins.name)
                            ag_a.ins.add_nosync_dependencies_from(d1)
                            d2 = InstructionNameOrderedSet()
                            d2.add(ag_a.ins.name)
                            gath_insts[gpos + 1].ins.add_nosync_dependencies_from(d2)

                # software pipelining: emit the next iteration's first LOP
                # lo gathers (they only need AG_a of this iteration) BEFORE
                # staging/triggering AG_b, so they run during AG_b's wire
                # instead of idling at the iteration boundary.
                if not last:
                    nbuf = 1 - buf
                    np_bufs = {}
                    np_insts = []
                    half = LOP // 2
                    for ci in range(min(half, len(lo_chunks))):
                        g, gi = emit_gather("lo", ci, nbuf)
                        np_bufs[ci] = g
                        np_insts.append(gi)
                    nc.sync.dma_start(
                        ag_in_b[nbuf][:].rearrange(
                            "(c p) f -> p c f", p=128),
                        h_sb[:, COLS_A:NCOL, :])
                    _emit_ag("b", nbuf)
                    for ci in range(min(half, len(lo_chunks)),
                                    min(LOP, len(lo_chunks))):
                        g, gi = emit_gather("lo", ci, nbuf)
                        np_bufs[ci] = g
                        np_insts.append(gi)
                    pending = (np_bufs, np_insts)

    # Post-scheduling: align each gather's SWDGE queue with its DMASW lane
    # (lanes are assigned round-robin in scheduled order and their sems are
    # queue-locked in ucode, so queue must follow lane, not emission order).
    import re as _re
    for _blk in nc.m.functions[0].blocks:
        for _inst in _blk.instructions:
            if isinstance(_inst, mybir.InstDMAGatherAnt):
                _si = _inst.sync_info
                _lane = None
                for _u in (_si.on_update if _si else []):
                    _m = _re.match(r"DMASW(\d+)_", _u.ant_name or "")
                    if _m:
                        _lane = int(_m.group(1))
                if _lane is not None:
                    _inst.queue_num = _lane % NQ
    nc.compile()
    return nc


_CACHE = {}


def _get_compiled(edge_index, num_iter=NUM_ITER):
    key = (hash(np.asarray(edge_index).tobytes()), num_iter)
    if key not in _CACHE:
        cores = _preprocess(edge_index)
        cores = _equalize(cores)
        nc = _build_uniform(cores[0], num_iter=num_iter)
        _CACHE[key] = (nc, cores)
    return _CACHE[key]


def _make_in_maps(x, atom_emb, Ws, bs, cores_meta):
    x = np.asarray(x)
    emb_pad = np.zeros((NUM_ATOM_FEATS * 128, D), ml_dtypes.bfloat16)
    ae = np.asarray(atom_emb, dtype=np.float32)
    for f in range(NUM_ATOM_FEATS):
        emb_pad[f * 128:f * 128 + ATOM_VOCAB] = ae[f]
    ws_t = np.ascontiguousarray(
        np.asarray(Ws, dtype=np.float32).reshape(NUM_LAYER * D, D))
    bs_t = np.ascontiguousarray(np.asarray(bs, dtype=np.float32))
    ident = np.eye(128, dtype=np.float32)
    identb = np.eye(128, dtype=ml_dtypes.bfloat16)

    in_maps = []
    for c, m in enumerate(cores_meta):
        nr = m["node_of_row"]  # [SHARD_PAD] local node or -1
        # one-hot: [128 vocab-pad, NCOL*9*128] with oh[v, (col,f,n)] = 1
        oh = np.zeros((128, NCOL * NUM_ATOM_FEATS * 128), ml_dtypes.bfloat16)
        xs = x[c * SHARD:(c + 1) * SHARD]  # [SHARD, 9]
        rows = np.arange(SHARD_PAD)
        valid = nr >= 0
        for f in range(NUM_ATOM_FEATS):
            vals = np.zeros(SHARD_PAD, np.int64)
            vals[valid] = xs[nr[valid], f]
            cols = (rows // 128) * NUM_ATOM_FEATS * 128 + f * 128 + rows % 128
            oh[vals[valid], cols[valid]] = 1.0
        lo = m["lo_idx"] if m["n_lo"] else np.zeros(16, np.int64)
        hi_ = m["hi_idx"] if m["n_hi"] else np.zeros(16, np.int64)
        in_maps.append({
            "embtab": emb_pad,
            "oh": np.ascontiguousarray(oh),
            "ws": ws_t,
            "bs": bs_t,
            "ident": ident,
            "identb": identb,
            "selfw": np.ascontiguousarray(m["selfw"]),
            "idx_lo": _wrap_idxs(lo),
            "idx_hi": _wrap_idxs(hi_),
            "s": np.ascontiguousarray(m["s_sb"]),
        })
    return in_maps


def _unpermute(res, cores_meta):
    out = np.zeros((N_NODES, D), np.float32)
    for c, m in enumerate(cores_meta):
        nr = m["node_of_row"]
        valid = nr >= 0
        r = np.asarray(res[c]["out"], dtype=np.float32)
        out[c * SHARD + nr[valid]] = r[valid]
    return out


def kernel(x, edge_index, atom_emb, Ws, bs):
    nc, cores_meta = _get_compiled(edge_index)
    in_maps = _make_in_maps(x, atom_emb, Ws, bs, cores_meta)
    res = run_bass_kernel_spmd(nc, in_maps, core_ids=list(range(NC)))
    return np.ascontiguousarray(_unpermute(res.results, cores_meta))


def run_profiled(x, edge_index, atom_emb, Ws, bs):
    import ntff_hook
    ntff_hook.install()
    nc, cores_meta = _get_compiled(edge_index)
    in_maps = _make_in_maps(x, atom_emb, Ws, bs, cores_meta)
    res = run_bass_kernel_spmd(nc, in_maps, core_ids=list(range(NC)),
                               trace=True)
    return (np.ascontiguousarray(_unpermute(res.results, cores_meta)),
            res.exec_time_ns)


# revision 7
# speedup vs baseline: 1.1668x; 1.0079x over previous
"""APPNP (GCN-normalized propagation) distributed Bass kernel for 8 TRN2 cores.

v2 strategy (dst-sharded message passing, gather-descriptor-optimized):
  - Nodes sharded across 8 cores (6250/core, padded to 6272 = 49*128 rows).
  - Per-core node->row permutation balances per-(block,dom) in-edge counts
    across cores so SPMD equalization padding is minimal.
  - Prologue: atom embedding via one-hot matmuls (stationary = padded
    embedding table, moving = host-built one-hot of x) directly producing
    hT; 3-layer MLP in transposed space; PE transpose back -> h (bf16),
    h0s = 0.1*h.
  - Exchange: two AllGathers per iteration into DRAM tables
    a (rows 0:2560/core -> 20480 rows) and b (rows 2560:6272 -> 29696 rows),
    both < 32768 so int16 gather indices reach everything. a is issued
    mid-iteration (after block-pair 19), b at the end; the next iteration
    issues ~40 a-sourced chunks first so b's wire time is absorbed.
  - Per iteration: dma_gather h[src] for in-edges grouped in 64-wide dst
    blocks (128-slot tiles), TensorEngine segment-sum via one-hot S
    matrices (bf16, SBUF-resident) in PSUM; h0s injected via identity
    matmul; self-loops are NOT slots: the Vector engine evicts PSUM with
    h_new = selfw * h_old + psum in one scalar_tensor_tensor op.
"""

import numpy as np
import ml_dtypes

import concourse.bacc as bacc
import concourse.bass as bass
import concourse.mybir as mybir
import concourse.tile as tile
from concourse.bass_utils import run_bass_kernel_spmd
from concourse.instruction_name_ordered_set import InstructionNameOrderedSet

# Problem constants (must match reference.py)
N_NODES = 50000
N_EDGES = 800000
D = 128
NUM_ITER = 10
NUM_LAYER = 3
ALPHA = 0.1
NUM_ATOM_FEATS = 9
ATOM_VOCAB = 119

NC = 8
SHARD = N_NODES // NC            # 6250
SHARD_PAD = 6272                 # 49 * 128
NCOL = SHARD_PAD // 128          # 49
W = 64                           # dst block width
NBLK = SHARD_PAD // W            # 98
CHUNK = 1024                     # max idxs per dma_gather
COLS_A = 30                      # shard cols in exchange table a
ROWS_A = COLS_A * 128            # 3840
ROWS_B = SHARD_PAD - ROWS_A      # 2432
N_A = NC * ROWS_A                # 30720 (< 32768)
N_B = NC * ROWS_B                # 19456 (< 32768)
NQ = 4                           # SWDGE queues
ABSORB = 0                       # lo chunks issued before first hi chunk
CHAIN = False                    # chain gather emission order

BF16 = mybir.dt.bfloat16
F32 = mybir.dt.float32
I16 = mybir.dt.int16
AF = mybir.ActivationFunctionType


def _wrap_idxs(idx):
    """slot i -> partition i%16 (replicated x8), col i//16."""
    n = idx.shape[0]
    assert n % 16 == 0
    w = idx.reshape(n // 16, 16).T.astype(np.int16)
    return np.ascontiguousarray(np.tile(w, (8, 1)))


def _pad128(a, fill=0):
    n = a.shape[0]
    m = (-n) % 128
    if m == 0:
        return a
    return np.concatenate([a, np.full((m,) + a.shape[1:], fill, a.dtype)])


def _balance_perm(indeg):
    """Greedy LPT: assign 6250 local nodes to 98 blocks of <=64 nodes,
    balancing total in-degree per block. Returns node_of_row[6272] with -1
    for pad rows (all pads in the last block)."""
    import heapq
    order = np.argsort(-indeg, kind="stable")
    cap = np.full(NBLK, 64, np.int64)
    cap[NBLK - 1] = SHARD - 64 * (NBLK - 1)  # 42 real nodes in last block
    fill = [[] for _ in range(NBLK)]
    heap = [(0, b) for b in range(NBLK)]
    heapq.heapify(heap)
    for nid in order:
        while True:
            tot, b = heapq.heappop(heap)
            if len(fill[b]) < cap[b]:
                break
        fill[b].append(nid)
        if len(fill[b]) < cap[b]:
            heapq.heappush(heap, (tot + int(indeg[nid]), b))
    node_of_row = np.full(SHARD_PAD, -1, np.int64)
    for b in range(NBLK):
        for j, nid in enumerate(fill[b]):
            node_of_row[b * 64 + j] = nid
    return node_of_row


def _preprocess(edge_index):
    """Host-side graph preprocessing -> per-core structures."""
    src = np.asarray(edge_index[0], dtype=np.int64)
    dst = np.asarray(edge_index[1], dtype=np.int64)
    deg = np.bincount(dst, minlength=N_NODES).astype(np.float64) + 1.0
    dinv = 1.0 / np.sqrt(deg)
    coef = ((1.0 - ALPHA) * dinv[src] * dinv[dst]).astype(np.float32)
    selfw_g = ((1.0 - ALPHA) * dinv * dinv).astype(np.float32)

    # pass A: per-core balanced permutation (total in-degree)
    indeg_all = np.bincount(dst, minlength=N_NODES)
    node_of_row = np.zeros((NC, SHARD_PAD), np.int64)
    row_of_node = np.zeros(N_NODES, np.int64)  # global node -> local row
    for c in range(NC):
        nr = _balance_perm(indeg_all[c * SHARD:(c + 1) * SHARD])
        node_of_row[c] = nr
        valid = nr >= 0
        row_of_node[c * SHARD + nr[valid]] = np.nonzero(valid)[0]

    # pass B: with src sides fixed by pass A, pack nodes into blocks so each
    # (block, dom) in-edge count lands just UNDER a multiple of 128 (the
    # gather-tile quantum) and aligns across cores. Targets are global (the
    # max core's totals) so SPMD equalization adds almost nothing.
    src_isa = (row_of_node[src] % SHARD_PAD) < ROWS_A
    lo_in = np.bincount(dst[src_isa], minlength=N_NODES)
    hi_in = np.bincount(dst[~src_isa], minlength=N_NODES)
    RESID = 118  # target residue mod 128 (margin 10 to the next tile)

    def _targets(total_max, nb):
        base_q = max(0, int((total_max / nb - RESID) // 128))
        t = np.full(nb, base_q * 128 + RESID, np.float64)
        k = 0
        while t.sum() < total_max and k < nb:
            t[k] += 128
            k += 1
        while t.sum() < total_max:
            t += 128
        return t

    side_meta = []
    for side in (0, 1):
        if side == 0:
            blocks = list(range(0, ROWS_A // W))
        else:
            blocks = list(range(ROWS_A // W, NBLK))
        lmax = hmax = 0.0
        for c in range(NC):
            rows0 = 0 if side == 0 else ROWS_A
            nrows = ROWS_A if side == 0 else ROWS_B
            nodes = node_of_row[c][rows0:rows0 + nrows]
            nodes = nodes[nodes >= 0]
            lmax = max(lmax, lo_in[c * SHARD + nodes].sum())
            hmax = max(hmax, hi_in[c * SHARD + nodes].sum())
        nb = len(blocks)
        side_meta.append((blocks, _targets(lmax, nb), _targets(hmax, nb)))

    for c in range(NC):
        nr_new = np.full(SHARD_PAD, -1, np.int64)
        for side in (0, 1):
            blocks, T_lo, T_hi = side_meta[side]
            rows0 = 0 if side == 0 else ROWS_A
            nrows = ROWS_A if side == 0 else ROWS_B
            old_nodes = node_of_row[c][rows0:rows0 + nrows]
            old_nodes = old_nodes[old_nodes >= 0]
            li = lo_in[c * SHARD + old_nodes].astype(np.float64)
            hi_ = hi_in[c * SHARD + old_nodes].astype(np.float64)
            nb = len(blocks)
            caps = np.array([64 if b != NBLK - 1 else
                             SHARD - 64 * (NBLK - 1) for b in blocks])
            order_n = np.argsort(-(li + hi_), kind="stable")
            cur = np.zeros((nb, 2))
            cnt = np.zeros(nb, np.int64)
            assign = np.zeros(len(old_nodes), np.int64)
            for j in order_n:
                cost = np.maximum((cur[:, 0] + li[j]) / T_lo,
                                  (cur[:, 1] + hi_[j]) / T_hi)
                cost[cnt >= caps] = np.inf
                bsel = int(np.argmin(cost))
                assign[j] = bsel
                cur[bsel, 0] += li[j]
                cur[bsel, 1] += hi_[j]
                cnt[bsel] += 1
            # swap refinement: push overshoot (beyond targets) to zero
            def over(cb):
                return (max(0.0, cb[0]) + max(0.0, cb[1]))
            ex = cur - np.stack([T_lo, T_hi], axis=1)
            rng = np.random.default_rng(c)
            for _ in range(4):
                bad = np.nonzero((ex[:, 0] > 0) | (ex[:, 1] > 0))[0]
                if bad.size == 0:
                    break
                improved = False
                for b1 in bad:
                    js = np.nonzero(assign == b1)[0]
                    cands = rng.permutation(nb)[:20]
                    done = False
                    for b2 in cands:
                        if b2 == b1:
                            continue
                        for j1 in js[np.argsort(-(li[js] + hi_[js]))][:12]:
                            js2 = np.nonzero(assign == b2)[0]
                            if js2.size == 0:
                                continue
                            d1 = np.array([li[j1], hi_[j1]])
                            base = (over(ex[b1]) + over(ex[b2]))
                            d2s = np.stack([li[js2], hi_[js2]], axis=1)
                            nb1 = ex[b1] - d1 + d2s
                            nb2 = ex[b2] + d1 - d2s
                            costs = (np.maximum(nb1, 0).sum(axis=1) +
                                     np.maximum(nb2, 0).sum(axis=1))
                            kk = int(np.argmin(costs))
                            if costs[kk] < base - 0.5:
                                j2 = js2[kk]
                                ex[b1] = nb1[kk]
                                ex[b2] = nb2[kk]
                                cur[b1] += d2s[kk] - d1
                                cur[b2] += d1 - d2s[kk]
                                assign[j1], assign[j2] = b2, b1
                                improved = True
                                done = True
                                break
                        if done:
                            break
                if not improved:
                    break
            for bi, b in enumerate(blocks):
                nodes_b = old_nodes[assign == bi]
                for j2, nid in enumerate(nodes_b):
                    nr_new[b * 64 + j2] = nid
        node_of_row[c] = nr_new
        valid = nr_new >= 0
        row_of_node[c * SHARD + nr_new[valid]] = np.nonzero(valid)[0]

    # edge srow (exchange-table row of the source)
    src_core = src // SHARD
    r = row_of_node[src]
    isa = r < ROWS_A
    srow = np.where(isa, src_core * ROWS_A + r,
                    src_core * ROWS_B + (r - ROWS_A))

    cores = []
    for c in range(NC):
        m = (dst >= c * SHARD) & (dst < (c + 1) * SHARD)
        ldr = row_of_node[dst[m]]  # local row of each in-edge's dst
        lsrow = srow[m]
        lcoef = coef[m]
        lisa = isa[m]
        blk = ldr // W
        off = ldr % W

        streams = {"lo": [], "hi": []}
        s_tiles = []
        tiles_by_block = [[] for _ in range(NBLK)]
        stream_ntiles = {"lo": 0, "hi": 0}
        for b in range(NBLK):
            bm = blk == b
            for dom, dm in (("lo", lisa), ("hi", ~lisa)):
                sel = bm & dm
                n = int(sel.sum())
                if n == 0:
                    continue
                idx = _pad128(lsrow[sel].astype(np.int64))
                cf = _pad128(lcoef[sel])
                of = _pad128(off[sel].astype(np.int64))
                ntile = idx.shape[0] // 128
                for t in range(ntile):
                    s = np.zeros((128, W), np.float32)
                    s[np.arange(128), of[t * 128:(t + 1) * 128]] = \
                        cf[t * 128:(t + 1) * 128]
                    tiles_by_block[b].append((dom, stream_ntiles[dom] + t,
                                              len(s_tiles)))
                    s_tiles.append(s)
                streams[dom].append(idx)
                stream_ntiles[dom] += ntile

        lo_idx = (np.concatenate(streams["lo"]) if streams["lo"]
                  else np.zeros(0, np.int64))
        hi_idx = (np.concatenate(streams["hi"]) if streams["hi"]
                  else np.zeros(0, np.int64))
        s_all = (np.stack(s_tiles) if s_tiles
                 else np.zeros((0, 128, W), np.float32))
        s_sb = np.ascontiguousarray(
            s_all.transpose(1, 0, 2).reshape(128, -1)).astype(ml_dtypes.bfloat16)
        # selfw per row [128, NCOL]
        sw = np.zeros(SHARD_PAD, np.float32)
        nr = node_of_row[c]
        valid = nr >= 0
        sw[valid] = selfw_g[c * SHARD + nr[valid]]
        cores.append(dict(
            lo_idx=lo_idx, hi_idx=hi_idx, s_sb=s_sb,
            tiles_by_block=tiles_by_block,
            n_lo=lo_idx.shape[0], n_hi=hi_idx.shape[0],
            ntiles=len(s_tiles),
            node_of_row=node_of_row[c],
            selfw=np.ascontiguousarray(
                sw.reshape(NCOL, 128).T),  # [128, NCOL]
        ))
    return cores


def _chunks(total):
    out = []
    o = 0
    while o < total:
        n = min(CHUNK, total - o)
        out.append((o, n))
        o += n
    return out


def _equalize(cores_meta):
    """Pad per-block/dom tile counts to the max across cores (SPMD)."""
    cnt = np.zeros((NC, NBLK, 2), np.int64)
    for c, m in enumerate(cores_meta):
        for b in range(NBLK):
            for dom, tpos, sidx in m["tiles_by_block"][b]:
                cnt[c, b, 0 if dom == "lo" else 1] += 1
    mx = cnt.max(axis=0)

    new = []
    for c, m in enumerate(cores_meta):
        lo_parts, hi_parts, s_parts = [], [], []
        tiles_by_block = [[] for _ in range(NBLK)]
        lo_idx, hi_idx = m["lo_idx"], m["hi_idx"]
        s_all = m["s_sb"].reshape(128, -1, W)
        lo_nt, hi_nt = 0, 0
        s_n = 0
        for b in range(NBLK):
            for di, dom in enumerate(("lo", "hi")):
                have = [t for t in m["tiles_by_block"][b] if t[0] == dom]
                need = int(mx[b, di])
                for k in range(need):
                    if k < len(have):
                        _, tpos, sidx = have[k]
                        idx_arr = (lo_idx if dom == "lo" else hi_idx)[
                            tpos * 128:(tpos + 1) * 128]
                        s_mat = s_all[:, sidx, :]
                    else:
                        idx_arr = np.zeros(128, np.int64)
                        s_mat = np.zeros((128, W), ml_dtypes.bfloat16)
                    (lo_parts if dom == "lo" else hi_parts).append(idx_arr)
                    s_parts.append(np.asarray(s_mat))
                    nt = lo_nt if dom == "lo" else hi_nt
                    tiles_by_block[b].append((dom, nt, s_n))
                    s_n += 1
                    if dom == "lo":
                        lo_nt += 1
                    else:
                        hi_nt += 1
        lo_cat = (np.concatenate(lo_parts) if lo_parts
                  else np.zeros(0, np.int64))
        hi_cat = (np.concatenate(hi_parts) if hi_parts
                  else np.zeros(0, np.int64))
        s_cat = (np.stack(s_parts) if s_parts
                 else np.zeros((0, 128, W), ml_dtypes.bfloat16))
        s_sb = np.ascontiguousarray(
            np.asarray(s_cat).transpose(1, 0, 2).reshape(128, -1))
        new.append(dict(
            lo_idx=lo_cat, hi_idx=hi_cat, s_sb=s_sb,
            tiles_by_block=tiles_by_block,
            n_lo=lo_cat.shape[0], n_hi=hi_cat.shape[0], ntiles=s_n,
            node_of_row=m["node_of_row"], selfw=m["selfw"],
        ))
    return new


def _build_uniform(meta0, num_iter=NUM_ITER):
    n_lo, n_hi, ntiles = meta0["n_lo"], meta0["n_hi"], meta0["ntiles"]
    tiles_by_block = meta0["tiles_by_block"]

    nc = bacc.Bacc("TRN2", target_bir_lowering=False, debug=False,
                   num_devices=NC, num_swdge_queues=NQ)

    embtab = nc.dram_tensor("embtab", [NUM_ATOM_FEATS * 128, D], BF16,
                            kind="ExternalInput")
    oh_d = nc.dram_tensor("oh", [128, NCOL * NUM_ATOM_FEATS * 128], BF16,
                          kind="ExternalInput")
    ws = nc.dram_tensor("ws", [NUM_LAYER * D, D], F32, kind="ExternalInput")
    bs = nc.dram_tensor("bs", [NUM_LAYER, D], F32, kind="ExternalInput")
    ident = nc.dram_tensor("ident", [128, 128], F32, kind="ExternalInput")
    identb_d = nc.dram_tensor("identb", [128, 128], BF16,
                              kind="ExternalInput")
    selfw_d = nc.dram_tensor("selfw", [128, NCOL], F32, kind="ExternalInput")
    idx_lo_d = nc.dram_tensor("idx_lo", [128, max(n_lo, 16) // 16], I16,
                              kind="ExternalInput")
    idx_hi_d = nc.dram_tensor("idx_hi", [128, max(n_hi, 16) // 16], I16,
                              kind="ExternalInput")
    s_d = nc.dram_tensor("s", [128, max(ntiles, 1) * W], BF16,
                         kind="ExternalInput")
    out_d = nc.dram_tensor("out", [SHARD_PAD, D], F32, kind="ExternalOutput")

    ag_in_a = [nc.dram_tensor(f"ag_in_a{i}", [ROWS_A, D], BF16,
                              kind="Internal") for i in range(2)]
    ag_in_b = [nc.dram_tensor(f"ag_in_b{i}", [ROWS_B, D], BF16,
                              kind="Internal") for i in range(2)]
    ag_out_a = [nc.dram_tensor(f"ag_out_a{i}", [N_A, D], BF16,
                               kind="Internal", addr_space="Shared")
                for i in range(2)]
    ag_out_b = [nc.dram_tensor(f"ag_out_b{i}", [N_B, D], BF16,
                               kind="Internal", addr_space="Shared")
                for i in range(2)]

    def _emit_ag(which, buf):
        src = (ag_in_a if which == "a" else ag_in_b)[buf]
        dst = (ag_out_a if which == "a" else ag_out_b)[buf]
        return nc.gpsimd.collective_compute(
            "AllGather", mybir.AluOpType.bypass,
            replica_groups=[list(range(NC))],
            ins=[src[:].opt()], outs=[dst[:].opt()])

    lo_chunks = _chunks(n_lo)
    hi_chunks = _chunks(n_hi)

    with tile.TileContext(nc) as tc:
      with tc.tile_pool(name="persist", bufs=1) as persist:
        h_sb = persist.tile([128, NCOL, D], BF16, tag="h")
        h0s = persist.tile([128, NCOL, D], BF16, tag="h0s")
        selfw = persist.tile([128, NCOL], F32, tag="selfw")
        nc.sync.dma_start(selfw[:], selfw_d[:])
        identb = persist.tile([128, 128], BF16, tag="identb")
        nc.sync.dma_start(identb[:], identb_d[:])

        # ---------------- prologue: one-hot embedding + MLP ----------------
        with (
            tc.tile_pool(name="pro", bufs=1) as pro,
            tc.tile_pool(name="mlp", bufs=3) as mlp_pool,
            tc.tile_pool(name="prps", bufs=2, space="PSUM") as prps,
        ):
            emb_sb = pro.tile([128, NUM_ATOM_FEATS, D], BF16, tag="emb")
            nc.sync.dma_start(
                emb_sb[:],
                embtab[:, :].rearrange("(f p) d -> p f d", p=128))
            idn = pro.tile([128, 128], F32, tag="idn")
            nc.sync.dma_start(idn[:], ident[:])
            w_sb = pro.tile([128, NUM_LAYER * D], F32, tag="w")
            b_sb = pro.tile([128, NUM_LAYER], F32, tag="b")
            for l in range(NUM_LAYER):
                nc.sync.dma_start(w_sb[:, l * D:(l + 1) * D],
                                  ws[l * D:(l + 1) * D, :])
                nc.sync.dma_start(b_sb[:, l:l + 1],
                                  bs[l:l + 1, :].rearrange("a k -> k a"))
            oh_sb = pro.tile([128, NCOL * NUM_ATOM_FEATS * 128], BF16,
                             tag="oh")
            for col in range(NCOL):
                o = col * NUM_ATOM_FEATS * 128
                nc.sync.dma_start(oh_sb[:, o:o + NUM_ATOM_FEATS * 128],
                                  oh_d[:, o:o + NUM_ATOM_FEATS * 128])

            for col in range(NCOL):
                o = col * NUM_ATOM_FEATS * 128
                ps = prps.tile([128, 128], F32, tag="ps")
                for f in range(NUM_ATOM_FEATS):
                    nc.tensor.matmul(
                        ps[:], emb_sb[:, f, :],
                        oh_sb[:, o + f * 128:o + (f + 1) * 128],
                        start=(f == 0), stop=(f == NUM_ATOM_FEATS - 1))
                cur = mlp_pool.tile([128, 128], F32, tag="t")
                nc.scalar.activation(cur[:], ps[:], AF.Copy)
                for l in range(NUM_LAYER):
                    ps2 = prps.tile([128, 128], F32, tag="ps2")
                    nc.tensor.matmul(ps2[:], w_sb[:, l * D:(l + 1) * D],
                                     cur[:], start=True, stop=True)
                    cur = mlp_pool.tile([128, 128], F32, tag="t")
                    nc.scalar.activation(
                        cur[:], ps2[:],
                        AF.Relu if l != NUM_LAYER - 1 else AF.Identity,
                        bias=b_sb[:, l:l + 1])
                # transpose back: h [nodes, d]
                pt = prps.tile([128, 128], F32, tag="pt")
                nc.tensor.transpose(pt[:], cur[:], idn[:])
                nc.scalar.activation(h_sb[:, col, :], pt[:], AF.Copy)
                nc.scalar.activation(h0s[:, col, :], pt[:], AF.Copy,
                                     scale=ALPHA)
                if col == COLS_A - 1:
                    nc.sync.dma_start(
                        ag_in_a[0][:].rearrange("(c p) f -> p c f", p=128),
                        h_sb[:, 0:COLS_A, :])
                    _emit_ag("a", 0)
                elif col == NCOL - 1:
                    nc.sync.dma_start(
                        ag_in_b[0][:].rearrange("(c p) f -> p c f", p=128),
                        h_sb[:, COLS_A:NCOL, :])

        # ---------------- main loop ----------------
        with (
            tc.tile_pool(name="sconst", bufs=1) as sconst,
            tc.tile_pool(name="glo", bufs=20) as glo_pool,
            tc.tile_pool(name="ghi", bufs=14) as ghi_pool,
            tc.tile_pool(name="ps", bufs=8, space="PSUM") as ps_pool,
            tc.tile_pool(name="stage", bufs=2) as stage_pool,
        ):
            s_sb = sconst.tile([128, max(ntiles, 1) * W], BF16, tag="s")
            nc.sync.dma_start(s_sb[:], s_d[:])
            ilo = sconst.tile([128, max(n_lo, 16) // 16], I16, tag="ilo")
            nc.sync.dma_start(ilo[:], idx_lo_d[:])
            ihi = sconst.tile([128, max(n_hi, 16) // 16], I16, tag="ihi")
            nc.sync.dma_start(ihi[:], idx_hi_d[:])

            qe_g = [0]
            LOP = 12  # lo chunks of iter t+1 emitted before AG_b(t)

            def emit_gather(dom, ci, buf):
                (o, n) = (lo_chunks if dom == "lo" else hi_chunks)[ci]
                pool = glo_pool if dom == "lo" else ghi_pool
                view = (ag_out_a if dom == "lo" else ag_out_b)[buf][:, :]
                isb = ilo if dom == "lo" else ihi
                g = pool.tile([128, 8, D], BF16, tag="g" + dom)
                gi = nc.gpsimd.dma_gather(
                    g[:, 0:n // 128, :], view,
                    isb[:, o // 16:(o + n) // 16], n, n, D,
                    queue_num=qe_g[0] % NQ)
                qe_g[0] += 1
                return g, gi

            pending = None  # prefix state for the next iteration
            for it in range(num_iter):
                buf = it % 2
                if pending is None:
                    lo_tiles_bufs = {}
                    lo_insts = []
                    for ci in range(min(LOP, len(lo_chunks))):
                        g, gi = emit_gather("lo", ci, buf)
                        lo_tiles_bufs[ci] = g
                        lo_insts.append(gi)
                    # prologue staged table b; trigger its AllGather now so
                    # the wire overlaps the prefix gathers above
                    _emit_ag("b", 0)
                else:
                    lo_tiles_bufs, lo_insts = pending

                hi_tiles_bufs = {}
                gath_insts = []
                hi_insts = []
                order = []
                li, hii = min(LOP, len(lo_chunks)), 0
                while li < len(lo_chunks) or hii < len(hi_chunks):
                    if li < len(lo_chunks):
                        order.append(("lo", li)); li += 1
                    if hii < len(hi_chunks):
                        order.append(("hi", hii)); hii += 1
                for dom, ci in order:
                    g, gi = emit_gather(dom, ci, buf)
                    gath_insts.append(gi)
                    (lo_insts if dom == "lo" else hi_insts).append(gi)
                    (lo_tiles_bufs if dom == "lo" else hi_tiles_bufs)[ci] = g

                # soft absorb: schedule the first hi chunks after a few
                # post-prefix lo chunks so the engine doesn't park on the
                # AG_b wait while runnable lo gathers sit behind it
                for j in range(min(6, len(hi_insts))):
                    anchor = min(LOP + 2 + 2 * j, len(lo_insts) - 1)
                    dd = InstructionNameOrderedSet()
                    dd.add(lo_insts[anchor].ins.name)
                    hi_insts[j].ins.add_nosync_dependencies_from(dd)

                last = it == num_iter - 1
                for p in range(NBLK // 2):
                    col = p
                    psum = ps_pool.tile([128, D], F32, tag="ps")
                    for half in range(2):
                        tl = tiles_by_block[2 * p + half]
                        ph = half * 64
                        nc.tensor.matmul(
                            psum[ph:ph + 64, :],
                            identb[:, ph:ph + 64],
                            h0s[:, col, :],
                            start=True, stop=(len(tl) == 0))
                        for j, (dom, tpos, sidx) in enumerate(tl):
                            bufs = (lo_tiles_bufs if dom == "lo"
                                    else hi_tiles_bufs)
                            g = bufs[tpos // 8]
                            nc.tensor.matmul(
                                psum[ph:ph + 64, :],
                                s_sb[:, sidx * W:(sidx + 1) * W],
                                g[:, tpos % 8, :],
                                start=False, stop=(j == len(tl) - 1))
                    # evict: h_new = selfw * h_old + psum  (one DVE op)
                    if last:
                        st = stage_pool.tile([128, D], F32, tag="st")
                        nc.vector.scalar_tensor_tensor(
                            st[:], h_sb[:, col, :], selfw[:, col:col + 1],
                            psum[:, :], mybir.AluOpType.mult,
                            mybir.AluOpType.add)
                        nc.sync.dma_start(
                            out_d[p * 128:(p + 1) * 128, :], st[:])
                    else:
                        nc.vector.scalar_tensor_tensor(
                            h_sb[:, col, :], h_sb[:, col, :],
                            selfw[:, col:col + 1],
                            psum[:, :], mybir.AluOpType.mult,
                            mybir.AluOpType.add)
                        if p == COLS_A - 1:
                            nc.sync.dma_start(
                                ag_in_a[1 - buf][:].rearrange(
                                    "(c p) f -> p c f", p=128),
                                h_sb[:, 0:COLS_A, :])
                            ag_a = _emit_ag("a", 1 - buf)
                            # pin the AG trigger into the GpSimd gather
                            # stream at ~75% so its wire time overlaps the
                            # remaining gathers (GpSimd is the only engine
                            # that can trigger collectives)
                            gpos = (len(gath_insts) * 3) // 4
                            d1 = InstructionNameOrderedSet()
                            d1.add(gath_insts[gpos].ins.name)
                            ag_a.ins.add_nosync_dependencies_from(d1)
                            d2 = InstructionNameOrderedSet()
                            d2.add(ag_a.ins.name)
                            gath_insts[gpos + 1].ins.add_nosync_dependencies_from(d2)

                # software pipelining: emit the next iteration's first LOP
                # lo gathers (they only need AG_a of this iteration) BEFORE
                # staging/triggering AG_b, so they run during AG_b's wire
                # instead of idling at the iteration boundary.
                if not last:
                    nbuf = 1 - buf
                    np_bufs = {}
                    np_insts = []
                    half = LOP // 2
                    for ci in range(min(half, len(lo_chunks))):
                        g, gi = emit_gather("lo", ci, nbuf)
                        np_bufs[ci] = g
                        np_insts.append(gi)
                    nc.sync.dma_start(
                        ag_in_b[nbuf][:].rearrange(
                            "(c p) f -> p c f", p=128),
                        h_sb[:, COLS_A:NCOL, :])
                    _emit_ag("b", nbuf)
                    for ci in range(min(half, len(lo_chunks)),
                                    min(LOP, len(lo_chunks))):
                        g, gi = emit_gather("lo", ci, nbuf)
                        np_bufs[ci] = g
                        np_insts.append(gi)
                    pending = (np_bufs, np_insts)

    # Post-scheduling: align each gather's SWDGE queue with its DMASW lane
    # (lanes are assigned round-robin in scheduled order and their sems are
    # queue-locked in ucode, so queue must follow lane, not emission order).
    import re as _re
    for _blk in nc.m.functions[0].blocks:
        for _inst in _blk.instructions:
            if isinstance(_inst, mybir.InstDMAGatherAnt):
                _si = _inst.sync_info
                _lane = None
                for _u in (_si.on_update if _si else []):
                    _m = _re.match(r"DMASW(\d+)_", _u.ant_name or "")
                    if _m:
                        _lane = int(_m.group(1))
                if _lane is not None:
                    _inst.queue_num = _lane % NQ
    nc.compile()
    return nc


_CACHE = {}


def _get_compiled(edge_index, num_iter=NUM_ITER):
    key = (hash(np.asarray(edge_index).tobytes()), num_iter)
    if key not in _CACHE:
        cores = _preprocess(edge_index)
        cores = _equalize(cores)
        nc = _build_uniform(cores[0], num_iter=num_iter)
        _CACHE[key] = (nc, cores)
    return _CACHE[key]


def _make_in_maps(x, atom_emb, Ws, bs, cores_meta):
    x = np.asarray(x)
    emb_pad = np.zeros((NUM_ATOM_FEATS * 128, D), ml_dtypes.bfloat16)
    ae = np.asarray(atom_emb, dtype=np.float32)
    for f in range(NUM_ATOM_FEATS):
        emb_pad[f * 128:f * 128 + ATOM_VOCAB] = ae[f]
    ws_t = np.ascontiguousarray(
        np.asarray(Ws, dtype=np.float32).reshape(NUM_LAYER * D, D))
    bs_t = np.ascontiguousarray(np.asarray(bs, dtype=np.float32))
    ident = np.eye(128, dtype=np.float32)
    identb = np.eye(128, dtype=ml_dtypes.bfloat16)

    in_maps = []
    for c, m in enumerate(cores_meta):
        nr = m["node_of_row"]  # [SHARD_PAD] local node or -1
        # one-hot: [128 vocab-pad, NCOL*9*128] with oh[v, (col,f,n)] = 1
        oh = np.zeros((128, NCOL * NUM_ATOM_FEATS * 128), ml_dtypes.bfloat16)
        xs = x[c * SHARD:(c + 1) * SHARD]  # [SHARD, 9]
        rows = np.arange(SHARD_PAD)
        valid = nr >= 0
        for f in range(NUM_ATOM_FEATS):
            vals = np.zeros(SHARD_PAD, np.int64)
            vals[valid] = xs[nr[valid], f]
            cols = (rows // 128) * NUM_ATOM_FEATS * 128 + f * 128 + rows % 128
            oh[vals[valid], cols[valid]] = 1.0
        lo = m["lo_idx"] if m["n_lo"] else np.zeros(16, np.int64)
        hi_ = m["hi_idx"] if m["n_hi"] else np.zeros(16, np.int64)
        in_maps.append({
            "embtab": emb_pad,
            "oh": np.ascontiguousarray(oh),
            "ws": ws_t,
            "bs": bs_t,
            "ident": ident,
            "identb": identb,
            "selfw": np.ascontiguousarray(m["selfw"]),
            "idx_lo": _wrap_idxs(lo),
            "idx_hi": _wrap_idxs(hi_),
            "s": np.ascontiguousarray(m["s_sb"]),
        })
    return in_maps


def _unpermute(res, cores_meta):
    out = np.zeros((N_NODES, D), np.float32)
    for c, m in enumerate(cores_meta):
        nr = m["node_of_row"]
        valid = nr >= 0
        r = np.asarray(res[c]["out"], dtype=np.float32)
        out[c * SHARD + nr[valid]] = r[valid]
    return out


def kernel(x, edge_index, atom_emb, Ws, bs):
    nc, cores_meta = _get_compiled(edge_index)
    in_maps = _make_in_maps(x, atom_emb, Ws, bs, cores_meta)
    res = run_bass_kernel_spmd(nc, in_maps, core_ids=list(range(NC)))
    return np.ascontiguousarray(_unpermute(res.results, cores_meta))


def run_profiled(x, edge_index, atom_emb, Ws, bs):
    import ntff_hook
    ntff_hook.install()
    nc, cores_meta = _get_compiled(edge_index)
    in_maps = _make_in_maps(x, atom_emb, Ws, bs, cores_meta)
    res = run_bass_kernel_spmd(nc, in_maps, core_ids=list(range(NC)),
                               trace=True)
    return (np.ascontiguousarray(_unpermute(res.results, cores_meta)),
            res.exec_time_ns)


# revision 8
# speedup vs baseline: 1.1703x; 1.0030x over previous
"""APPNP (GCN-normalized propagation) distributed Bass kernel for 8 TRN2 cores.

v2 strategy (dst-sharded message passing, gather-descriptor-optimized):
  - Nodes sharded across 8 cores (6250/core, padded to 6272 = 49*128 rows).
  - Per-core node->row permutation balances per-(block,dom) in-edge counts
    across cores so SPMD equalization padding is minimal.
  - Prologue: atom embedding via one-hot matmuls (stationary = padded
    embedding table, moving = host-built one-hot of x) directly producing
    hT; 3-layer MLP in transposed space; PE transpose back -> h (bf16),
    h0s = 0.1*h.
  - Exchange: two AllGathers per iteration into DRAM tables
    a (rows 0:2560/core -> 20480 rows) and b (rows 2560:6272 -> 29696 rows),
    both < 32768 so int16 gather indices reach everything. a is issued
    mid-iteration (after block-pair 19), b at the end; the next iteration
    issues ~40 a-sourced chunks first so b's wire time is absorbed.
  - Per iteration: dma_gather h[src] for in-edges grouped in 64-wide dst
    blocks (128-slot tiles), TensorEngine segment-sum via one-hot S
    matrices (bf16, SBUF-resident) in PSUM; h0s injected via identity
    matmul; self-loops are NOT slots: the Vector engine evicts PSUM with
    h_new = selfw * h_old + psum in one scalar_tensor_tensor op.
"""

import numpy as np
import ml_dtypes

import concourse.bacc as bacc
import concourse.bass as bass
import concourse.mybir as mybir
import concourse.tile as tile
from concourse.bass_utils import run_bass_kernel_spmd
from concourse.instruction_name_ordered_set import InstructionNameOrderedSet

# Problem constants (must match reference.py)
N_NODES = 50000
N_EDGES = 800000
D = 128
NUM_ITER = 10
NUM_LAYER = 3
ALPHA = 0.1
NUM_ATOM_FEATS = 9
ATOM_VOCAB = 119

NC = 8
SHARD = N_NODES // NC            # 6250
SHARD_PAD = 6272                 # 49 * 128
NCOL = SHARD_PAD // 128          # 49
W = 64                           # dst block width
NBLK = SHARD_PAD // W            # 98
CHUNK = 1024                     # max idxs per dma_gather
COLS_A = 30                      # shard cols in exchange table a
ROWS_A = COLS_A * 128            # 3840
ROWS_B = SHARD_PAD - ROWS_A      # 2432
N_A = NC * ROWS_A                # 30720 (< 32768)
N_B = NC * ROWS_B                # 19456 (< 32768)
NQ = 4                           # SWDGE queues
ABSORB = 0                       # lo chunks issued before first hi chunk
CHAIN = False                    # chain gather emission order

BF16 = mybir.dt.bfloat16
F32 = mybir.dt.float32
I16 = mybir.dt.int16
AF = mybir.ActivationFunctionType


def _wrap_idxs(idx):
    """slot i -> partition i%16 (replicated x8), col i//16."""
    n = idx.shape[0]
    assert n % 16 == 0
    w = idx.reshape(n // 16, 16).T.astype(np.int16)
    return np.ascontiguousarray(np.tile(w, (8, 1)))


def _pad128(a, fill=0):
    n = a.shape[0]
    m = (-n) % 128
    if m == 0:
        return a
    return np.concatenate([a, np.full((m,) + a.shape[1:], fill, a.dtype)])


def _balance_perm(indeg):
    """Greedy LPT: assign 6250 local nodes to 98 blocks of <=64 nodes,
    balancing total in-degree per block. Returns node_of_row[6272] with -1
    for pad rows (all pads in the last block)."""
    import heapq
    order = np.argsort(-indeg, kind="stable")
    cap = np.full(NBLK, 64, np.int64)
    cap[NBLK - 1] = SHARD - 64 * (NBLK - 1)  # 42 real nodes in last block
    fill = [[] for _ in range(NBLK)]
    heap = [(0, b) for b in range(NBLK)]
    heapq.heapify(heap)
    for nid in order:
        while True:
            tot, b = heapq.heappop(heap)
            if len(fill[b]) < cap[b]:
                break
        fill[b].append(nid)
        if len(fill[b]) < cap[b]:
            heapq.heappush(heap, (tot + int(indeg[nid]), b))
    node_of_row = np.full(SHARD_PAD, -1, np.int64)
    for b in range(NBLK):
        for j, nid in enumerate(fill[b]):
            node_of_row[b * 64 + j] = nid
    return node_of_row


def _preprocess(edge_index):
    """Host-side graph preprocessing -> per-core structures."""
    src = np.asarray(edge_index[0], dtype=np.int64)
    dst = np.asarray(edge_index[1], dtype=np.int64)
    deg = np.bincount(dst, minlength=N_NODES).astype(np.float64) + 1.0
    dinv = 1.0 / np.sqrt(deg)
    coef = ((1.0 - ALPHA) * dinv[src] * dinv[dst]).astype(np.float32)
    selfw_g = ((1.0 - ALPHA) * dinv * dinv).astype(np.float32)

    # pass A: per-core balanced permutation (total in-degree)
    indeg_all = np.bincount(dst, minlength=N_NODES)
    node_of_row = np.zeros((NC, SHARD_PAD), np.int64)
    row_of_node = np.zeros(N_NODES, np.int64)  # global node -> local row
    for c in range(NC):
        nr = _balance_perm(indeg_all[c * SHARD:(c + 1) * SHARD])
        node_of_row[c] = nr
        valid = nr >= 0
        row_of_node[c * SHARD + nr[valid]] = np.nonzero(valid)[0]

    # pass B: with src sides fixed by pass A, pack nodes into blocks so each
    # (block, dom) in-edge count lands just UNDER a multiple of 128 (the
    # gather-tile quantum) and aligns across cores. Targets are global (the
    # max core's totals) so SPMD equalization adds almost nothing.
    src_isa = (row_of_node[src] % SHARD_PAD) < ROWS_A
    lo_in = np.bincount(dst[src_isa], minlength=N_NODES)
    hi_in = np.bincount(dst[~src_isa], minlength=N_NODES)
    RESID = 118  # target residue mod 128 (margin 10 to the next tile)

    def _targets(total_max, nb):
        base_q = max(0, int((total_max / nb - RESID) // 128))
        t = np.full(nb, base_q * 128 + RESID, np.float64)
        k = 0
        while t.sum() < total_max and k < nb:
            t[k] += 128
            k += 1
        while t.sum() < total_max:
            t += 128
        return t

    side_meta = []
    for side in (0, 1):
        if side == 0:
            blocks = list(range(0, ROWS_A // W))
        else:
            blocks = list(range(ROWS_A // W, NBLK))
        lmax = hmax = 0.0
        for c in range(NC):
            rows0 = 0 if side == 0 else ROWS_A
            nrows = ROWS_A if side == 0 else ROWS_B
            nodes = node_of_row[c][rows0:rows0 + nrows]
            nodes = nodes[nodes >= 0]
            lmax = max(lmax, lo_in[c * SHARD + nodes].sum())
            hmax = max(hmax, hi_in[c * SHARD + nodes].sum())
        nb = len(blocks)
        side_meta.append((blocks, _targets(lmax, nb), _targets(hmax, nb)))

    for c in range(NC):
        nr_new = np.full(SHARD_PAD, -1, np.int64)
        for side in (0, 1):
            blocks, T_lo, T_hi = side_meta[side]
            rows0 = 0 if side == 0 else ROWS_A
            nrows = ROWS_A if side == 0 else ROWS_B
            old_nodes = node_of_row[c][rows0:rows0 + nrows]
            old_nodes = old_nodes[old_nodes >= 0]
            li = lo_in[c * SHARD + old_nodes].astype(np.float64)
            hi_ = hi_in[c * SHARD + old_nodes].astype(np.float64)
            nb = len(blocks)
            caps = np.array([64 if b != NBLK - 1 else
                             SHARD - 64 * (NBLK - 1) for b in blocks])
            order_n = np.argsort(-(li + hi_), kind="stable")
            cur = np.zeros((nb, 2))
            cnt = np.zeros(nb, np.int64)
            assign = np.zeros(len(old_nodes), np.int64)
            for j in order_n:
                cost = np.maximum((cur[:, 0] + li[j]) / T_lo,
                                  (cur[:, 1] + hi_[j]) / T_hi)
                cost[cnt >= caps] = np.inf
                bsel = int(np.argmin(cost))
                assign[j] = bsel
                cur[bsel, 0] += li[j]
                cur[bsel, 1] += hi_[j]
                cnt[bsel] += 1
            # swap refinement: push overshoot (beyond targets) to zero
            def over(cb):
                return (max(0.0, cb[0]) + max(0.0, cb[1]))
            ex = cur - np.stack([T_lo, T_hi], axis=1)
            rng = np.random.default_rng(c)
            for _ in range(4):
                bad = np.nonzero((ex[:, 0] > 0) | (ex[:, 1] > 0))[0]
                if bad.size == 0:
                    break
                improved = False
                for b1 in bad:
                    js = np.nonzero(assign == b1)[0]
                    cands = rng.permutation(nb)[:20]
                    done = False
                    for b2 in cands:
                        if b2 == b1:
                            continue
                        for j1 in js[np.argsort(-(li[js] + hi_[js]))][:12]:
                            js2 = np.nonzero(assign == b2)[0]
                            if js2.size == 0:
                                continue
                            d1 = np.array([li[j1], hi_[j1]])
                            base = (over(ex[b1]) + over(ex[b2]))
                            d2s = np.stack([li[js2], hi_[js2]], axis=1)
                            nb1 = ex[b1] - d1 + d2s
                            nb2 = ex[b2] + d1 - d2s
                            costs = (np.maximum(nb1, 0).sum(axis=1) +
                                     np.maximum(nb2, 0).sum(axis=1))
                            kk = int(np.argmin(costs))
                            if costs[kk] < base - 0.5:
                                j2 = js2[kk]
                                ex[b1] = nb1[kk]
                                ex[b2] = nb2[kk]
                                cur[b1] += d2s[kk] - d1
                                cur[b2] += d1 - d2s[kk]
                                assign[j1], assign[j2] = b2, b1
                                improved = True
                                done = True
                                break
                        if done:
                            break
                if not improved:
                    break
            for bi, b in enumerate(blocks):
                nodes_b = old_nodes[assign == bi]
                for j2, nid in enumerate(nodes_b):
                    nr_new[b * 64 + j2] = nid
        node_of_row[c] = nr_new
        valid = nr_new >= 0
        row_of_node[c * SHARD + nr_new[valid]] = np.nonzero(valid)[0]

    # edge srow (exchange-table row of the source)
    src_core = src // SHARD
    r = row_of_node[src]
    isa = r < ROWS_A
    srow = np.where(isa, src_core * ROWS_A + r,
                    src_core * ROWS_B + (r - ROWS_A))

    cores = []
    for c in range(NC):
        m = (dst >= c * SHARD) & (dst < (c + 1) * SHARD)
        ldr = row_of_node[dst[m]]  # local row of each in-edge's dst
        lsrow = srow[m]
        lcoef = coef[m]
        lisa = isa[m]
        blk = ldr // W
        off = ldr % W

        streams = {"lo": [], "hi": []}
        s_tiles = []
        tiles_by_block = [[] for _ in range(NBLK)]
        stream_ntiles = {"lo": 0, "hi": 0}
        for b in range(NBLK):
            bm = blk == b
            for dom, dm in (("lo", lisa), ("hi", ~lisa)):
                sel = bm & dm
                n = int(sel.sum())
                if n == 0:
                    continue
                idx = _pad128(lsrow[sel].astype(np.int64))
                cf = _pad128(lcoef[sel])
                of = _pad128(off[sel].astype(np.int64))
                ntile = idx.shape[0] // 128
                for t in range(ntile):
                    s = np.zeros((128, W), np.float32)
                    s[np.arange(128), of[t * 128:(t + 1) * 128]] = \
                        cf[t * 128:(t + 1) * 128]
                    tiles_by_block[b].append((dom, stream_ntiles[dom] + t,
                                              len(s_tiles)))
                    s_tiles.append(s)
                streams[dom].append(idx)
                stream_ntiles[dom] += ntile

        lo_idx = (np.concatenate(streams["lo"]) if streams["lo"]
                  else np.zeros(0, np.int64))
        hi_idx = (np.concatenate(streams["hi"]) if streams["hi"]
                  else np.zeros(0, np.int64))
        s_all = (np.stack(s_tiles) if s_tiles
                 else np.zeros((0, 128, W), np.float32))
        s_sb = np.ascontiguousarray(
            s_all.transpose(1, 0, 2).reshape(128, -1)).astype(ml_dtypes.bfloat16)
        # selfw per row [128, NCOL]
        sw = np.zeros(SHARD_PAD, np.float32)
        nr = node_of_row[c]
        valid = nr >= 0
        sw[valid] = selfw_g[c * SHARD + nr[valid]]
        cores.append(dict(
            lo_idx=lo_idx, hi_idx=hi_idx, s_sb=s_sb,
            tiles_by_block=tiles_by_block,
            n_lo=lo_idx.shape[0], n_hi=hi_idx.shape[0],
            ntiles=len(s_tiles),
            node_of_row=node_of_row[c],
            selfw=np.ascontiguousarray(
                sw.reshape(NCOL, 128).T),  # [128, NCOL]
        ))
    return cores


def _chunks(total):
    out = []
    o = 0
    while o < total:
        n = min(CHUNK, total - o)
        out.append((o, n))
        o += n
    return out


def _equalize(cores_meta):
    """Pad per-block/dom tile counts to the max across cores (SPMD)."""
    cnt = np.zeros((NC, NBLK, 2), np.int64)
    for c, m in enumerate(cores_meta):
        for b in range(NBLK):
            for dom, tpos, sidx in m["tiles_by_block"][b]:
                cnt[c, b, 0 if dom == "lo" else 1] += 1
    mx = cnt.max(axis=0)

    new = []
    for c, m in enumerate(cores_meta):
        lo_parts, hi_parts, s_parts = [], [], []
        tiles_by_block = [[] for _ in range(NBLK)]
        lo_idx, hi_idx = m["lo_idx"], m["hi_idx"]
        s_all = m["s_sb"].reshape(128, -1, W)
        lo_nt, hi_nt = 0, 0
        s_n = 0
        for b in range(NBLK):
            for di, dom in enumerate(("lo", "hi")):
                have = [t for t in m["tiles_by_block"][b] if t[0] == dom]
                need = int(mx[b, di])
                for k in range(need):
                    if k < len(have):
                        _, tpos, sidx = have[k]
                        idx_arr = (lo_idx if dom == "lo" else hi_idx)[
                            tpos * 128:(tpos + 1) * 128]
                        s_mat = s_all[:, sidx, :]
                    else:
                        idx_arr = np.zeros(128, np.int64)
                        s_mat = np.zeros((128, W), ml_dtypes.bfloat16)
                    (lo_parts if dom == "lo" else hi_parts).append(idx_arr)
                    s_parts.append(np.asarray(s_mat))
                    nt = lo_nt if dom == "lo" else hi_nt
                    tiles_by_block[b].append((dom, nt, s_n))
                    s_n += 1
                    if dom == "lo":
                        lo_nt += 1
                    else:
                        hi_nt += 1
        lo_cat = (np.concatenate(lo_parts) if lo_parts
                  else np.zeros(0, np.int64))
        hi_cat = (np.concatenate(hi_parts) if hi_parts
                  else np.zeros(0, np.int64))
        s_cat = (np.stack(s_parts) if s_parts
                 else np.zeros((0, 128, W), ml_dtypes.bfloat16))
        s_sb = np.ascontiguousarray(
            np.asarray(s_cat).transpose(1, 0, 2).reshape(128, -1))
        new.append(dict(
            lo_idx=lo_cat, hi_idx=hi_cat, s_sb=s_sb,
            tiles_by_block=tiles_by_block,
            n_lo=lo_cat.shape[0], n_hi=hi_cat.shape[0], ntiles=s_n,
            node_of_row=m["node_of_row"], selfw=m["selfw"],
        ))
    return new


def _build_uniform(meta0, num_iter=NUM_ITER):
    n_lo, n_hi, ntiles = meta0["n_lo"], meta0["n_hi"], meta0["ntiles"]
    tiles_by_block = meta0["tiles_by_block"]

    nc = bacc.Bacc("TRN2", target_bir_lowering=False, debug=False,
                   num_devices=NC, num_swdge_queues=NQ)

    embtab = nc.dram_tensor("embtab", [NUM_ATOM_FEATS * 128, D], BF16,
                            kind="ExternalInput")
    oh_d = nc.dram_tensor("oh", [128, NCOL * NUM_ATOM_FEATS * 128], BF16,
                          kind="ExternalInput")
    ws = nc.dram_tensor("ws", [NUM_LAYER * D, D], F32, kind="ExternalInput")
    bs = nc.dram_tensor("bs", [NUM_LAYER, D], F32, kind="ExternalInput")
    ident = nc.dram_tensor("ident", [128, 128], F32, kind="ExternalInput")
    identb_d = nc.dram_tensor("identb", [128, 128], BF16,
                              kind="ExternalInput")
    selfw_d = nc.dram_tensor("selfw", [128, NCOL], F32, kind="ExternalInput")
    idx_lo_d = nc.dram_tensor("idx_lo", [128, max(n_lo, 16) // 16], I16,
                              kind="ExternalInput")
    idx_hi_d = nc.dram_tensor("idx_hi", [128, max(n_hi, 16) // 16], I16,
                              kind="ExternalInput")
    s_d = nc.dram_tensor("s", [128, max(ntiles, 1) * W], BF16,
                         kind="ExternalInput")
    out_d = nc.dram_tensor("out", [SHARD_PAD, D], F32, kind="ExternalOutput")

    ag_in_a = [nc.dram_tensor(f"ag_in_a{i}", [ROWS_A, D], BF16,
                              kind="Internal") for i in range(2)]
    ag_in_b = [nc.dram_tensor(f"ag_in_b{i}", [ROWS_B, D], BF16,
                              kind="Internal") for i in range(2)]
    ag_out_a = [nc.dram_tensor(f"ag_out_a{i}", [N_A, D], BF16,
                               kind="Internal", addr_space="Shared")
                for i in range(2)]
    ag_out_b = [nc.dram_tensor(f"ag_out_b{i}", [N_B, D], BF16,
                               kind="Internal", addr_space="Shared")
                for i in range(2)]

    def _emit_ag(which, buf):
        src = (ag_in_a if which == "a" else ag_in_b)[buf]
        dst = (ag_out_a if which == "a" else ag_out_b)[buf]
        return nc.gpsimd.collective_compute(
            "AllGather", mybir.AluOpType.bypass,
            replica_groups=[list(range(NC))],
            ins=[src[:].opt()], outs=[dst[:].opt()])

    lo_chunks = _chunks(n_lo)
    hi_chunks = _chunks(n_hi)

    with tile.TileContext(nc) as tc:
      with tc.tile_pool(name="persist", bufs=1) as persist:
        h_sb = persist.tile([128, NCOL, D], BF16, tag="h")
        h0s = persist.tile([128, NCOL, D], BF16, tag="h0s")
        selfw = persist.tile([128, NCOL], F32, tag="selfw")
        nc.sync.dma_start(selfw[:], selfw_d[:])
        identb = persist.tile([128, 128], BF16, tag="identb")
        nc.sync.dma_start(identb[:], identb_d[:])

        # ---------------- prologue: one-hot embedding + MLP ----------------
        with (
            tc.tile_pool(name="pro", bufs=1) as pro,
            tc.tile_pool(name="mlp", bufs=3) as mlp_pool,
            tc.tile_pool(name="prps", bufs=2, space="PSUM") as prps,
        ):
            emb_sb = pro.tile([128, NUM_ATOM_FEATS, D], BF16, tag="emb")
            nc.sync.dma_start(
                emb_sb[:],
                embtab[:, :].rearrange("(f p) d -> p f d", p=128))
            idn = pro.tile([128, 128], F32, tag="idn")
            nc.sync.dma_start(idn[:], ident[:])
            w_sb = pro.tile([128, NUM_LAYER * D], F32, tag="w")
            b_sb = pro.tile([128, NUM_LAYER], F32, tag="b")
            for l in range(NUM_LAYER):
                nc.sync.dma_start(w_sb[:, l * D:(l + 1) * D],
                                  ws[l * D:(l + 1) * D, :])
                nc.sync.dma_start(b_sb[:, l:l + 1],
                                  bs[l:l + 1, :].rearrange("a k -> k a"))
            oh_sb = pro.tile([128, NCOL * NUM_ATOM_FEATS * 128], BF16,
                             tag="oh")
            for col in range(NCOL):
                o = col * NUM_ATOM_FEATS * 128
                nc.sync.dma_start(oh_sb[:, o:o + NUM_ATOM_FEATS * 128],
                                  oh_d[:, o:o + NUM_ATOM_FEATS * 128])

            for col in range(NCOL):
                o = col * NUM_ATOM_FEATS * 128
                ps = prps.tile([128, 128], F32, tag="ps")
                for f in range(NUM_ATOM_FEATS):
                    nc.tensor.matmul(
                        ps[:], emb_sb[:, f, :],
                        oh_sb[:, o + f * 128:o + (f + 1) * 128],
                        start=(f == 0), stop=(f == NUM_ATOM_FEATS - 1))
                cur = mlp_pool.tile([128, 128], F32, tag="t")
                nc.scalar.activation(cur[:], ps[:], AF.Copy)
                for l in range(NUM_LAYER):
                    ps2 = prps.tile([128, 128], F32, tag="ps2")
                    nc.tensor.matmul(ps2[:], w_sb[:, l * D:(l + 1) * D],
                                     cur[:], start=True, stop=True)
                    cur = mlp_pool.tile([128, 128], F32, tag="t")
                    nc.scalar.activation(
                        cur[:], ps2[:],
                        AF.Relu if l != NUM_LAYER - 1 else AF.Identity,
                        bias=b_sb[:, l:l + 1])
                # transpose back: h [nodes, d]
                pt = prps.tile([128, 128], F32, tag="pt")
                nc.tensor.transpose(pt[:], cur[:], idn[:])
                nc.scalar.activation(h_sb[:, col, :], pt[:], AF.Copy)
                nc.scalar.activation(h0s[:, col, :], pt[:], AF.Copy,
                                     scale=ALPHA)
                if col == COLS_A - 1:
                    nc.sync.dma_start(
                        ag_in_a[0][:].rearrange("(c p) f -> p c f", p=128),
                        h_sb[:, 0:COLS_A, :])
                    _emit_ag("a", 0)
                elif col == NCOL - 1:
                    nc.sync.dma_start(
                        ag_in_b[0][:].rearrange("(c p) f -> p c f", p=128),
                        h_sb[:, COLS_A:NCOL, :])

        # ---------------- main loop ----------------
        with (
            tc.tile_pool(name="sconst", bufs=1) as sconst,
            tc.tile_pool(name="glo", bufs=22) as glo_pool,
            tc.tile_pool(name="ghi", bufs=12) as ghi_pool,
            tc.tile_pool(name="ps", bufs=8, space="PSUM") as ps_pool,
            tc.tile_pool(name="stage", bufs=2) as stage_pool,
        ):
            s_sb = sconst.tile([128, max(ntiles, 1) * W], BF16, tag="s")
            nc.sync.dma_start(s_sb[:], s_d[:])
            ilo = sconst.tile([128, max(n_lo, 16) // 16], I16, tag="ilo")
            nc.sync.dma_start(ilo[:], idx_lo_d[:])
            ihi = sconst.tile([128, max(n_hi, 16) // 16], I16, tag="ihi")
            nc.sync.dma_start(ihi[:], idx_hi_d[:])

            qe_g = [0]
            LOP = 12  # lo chunks of iter t+1 emitted before AG_b(t)

            def emit_gather(dom, ci, buf):
                (o, n) = (lo_chunks if dom == "lo" else hi_chunks)[ci]
                pool = glo_pool if dom == "lo" else ghi_pool
                view = (ag_out_a if dom == "lo" else ag_out_b)[buf][:, :]
                isb = ilo if dom == "lo" else ihi
                g = pool.tile([128, 8, D], BF16, tag="g" + dom)
                gi = nc.gpsimd.dma_gather(
                    g[:, 0:n // 128, :], view,
                    isb[:, o // 16:(o + n) // 16], n, n, D,
                    queue_num=qe_g[0] % NQ)
                qe_g[0] += 1
                return g, gi

            pending = None  # prefix state for the next iteration
            for it in range(num_iter):
                buf = it % 2
                if pending is None:
                    lo_tiles_bufs = {}
                    lo_insts = []
                    for ci in range(min(LOP, len(lo_chunks))):
                        g, gi = emit_gather("lo", ci, buf)
                        lo_tiles_bufs[ci] = g
                        lo_insts.append(gi)
                    # prologue staged table b; trigger its AllGather now so
                    # the wire overlaps the prefix gathers above
                    _emit_ag("b", 0)
                else:
                    lo_tiles_bufs, lo_insts = pending

                hi_tiles_bufs = {}
                gath_insts = []
                hi_insts = []
                order = []
                li, hii = min(LOP, len(lo_chunks)), 0
                while li < len(lo_chunks) or hii < len(hi_chunks):
                    if li < len(lo_chunks):
                        order.append(("lo", li)); li += 1
                    if hii < len(hi_chunks):
                        order.append(("hi", hii)); hii += 1
                for dom, ci in order:
                    g, gi = emit_gather(dom, ci, buf)
                    gath_insts.append(gi)
                    (lo_insts if dom == "lo" else hi_insts).append(gi)
                    (lo_tiles_bufs if dom == "lo" else hi_tiles_bufs)[ci] = g

                # soft absorb: schedule the first hi chunks after a few
                # post-prefix lo chunks so the engine doesn't park on the
                # AG_b wait while runnable lo gathers sit behind it
                for j in range(min(6, len(hi_insts))):
                    anchor = min(LOP + 2 + 2 * j, len(lo_insts) - 1)
                    dd = InstructionNameOrderedSet()
                    dd.add(lo_insts[anchor].ins.name)
                    hi_insts[j].ins.add_nosync_dependencies_from(dd)

                last = it == num_iter - 1
                for p in range(NBLK // 2):
                    col = p
                    psum = ps_pool.tile([128, D], F32, tag="ps")
                    for half in range(2):
                        tl = tiles_by_block[2 * p + half]
                        ph = half * 64
                        nc.tensor.matmul(
                            psum[ph:ph + 64, :],
                            identb[:, ph:ph + 64],
                            h0s[:, col, :],
                            start=True, stop=(len(tl) == 0))
                        for j, (dom, tpos, sidx) in enumerate(tl):
                            bufs = (lo_tiles_bufs if dom == "lo"
                                    else hi_tiles_bufs)
                            g = bufs[tpos // 8]
                            nc.tensor.matmul(
                                psum[ph:ph + 64, :],
                                s_sb[:, sidx * W:(sidx + 1) * W],
                                g[:, tpos % 8, :],
                                start=False, stop=(j == len(tl) - 1))
                    # evict: h_new = selfw * h_old + psum  (one DVE op)
                    if last:
                        st = stage_pool.tile([128, D], F32, tag="st")
                        nc.vector.scalar_tensor_tensor(
                            st[:], h_sb[:, col, :], selfw[:, col:col + 1],
                            psum[:, :], mybir.AluOpType.mult,
                            mybir.AluOpType.add)
                        nc.sync.dma_start(
                            out_d[p * 128:(p + 1) * 128, :], st[:])
                    else:
                        nc.vector.scalar_tensor_tensor(
                            h_sb[:, col, :], h_sb[:, col, :],
                            selfw[:, col:col + 1],
                            psum[:, :], mybir.AluOpType.mult,
                            mybir.AluOpType.add)
                        if p == COLS_A - 1:
                            nc.sync.dma_start(
                                ag_in_a[1 - buf][:].rearrange(
                                    "(c p) f -> p c f", p=128),
                                h_sb[:, 0:COLS_A, :])
                            ag_a = _emit_ag("a", 1 - buf)
                            # pin the AG trigger into the GpSimd gather
                            # stream at ~75% so its wire time overlaps the
                            # remaining gathers (GpSimd is the only engine
                            # that can trigger collectives)
                            gpos = (len(gath_insts) * 3) // 4
                            d1 = InstructionNameOrderedSet()
                            d1.add(gath_insts[gpos].ins.name)
                            ag_a.ins.add_nosync_dependencies_from(d1)
                            d2 = InstructionNameOrderedSet()
                            d2.add(ag_a.ins.name)
                            gath_insts[gpos + 1].ins.add_nosync_dependencies_from(d2)

                # software pipelining: emit the next iteration's first LOP
                # lo gathers (they only need AG_a of this iteration) BEFORE
                # staging/triggering AG_b, so they run during AG_b's wire
                # instead of idling at the iteration boundary.
                if not last:
                    nbuf = 1 - buf
                    np_bufs = {}
                    np_insts = []
                    half = LOP // 2
                    for ci in range(min(half, len(lo_chunks))):
                        g, gi = emit_gather("lo", ci, nbuf)
                        np_bufs[ci] = g
                        np_insts.append(gi)
                    nc.sync.dma_start(
                        ag_in_b[nbuf][:].rearrange(
                            "(c p) f -> p c f", p=128),
                        h_sb[:, COLS_A:NCOL, :])
                    _emit_ag("b", nbuf)
                    for ci in range(min(half, len(lo_chunks)),
                                    min(LOP, len(lo_chunks))):
                        g, gi = emit_gather("lo", ci, nbuf)
                        np_bufs[ci] = g
                        np_insts.append(gi)
                    pending = (np_bufs, np_insts)

    # Post-scheduling: align each gather's SWDGE queue with its DMASW lane
    # (lanes are assigned round-robin in scheduled order and their sems are
    # queue-locked in ucode, so queue must follow lane, not emission order).
    import re as _re
    for _blk in nc.m.functions[0].blocks:
        for _inst in _blk.instructions:
            if isinstance(_inst, mybir.InstDMAGatherAnt):
                _si = _inst.sync_info
                _lane = None
                for _u in (_si.on_update if _si else []):
                    _m = _re.match(r"DMASW(\d+)_", _u.ant_name or "")
                    if _m:
                        _lane = int(_m.group(1))
                if _lane is not None:
                    _inst.queue_num = _lane % NQ
    nc.compile()
    return nc


_CACHE = {}


def _get_compiled(edge_index, num_iter=NUM_ITER):
    key = (hash(np.asarray(edge_index).tobytes()), num_iter)
    if key not in _CACHE:
        cores = _preprocess(edge_index)
        cores = _equalize(cores)
        nc = _build_uniform(cores[0], num_iter=num_iter)
        _CACHE[key] = (nc, cores)
    return _CACHE[key]


def _make_in_maps(x, atom_emb, Ws, bs, cores_meta):
    x = np.asarray(x)
    emb_pad = np.zeros((NUM_ATOM_FEATS * 128, D), ml_dtypes.bfloat16)
    ae = np.asarray(atom_emb, dtype=np.float32)
    for f in range(NUM_ATOM_FEATS):
        emb_pad[f * 128:f * 128 + ATOM_VOCAB] = ae[f]
    ws_t = np.ascontiguousarray(
        np.asarray(Ws, dtype=np.float32).reshape(NUM_LAYER * D, D))
    bs_t = np.ascontiguousarray(np.asarray(bs, dtype=np.float32))
    ident = np.eye(128, dtype=np.float32)
    identb = np.eye(128, dtype=ml_dtypes.bfloat16)

    in_maps = []
    for c, m in enumerate(cores_meta):
        nr = m["node_of_row"]  # [SHARD_PAD] local node or -1
        # one-hot: [128 vocab-pad, NCOL*9*128] with oh[v, (col,f,n)] = 1
        oh = np.zeros((128, NCOL * NUM_ATOM_FEATS * 128), ml_dtypes.bfloat16)
        xs = x[c * SHARD:(c + 1) * SHARD]  # [SHARD, 9]
        rows = np.arange(SHARD_PAD)
        valid = nr >= 0
        for f in range(NUM_ATOM_FEATS):
            vals = np.zeros(SHARD_PAD, np.int64)
            vals[valid] = xs[nr[valid], f]
            cols = (rows // 128) * NUM_ATOM_FEATS * 128 + f * 128 + rows % 128
            oh[vals[valid], cols[valid]] = 1.0
        lo = m["lo_idx"] if m["n_lo"] else np.zeros(16, np.int64)
        hi_ = m["hi_idx"] if m["n_hi"] else np.zeros(16, np.int64)
        in_maps.append({
            "embtab": emb_pad,
            "oh": np.ascontiguousarray(oh),
            "ws": ws_t,
            "bs": bs_t,
            "ident": ident,
            "identb": identb,
            "selfw": np.ascontiguousarray(m["selfw"]),
            "idx_lo": _wrap_idxs(lo),
            "idx_hi": _wrap_idxs(hi_),
            "s": np.ascontiguousarray(m["s_sb"]),
        })
    return in_maps


def _unpermute(res, cores_meta):
    out = np.zeros((N_NODES, D), np.float32)
    for c, m in enumerate(cores_meta):
        nr = m["node_of_row"]
        valid = nr >= 0
        r = np.asarray(res[c]["out"], dtype=np.float32)
        out[c * SHARD + nr[valid]] = r[valid]
    return out


def kernel(x, edge_index, atom_emb, Ws, bs):
    nc, cores_meta = _get_compiled(edge_index)
    in_maps = _make_in_maps(x, atom_emb, Ws, bs, cores_meta)
    res = run_bass_kernel_spmd(nc, in_maps, core_ids=list(range(NC)))
    return np.ascontiguousarray(_unpermute(res.results, cores_meta))


def run_profiled(x, edge_index, atom_emb, Ws, bs):
    import ntff_hook
    ntff_hook.install()
    nc, cores_meta = _get_compiled(edge_index)
    in_maps = _make_in_maps(x, atom_emb, Ws, bs, cores_meta)
    res = run_bass_kernel_spmd(nc, in_maps, core_ids=list(range(NC)),
                               trace=True)
    return (np.ascontiguousarray(_unpermute(res.results, cores_meta)),
            res.exec_time_ns)
